# revision 1
# baseline (speedup 1.0000x reference)
"""Trainium2 Bass kernel for nn_EnhancedTransformerLayer (RoPE attention + MoE).

Sharding: 8 cores; core c -> batch b=c//4, qc=c%4. Four distinct NEFFs (one per
qc), each run on 2 cores (b=0,1). Core qc owns interleaved query blocks
{qc, qc+4, qc+8, qc+12} (4 x 128 tokens) so causal work is balanced, and only
computes K/V up to its last block. Dense 8-expert MoE on its 512 tokens with
top-2 combine. float32r for all matmuls on the routing-sensitive path
(~1.5e-4), bf16 expert weights (value path). Host pre-transposes/permutes.
"""
import sys, os
sys.path.insert(0, '/opt/trn_rl_repo')
import numpy as np
import ml_dtypes

import concourse.bass as bass
from concourse import bacc
import concourse.tile as tile
from concourse import mybir
from concourse.masks import make_identity

R = mybir.dt.float32r
F = mybir.dt.float32
BF = mybir.dt.bfloat16
P = 128
B, S, E, H, D, NE = 2, 2048, 1024, 16, 64, 8
NC = E // P
QL = 512
EXP_SCALE = 1.0 / (D ** 0.5)
LN_EPS = 1e-5

_cache = {}


def _kv_plan(qc):
    """K/V token coverage and span split for this qc."""
    kv_tok = 128 * (qc + 13)            # last interleaved block end
    rem = kv_tok - 1536
    rem = max(rem, 256)                 # fp32r needs N>=256
    kv_tok = 1536 + rem
    spans = [(0, 512), (512, 512), (1024, 512), (1536, rem)]
    return kv_tok, spans


def _build(qc):
    nc = bacc.Bacc("TRN2", target_bir_lowering=False, debug=False, num_devices=8,
                   name=f"moe_qc{qc}", enable_partition_id=False)
    kv_tok, kv_spans = _kv_plan(qc)
    KCN = kv_tok // P                   # key chunk count

    def din(name, shape, dt=R):
        return nc.dram_tensor(name, shape, dt, kind="ExternalInput")

    xt = din("xt", [E, S])
    xtf = din("xtf", [E, S], F)
    xtq = din("xtq", [E, QL], F)
    xres = din("xres", [E, QL])
    wq = din("wq", [E, E]); wk = din("wk", [E, E]); wv = din("wv", [E, E])
    bq = din("bq", [P, NC], F); bk = din("bk", [P, NC], F)
    bvr = din("bvr", [1, E])
    wo = din("wo", [E, E]); bo = din("bo", [P, NC], F)
    gw = din("gw", [E, NE]); gb = din("gb", [NE, 1], F)
    cos2 = din("cos2", [P, S], F); sin2 = din("sin2", [P, S], F)
    cos2q = din("cos2q", [P, QL], F); sin2q = din("sin2q", [P, QL], F)
    trid = din("trid", [P, P])          # tri[k, q] = 1 if q >= k (within a block)
    ew = din("ew", [NE, NC, P, E], BF)
    ebr = din("ebr", [P, NE * NC], F)
    ln1w = din("ln1w", [P, NC], F); ln1b = din("ln1b", [P, NC], F)
    ln2w = din("ln2w", [P, NC], F); ln2b = din("ln2b", [P, NC], F)
    out = nc.dram_tensor("out", [E, QL], R, kind="ExternalOutput")
    nsp = len(kv_spans)
    xrd = [nc.dram_tensor(f"xrd{i}", [E, kv_spans[i][1]], R) for i in range(nsp)]
    vsch = [nc.dram_tensor(f"vsc{i}", [kv_tok, 512], R) for i in range(2)]

    AX = mybir.AxisListType.X
    OP = mybir.AluOpType
    AF = mybir.ActivationFunctionType
    import contextlib

    def ropeF(dst, src, cos_sb, sin_sb, tmppool, width):
        """src/tmps f32 (2x DVE); dst may be f32r (cast on final write)."""
        for c in range(4):
            t1 = tmppool.tile([P, width], F, tag="ropet1")
            t2 = tmppool.tile([P, width], F, tag="ropet2")
            t3 = tmppool.tile([P, width], F, tag="ropet3")
            t4 = tmppool.tile([P, width], F, tag="ropet4")
            nc.vector.tensor_tensor(out=t1[:], in0=src[:, c, :], in1=cos_sb[:], op=OP.mult)
            nc.vector.tensor_tensor(out=t2[:], in0=src[:, c + 4, :], in1=sin_sb[:], op=OP.mult)
            nc.vector.tensor_tensor(out=dst[:, c, :], in0=t1[:], in1=t2[:], op=OP.subtract)
            nc.vector.tensor_tensor(out=t3[:], in0=src[:, c, :], in1=sin_sb[:], op=OP.mult)
            nc.vector.tensor_tensor(out=t4[:], in0=src[:, c + 4, :], in1=cos_sb[:], op=OP.mult)
            nc.vector.tensor_tensor(out=dst[:, c + 4, :], in0=t4[:], in1=t3[:], op=OP.add)

    with tile.TileContext(nc) as tc, \
         nc.allow_low_precision(reason="float32r is bit-identical to float32"), \
         contextlib.ExitStack() as es:
        consts = es.enter_context(tc.tile_pool(name="consts", bufs=1))

        ones_f = consts.tile([P, 1], F, tag="ones_f")
        nc.vector.memset(ones_f[:], 1.0)
        ones = consts.tile([P, 1], R, tag="ones")
        nc.vector.tensor_copy(out=ones[:], in_=ones_f[:])
        ones1_f = consts.tile([1, P], F, tag="ones1_f")
        nc.vector.memset(ones1_f[:], 1.0)
        ones1 = consts.tile([1, P], R, tag="ones1")
        nc.vector.tensor_copy(out=ones1[:], in_=ones1_f[:])
        iden_f = consts.tile([P, P], F, tag="iden_f")
        make_identity(nc, iden_f[:])
        iden = consts.tile([P, P], R, tag="iden")
        nc.vector.tensor_copy(out=iden[:], in_=iden_f[:])
        eps1 = consts.tile([1, 1], F, tag="eps1")
        nc.vector.memset(eps1[:], LN_EPS)
        tri_sb = consts.tile([P, P], R, tag="tri")
        nc.sync.dma_start(tri_sb[:], trid[:])

        bq_sb = consts.tile([P, NC], F, tag="bq"); nc.sync.dma_start(bq_sb[:], bq[:])
        bk_sb = consts.tile([P, NC], F, tag="bk"); nc.sync.dma_start(bk_sb[:], bk[:])
        bv_sb = consts.tile([1, E], R, tag="bv"); nc.sync.dma_start(bv_sb[:], bvr[:])
        bo_sb = consts.tile([P, NC], F, tag="bo"); nc.sync.dma_start(bo_sb[:], bo[:])
        gb_sb = consts.tile([NE, 1], F, tag="gb"); nc.sync.dma_start(gb_sb[:], gb[:])
        gw_sb = consts.tile([P, NC, NE], R, tag="gw")
        nc.sync.dma_start(gw_sb[:], gw.rearrange("(c p) g -> p c g", p=P))
        eb_sb = consts.tile([P, NE * NC], F, tag="eb"); nc.sync.dma_start(eb_sb[:], ebr[:])
        ln_sb = {}
        for nm, t in (("ln1w", ln1w), ("ln1b", ln1b), ("ln2w", ln2w), ("ln2b", ln2b)):
            ln_sb[nm] = consts.tile([P, NC], F, tag=nm, name=nm)
            nc.sync.dma_start(ln_sb[nm][:], t[:])

        persist = es.enter_context(tc.tile_pool(name="persist", bufs=1))
        ctx_sb = persist.tile([P, NC, QL], R, tag="ctx")
        xrqp_cm = tc.tile_pool(name="xrqp", bufs=1)
        xrqp = xrqp_cm.__enter__()
        xrq2 = xrqp.tile([P, NC, QL], R, tag="xrq2")

        # ===== Phase A1: rope (f32 for 2x DVE) + V projection, same scope =====
        xt_r = xt.rearrange("(c p) s -> p c s", p=P)
        xtf_r = xtf.rearrange("(c p) s -> p c s", p=P)
        with tc.tile_pool(name="ropep", bufs=1) as ropep, \
             tc.tile_pool(name="xt2p", bufs=3) as xt2p, \
             tc.tile_pool(name="xrev", bufs=3) as xrev, \
             tc.tile_pool(name="a1tmp", bufs=2) as a1tmp, \
             tc.tile_pool(name="wvp", bufs=1) as wvp, \
             tc.tile_pool(name="xtok", bufs=3) as xtok, \
             tc.tile_pool(name="vev_p", bufs=3) as vev_p, \
             tc.tile_pool(name="vps", bufs=4, space="PSUM") as vps:
            cos_sb = ropep.tile([P, S], F, tag="cos2")
            sin_sb = ropep.tile([P, S], F, tag="sin2")
            nc.sync.dma_start(cos_sb[:], cos2[:])
            nc.sync.dma_start(sin_sb[:], sin2[:])

            # q-chunk rope first (feeds Q projection early)
            xtq_sb = ropep.tile([P, NC, QL], F, tag="xtq")
            nc.sync.dma_start(xtq_sb[:], xtq.rearrange("(c p) q -> p c q", p=P))
            cosq_sb = ropep.tile([P, QL], F, tag="cosq")
            sinq_sb = ropep.tile([P, QL], F, tag="sinq")
            nc.sync.dma_start(cosq_sb[:], cos2q[:])
            nc.sync.dma_start(sinq_sb[:], sin2q[:])
            ropeF(xrq2, xtq_sb, cosq_sb, sinq_sb, a1tmp, QL)

            # main rope over kv spans -> xrd[sp]
            for sp, (s0, sl) in enumerate(kv_spans):
                ssl = slice(s0, s0 + sl)
                xrd_r = xrd[sp].rearrange("(c p) s -> p c s", p=P)
                for c in range(4):
                    xt2 = xt2p.tile([P, 2, 512], F, tag="xt2")
                    nc.sync.dma_start(xt2[:, 0, :sl], xtf_r[:, c, ssl])
                    nc.sync.dma_start(xt2[:, 1, :sl], xtf_r[:, c + 4, ssl])
                    t1 = a1tmp.tile([P, 512], F, tag="ropet1")
                    t2 = a1tmp.tile([P, 512], F, tag="ropet2")
                    t3 = a1tmp.tile([P, 512], F, tag="ropet3")
                    t4 = a1tmp.tile([P, 512], F, tag="ropet4")
                    xo = xrev.tile([P, 2, 512], R, tag="xo")
                    cs = cos_sb[:, ssl]; sn = sin_sb[:, ssl]
                    nc.vector.tensor_tensor(out=t1[:, :sl], in0=xt2[:, 0, :sl], in1=cs, op=OP.mult)
                    nc.vector.tensor_tensor(out=t2[:, :sl], in0=xt2[:, 1, :sl], in1=sn, op=OP.mult)
                    nc.vector.tensor_tensor(out=xo[:, 0, :sl], in0=t1[:, :sl], in1=t2[:, :sl], op=OP.subtract)
                    nc.vector.tensor_tensor(out=t3[:, :sl], in0=xt2[:, 0, :sl], in1=sn, op=OP.mult)
                    nc.vector.tensor_tensor(out=t4[:, :sl], in0=xt2[:, 1, :sl], in1=cs, op=OP.mult)
                    nc.vector.tensor_tensor(out=xo[:, 1, :sl], in0=t4[:, :sl], in1=t3[:, :sl], op=OP.add)
                    nc.sync.dma_start(xrd_r[:, c, :], xo[:, 0, :sl])
                    nc.sync.dma_start(xrd_r[:, c + 4, :], xo[:, 1, :sl])

            # V projection (reads xt in token blocks; runs alongside rope)
            wv_sb = wvp.tile([P, NC, E], R, tag="wv")
            for c in range(NC):
                nc.sync.dma_start(wv_sb[:, c, :],
                                  wv.rearrange("(c p) m -> p c m", p=P)[:, c, :])
            for dvs in range(2):
                for tkc in range(KCN):
                    xt_tok = xtok.tile([P, NC, P], R, tag="xt_tok")
                    nc.sync.dma_start(xt_tok[:], xt_r[:, :, tkc * P:(tkc + 1) * P])
                    vp = vps.tile([P, 512], F, tag="vps")
                    for dc in range(NC):
                        nc.tensor.matmul(
                            vp[:], xt_tok[:, dc, :],
                            wv_sb[:, dc, dvs * 512:(dvs + 1) * 512],
                            start=(dc == 0), stop=False)
                    nc.tensor.matmul(
                        vp[:], ones1[:, :], bv_sb[:, dvs * 512:(dvs + 1) * 512],
                        start=False, stop=True)
                    vev = vev_p.tile([P, 512], R, tag="vev")
                    nc.scalar.copy(out=vev[:], in_=vp[:])
                    nc.sync.dma_start(vsch[dvs][tkc * P:(tkc + 1) * P, :], vev[:])

        # ===== Phase A2: Q then K projections (kT split per dout chunk) =====
        attn_cm = tc.tile_pool(name="attn_res", bufs=1)
        attn_res = attn_cm.__enter__()
        qT = attn_res.tile([P, NC, QL], R, tag="qT")
        kTs = [attn_res.tile([P, kv_tok], R, tag=f"kT{oc}", name=f"kT{oc}")
               for oc in range(NC)]
        with tc.tile_pool(name="wqp", bufs=1) as wqp, \
             tc.tile_pool(name="qps_p", bufs=4, space="PSUM") as qps_p:
            wq_sb = wqp.tile([P, NC, E], R, tag="wq_sb")
            for c in range(NC):
                nc.sync.dma_start(wq_sb[:, c, :],
                                  wq.rearrange("(c p) m -> p c m", p=P)[:, c, :])
            for oc in range(NC):
                qp = qps_p.tile([P, 512], F, tag="qps")
                for dc in range(NC):
                    nc.tensor.matmul(
                        qp[:], wq_sb[:, dc, oc * P:(oc + 1) * P], xrq2[:, dc, :],
                        start=(dc == 0), stop=(dc == NC - 1))
                nc.vector.tensor_scalar(
                    out=qT[:, oc, :], in0=qp[:],
                    scalar1=bq_sb[:, oc:oc + 1], scalar2=None, op0=OP.add)
        with tc.tile_pool(name="wkp", bufs=1) as wkp, \
             tc.tile_pool(name="xrsp", bufs=3) as xrsp, \
             tc.tile_pool(name="kqps", bufs=4, space="PSUM") as kqps:
            wk_sb = wkp.tile([P, NC, E], R, tag="wk_sb")
            for c in range(NC):
                nc.sync.dma_start(wk_sb[:, c, :],
                                  wk.rearrange("(c p) m -> p c m", p=P)[:, c, :])
            halves = []
            for (s0, sl) in kv_spans:
                halves.append((s0, min(256, sl)))
                if sl > 256:
                    halves.append((s0 + 256, sl - 256))
            for hi, (h0, hl) in enumerate(halves):
                xr_sp = xrsp.tile([P, NC, 256], R, tag="xr_sp")
                # locate source span
                for spi, (s0, sl) in enumerate(kv_spans):
                    if s0 <= h0 < s0 + sl:
                        break
                nc.sync.dma_start(
                    xr_sp[:, :, :hl],
                    xrd[spi].rearrange("(c p) s -> p c s", p=P)[:, :, h0 - s0:h0 - s0 + hl])
                for oc in range(NC):
                    kp = kqps.tile([P, 256], F, tag="kps")
                    for dc in range(NC):
                        nc.tensor.matmul(
                            kp[:, :hl], wk_sb[:, dc, oc * P:(oc + 1) * P], xr_sp[:, dc, :hl],
                            start=(dc == 0), stop=(dc == NC - 1))
                    nc.vector.tensor_scalar(
                        out=kTs[oc][:, h0:h0 + hl], in0=kp[:, :hl],
                        scalar1=bk_sb[:, oc:oc + 1], scalar2=None, op0=OP.add)

        # ===== Phase B: causal attention over interleaved blocks =====
        with tc.tile_pool(name="bt", bufs=6) as bt, \
             tc.tile_pool(name="vt_p", bufs=2) as vt_p, \
             tc.tile_pool(name="scps", bufs=3, space="PSUM") as scps, \
             tc.tile_pool(name="ctxps", bufs=2, space="PSUM") as ctxps, \
             tc.tile_pool(name="rowps", bufs=2, space="PSUM") as rowps:
            for hp in range(NC):
                vt = vt_p.tile([P, KCN, P], R, tag="vt")
                for dvs in range(2):
                    nc.sync.dma_start(
                        vt[:, :, dvs * 64:(dvs + 1) * 64],
                        vsch[hp // 4].rearrange("(k p) d -> p k d", p=P)
                        [:, :, (hp % 4) * P + dvs * 64:(hp % 4) * P + (dvs + 1) * 64])
                ctxp2 = [ctxps.tile([64, QL], F, tag="ctxps", name=f"ctxp_{hp}_{hh}")
                         for hh in range(2)]
                denp2 = [rowps.tile([1, QL], F, tag="denps", name=f"denp_{hp}_{hh}")
                         for hh in range(2)]
                kcs = [kc for kc in range(KCN) if -(-(kc - qc) // 4) < 4]
                for kc in kcs:
                    j0 = max(0, -(-(kc - qc) // 4))
                    q0 = j0 * P
                    for hh in range(2):
                        scp = scps.tile([P, QL], F, tag="scps")
                        nc.tensor.matmul(
                            scp[:, q0:], kTs[hp][hh * 64:(hh + 1) * 64, kc * P:(kc + 1) * P],
                            qT[hh * 64:(hh + 1) * 64, hp, q0:], start=True, stop=True)
                        st = bt.tile([P, QL], R, tag="st")
                        nc.scalar.activation(out=st[:, q0:], in_=scp[:, q0:],
                                             func=AF.Exp, scale=EXP_SCALE)
                        if kc >= qc and (kc - qc) % 4 == 0:
                            j = (kc - qc) // 4
                            nc.vector.tensor_tensor(
                                out=st[:, j * P:(j + 1) * P], in0=st[:, j * P:(j + 1) * P],
                                in1=tri_sb[:], op=OP.mult)
                        nc.tensor.matmul(denp2[hh][:, q0:], ones[:], st[:, q0:],
                                         start=(kc == kcs[0]), stop=(kc == kcs[-1]))
                        nc.tensor.matmul(ctxp2[hh][:, q0:],
                                         vt[:, kc, hh * 64:(hh + 1) * 64], st[:, q0:],
                                         start=(kc == kcs[0]), stop=(kc == kcs[-1]))
                for hh in range(2):
                    rden = bt.tile([1, QL], R, tag="rden")
                    nc.vector.reciprocal(out=rden[:], in_=denp2[hh][:])
                    rbp = scps.tile([P, QL], F, tag="scps")
                    nc.tensor.matmul(rbp[0:64, :], ones1[:, 0:64], rden[:], start=True, stop=True)
                    rb_sb = bt.tile([64, QL], R, tag="rb_sb")
                    nc.scalar.copy(out=rb_sb[:], in_=rbp[0:64, :])
                    nc.vector.tensor_tensor(
                        out=ctx_sb[hh * 64:(hh + 1) * 64, hp, :],
                        in0=ctxp2[hh][:], in1=rb_sb[:], op=OP.mult)
        attn_cm.__exit__(None, None, None)
        xrqp_cm.__exit__(None, None, None)

        # ===== LN helper =====
        def layernorm(src, dst, wtile, btile, tmp, ps_row, ps_big):
            sp_ = ps_row.tile([1, QL], F, tag="lnsum")
            for c in range(NC):
                nc.tensor.matmul(sp_[:], ones[:], src[:, c, :],
                                 start=(c == 0), stop=(c == NC - 1))
            s2p = ps_row.tile([1, QL], F, tag="lnsum2")
            for c in range(NC):
                sq = tmp.tile([P, QL], R, tag="lnsq")
                nc.scalar.activation(out=sq[:], in_=src[:, c, :], func=AF.Square)
                nc.tensor.matmul(s2p[:], ones[:], sq[:],
                                 start=(c == 0), stop=(c == NC - 1))
            mean = tmp.tile([1, QL], R, tag="lnmean")
            nc.scalar.mul(out=mean[:], in_=sp_[:], mul=1.0 / E)
            msq = tmp.tile([1, QL], R, tag="lnmsq")
            nc.scalar.mul(out=msq[:], in_=s2p[:], mul=1.0 / E)
            var = tmp.tile([1, QL], R, tag="lnvar")
            nc.vector.tensor_tensor(out=var[:], in0=mean[:], in1=mean[:], op=OP.mult)
            nc.vector.tensor_tensor(out=var[:], in0=msq[:], in1=var[:], op=OP.subtract)
            std = tmp.tile([1, QL], R, tag="lnstd")
            nc.scalar.activation(out=std[:], in_=var[:], func=AF.Sqrt, bias=eps1[:])
            rstd = tmp.tile([1, QL], R, tag="lnrstd")
            nc.vector.reciprocal(out=rstd[:], in_=std[:])
            mb = ps_big.tile([P, QL], F, tag="bigc")
            nc.tensor.matmul(mb[:], ones1[:], mean[:], start=True, stop=True)
            rb = ps_big.tile([P, QL], F, tag="bigc")
            nc.tensor.matmul(rb[:], ones1[:], rstd[:], start=True, stop=True)
            mbs = tmp.tile([P, QL], R, tag="lnmbs")
            nc.scalar.copy(out=mbs[:], in_=mb[:])
            rbs = tmp.tile([P, QL], R, tag="lnrbs")
            nc.scalar.copy(out=rbs[:], in_=rb[:])
            for c in range(NC):
                t = tmp.tile([P, QL], R, tag="lnt")
                nc.vector.tensor_tensor(out=t[:], in0=src[:, c, :], in1=mbs[:], op=OP.subtract)
                nc.vector.tensor_tensor(out=t[:], in0=t[:], in1=rbs[:], op=OP.mult)
                nc.vector.tensor_scalar(out=dst[:, c, :], in0=t[:],
                                        scalar1=wtile[:, c:c + 1], scalar2=btile[:, c:c + 1],
                                        op0=OP.mult, op1=OP.add)

        # ===== Phase C: out-proj + LN1 + gates =====
        cres = es.enter_context(tc.tile_pool(name="cres", bufs=1))
        xres_sb = cres.tile([P, NC, QL], R, tag="xres")
        nc.sync.dma_start(xres_sb[:], xres.rearrange("(c p) q -> p c q", p=P))
        x1 = cres.tile([P, NC, QL], R, tag="x1")
        wbc = cres.tile([P, NE, QL], R, tag="wbc")
        x1b = cres.tile([P, NC, QL], BF, tag="x1b")
        wrow = cres.tile([1, NE, QL], R, tag="wrow")
        with tc.tile_pool(name="cslab", bufs=2) as cslab, \
             tc.tile_pool(name="ct", bufs=2) as ct, \
             tc.tile_pool(name="h1p", bufs=1) as h1p, \
             tc.tile_pool(name="cps", bufs=3, space="PSUM") as cps, \
             tc.tile_pool(name="crow", bufs=1, space="PSUM") as crow, \
             tc.tile_pool(name="cg", bufs=1, space="PSUM") as cg:
            h1 = h1p.tile([P, NC, QL], R, tag="h1")
            for oc in range(NC):
                wo_sl = cslab.tile([P, NC, P], R, tag="wo_sl")
                nc.sync.dma_start(
                    wo_sl[:], wo.rearrange("(c p) m -> p c m", p=P)[:, :, oc * P:(oc + 1) * P])
                ap = cps.tile([P, QL], F, tag="bigc")
                for dc in range(NC):
                    nc.tensor.matmul(ap[:], wo_sl[:, dc, :], ctx_sb[:, dc, :],
                                     start=(dc == 0), stop=(dc == NC - 1))
                nc.vector.tensor_scalar(out=h1[:, oc, :], in0=ap[:],
                                        scalar1=bo_sb[:, oc:oc + 1], scalar2=None, op0=OP.add)
                nc.vector.tensor_tensor(out=h1[:, oc, :], in0=h1[:, oc, :],
                                        in1=xres_sb[:, oc, :], op=OP.add)

            layernorm(h1, x1, ln_sb["ln1w"], ln_sb["ln1b"], ct, crow, cps)

            gp = cg.tile([NE, QL], F, tag="gps")
            for c in range(NC):
                nc.tensor.matmul(gp[:], gw_sb[:, c, :], x1[:, c, :],
                                 start=(c == 0), stop=(c == NC - 1))
            glog = ct.tile([NE, QL], R, tag="glog")
            nc.vector.tensor_scalar(out=glog[:], in0=gp[:], scalar1=gb_sb[:],
                                    scalar2=None, op0=OP.add)
            gexp = ct.tile([NE, QL], R, tag="gexp")
            nc.scalar.activation(out=gexp[:], in_=glog[:], func=AF.Exp)
            dgp = crow.tile([1, QL], F, tag="lnsum")
            nc.tensor.matmul(dgp[:], ones[0:NE, :], gexp[:], start=True, stop=True)
            rg = ct.tile([1, QL], R, tag="rg")
            nc.vector.reciprocal(out=rg[:], in_=dgp[:])

            gt = ct.tile([P, 4, NE], R, tag="gt")
            for qb in range(4):
                gtp = cg.tile([P, NE], R, tag="gmix")
                nc.tensor.transpose(gtp[:], gexp[:, qb * P:(qb + 1) * P], iden[0:NE, 0:NE])
                nc.vector.tensor_copy(out=gt[:, qb, :], in_=gtp[:])
            m1 = ct.tile([P, 4], F, tag="m1")
            nc.vector.reduce_max(out=m1[:], in_=gt[:], axis=AX)
            msel = ct.tile([P, 4, NE], R, tag="msel")
            g2 = ct.tile([P, 4, NE], R, tag="g2")
            for qb in range(4):
                nc.vector.tensor_scalar(out=msel[:, qb, :], in0=gt[:, qb, :],
                                        scalar1=m1[:, qb:qb + 1], scalar2=None, op0=OP.is_equal)
            nc.vector.tensor_tensor(out=g2[:], in0=gt[:], in1=msel[:], op=OP.mult)
            nc.vector.tensor_tensor(out=g2[:], in0=gt[:], in1=g2[:], op=OP.subtract)
            m2 = ct.tile([P, 4], F, tag="m2")
            nc.vector.reduce_max(out=m2[:], in_=g2[:], axis=AX)
            msel2 = ct.tile([P, 4, NE], R, tag="msel2")
            for qb in range(4):
                nc.vector.tensor_scalar(out=msel2[:, qb, :], in0=g2[:, qb, :],
                                        scalar1=m2[:, qb:qb + 1], scalar2=None, op0=OP.is_equal)
            nc.vector.tensor_tensor(out=msel[:], in0=msel[:], in1=msel2[:], op=OP.add)
            wsel = ct.tile([P, 4, NE], R, tag="wsel")
            nc.vector.tensor_tensor(out=wsel[:], in0=gt[:], in1=msel[:], op=OP.mult)

            for qb in range(4):
                for e in range(NE):
                    wtp = cg.tile([1, P], R, tag="gmix")
                    nc.tensor.transpose(wtp[:], wsel[:, qb, e:e + 1], iden[:])
                    nc.vector.tensor_copy(out=wrow[:, e, qb * P:(qb + 1) * P], in_=wtp[:])
            for e in range(NE):
                nc.vector.tensor_tensor(out=wrow[:, e, :], in0=wrow[:, e, :],
                                        in1=rg[:], op=OP.mult)
                bcp = cps.tile([P, QL], F, tag="bigc")
                nc.tensor.matmul(bcp[:], ones1[:], wrow[:, e, :], start=True, stop=True)
                nc.vector.tensor_copy(out=wbc[:, e, :], in_=bcp[:])

            for c in range(NC):
                nc.gpsimd.tensor_copy(out=x1b[:, c, :], in_=x1[:, c, :])

        # ===== Phase D: dense MoE =====
        moe = cres.tile([P, NC, QL], R, tag="moe")
        with tc.tile_pool(name="ewp", bufs=2) as ewp, \
             tc.tile_pool(name="dt", bufs=3) as dt_, \
             tc.tile_pool(name="dps", bufs=4, space="PSUM") as dps:
            for e in range(NE):
                ew_sl = ewp.tile([P, NC, E], BF, tag="ew_sl")
                nc.sync.dma_start(ew_sl[:], ew[e].rearrange("c p d -> p c d"))
                for oc in range(NC):
                    yp = dps.tile([P, QL], F, tag="yps")
                    for dc in range(NC):
                        nc.tensor.matmul(
                            yp[:], ew_sl[:, dc, oc * P:(oc + 1) * P], x1b[:, dc, :],
                            start=(dc == 0), stop=(dc == NC - 1))
                    t = dt_.tile([P, QL], R, tag="moet")
                    nc.scalar.activation(out=t[:], in_=yp[:], func=AF.Identity,
                                         bias=eb_sb[:, e * NC + oc:e * NC + oc + 1])
                    nc.vector.tensor_tensor(out=t[:], in0=t[:], in1=wbc[:, e, :], op=OP.mult)
                    if e == 0:
                        nc.gpsimd.tensor_copy(out=moe[:, oc, :], in_=t[:])
                    else:
                        nc.gpsimd.tensor_tensor(out=moe[:, oc, :], in0=moe[:, oc, :],
                                                in1=t[:], op=OP.add)

        # ===== Phase E: residual + LN2 + store =====
        with tc.tile_pool(name="et", bufs=3) as et, \
             tc.tile_pool(name="eps_", bufs=2, space="PSUM") as eps_, \
             tc.tile_pool(name="erow", bufs=1, space="PSUM") as erow:
            for c in range(NC):
                nc.vector.tensor_tensor(out=moe[:, c, :], in0=x1[:, c, :],
                                        in1=moe[:, c, :], op=OP.add)
            layernorm(moe, moe, ln_sb["ln2w"], ln_sb["ln2b"], et, erow, eps_)
            for c in range(NC):
                nc.sync.dma_start(out.rearrange("(c p) q -> p c q", p=P)[:, c, :], moe[:, c, :])

    nc.compile()
    return nc


def _prep_inputs(inputs):
    x = np.asarray(inputs['x'], dtype=np.float32)
    ipw = np.asarray(inputs['in_proj_w'], dtype=np.float32)
    ipb = np.asarray(inputs['in_proj_b'], dtype=np.float32)
    opw = np.asarray(inputs['out_proj_w'], dtype=np.float32)
    opb = np.asarray(inputs['out_proj_b'], dtype=np.float32)
    gww = np.asarray(inputs['gate_w'], dtype=np.float32)
    gbb = np.asarray(inputs['gate_b'], dtype=np.float32)
    eww = np.asarray(inputs['expert_w'], dtype=np.float32)
    ebb = np.asarray(inputs['expert_b'], dtype=np.float32)

    perm = np.empty(E, dtype=np.int64)
    idx = 0
    for h in range(H):
        for i in range(D // 2):
            perm[idx] = 64 * h + 2 * i; idx += 1
    for h in range(H):
        for i in range(D // 2):
            perm[idx] = 64 * h + 2 * i + 1; idx += 1

    Wq, Wk, Wv = ipw[0:E], ipw[E:2 * E], ipw[2 * E:3 * E]
    bq_, bk_, bv_ = ipb[0:E], ipb[E:2 * E], ipb[2 * E:3 * E]
    common = {
        "wq": np.ascontiguousarray(Wq[:, perm].T),
        "wk": np.ascontiguousarray(Wk[:, perm].T),
        "wv": np.ascontiguousarray(Wv[:, perm].T),
        "bq": np.ascontiguousarray(bq_.reshape(NC, P).T),
        "bk": np.ascontiguousarray(bk_.reshape(NC, P).T),
        "bvr": bv_.reshape(1, E).copy(),
        "wo": np.ascontiguousarray(opw.T),
        "bo": np.ascontiguousarray(opb.reshape(NC, P).T),
        "gw": np.ascontiguousarray(gww.T),
        "gb": gbb.reshape(NE, 1).copy(),
        "ew": np.ascontiguousarray(
            eww.transpose(0, 2, 1).reshape(NE, NC, P, E)).astype(ml_dtypes.bfloat16),
        "ebr": np.ascontiguousarray(ebb.reshape(NE, NC, P).transpose(2, 0, 1).reshape(P, NE * NC)),
        "ln1w": np.ascontiguousarray(np.asarray(inputs['ln1_w'], np.float32).reshape(NC, P).T),
        "ln1b": np.ascontiguousarray(np.asarray(inputs['ln1_b'], np.float32).reshape(NC, P).T),
        "ln2w": np.ascontiguousarray(np.asarray(inputs['ln2_w'], np.float32).reshape(NC, P).T),
        "ln2b": np.ascontiguousarray(np.asarray(inputs['ln2_b'], np.float32).reshape(NC, P).T),
        "trid": np.ascontiguousarray(
            (np.arange(P)[None, :] >= np.arange(P)[:, None]).astype(np.float32)),
    }
    inv_freq = 1.0 / (10000.0 ** (np.arange(0, D, 2, dtype=np.float64) / D))
    freqs = np.arange(S, dtype=np.float64)[:, None] * inv_freq[None, :]
    cos_t = np.cos(freqs).T.astype(np.float32)
    sin_t = np.sin(freqs).T.astype(np.float32)
    cos2 = np.ascontiguousarray(np.tile(cos_t, (4, 1)))
    sin2 = np.ascontiguousarray(np.tile(sin_t, (4, 1)))
    common["cos2"] = cos2
    common["sin2"] = sin2

    in_maps = []
    for c in range(8):
        b, qc = c // 4, c % 4
        blocks = [qc + 4 * i for i in range(4)]
        cols = np.concatenate([np.arange(blk * P, (blk + 1) * P) for blk in blocks])
        xtb = np.ascontiguousarray(x[b].T)
        xtp = np.ascontiguousarray(xtb[perm])
        m = dict(common)
        m["xt"] = xtp
        m["xtf"] = xtp
        m["xtq"] = np.ascontiguousarray(xtp[:, cols])
        m["xres"] = np.ascontiguousarray(xtb[:, cols])
        m["cos2q"] = np.ascontiguousarray(cos2[:, cols])
        m["sin2q"] = np.ascontiguousarray(sin2[:, cols])
        in_maps.append(m)
    return in_maps


def _run_multi(ncs, in_maps):
    """Run the 4 NEFFs concurrently: graph qc on devices {qc, qc+4} (b=0,1)."""
    import jax
    from jax.sharding import Mesh, PartitionSpec
    from jax.experimental.shard_map import shard_map
    from concourse import bass2jax
    from concourse import mybir as _mb

    bass2jax.install_neuronx_cc_hook()
    devices = jax.devices()

    if "jits" not in _cache:
        _cache["jits"] = {}
    handles = []
    for qc in range(4):
        nc = ncs[qc]
        if qc not in _cache["jits"]:
            in_names, out_names, out_avals, zero_outs = [], [], [], []
            for alloc in nc.m.functions[0].allocations:
                if not isinstance(alloc, _mb.MemoryLocationSet):
                    continue
                name = alloc.memorylocations[0].name
                if alloc.kind == "ExternalInput":
                    in_names.append(name)
                elif alloc.kind == "ExternalOutput":
                    out_names.append(name)
                    shape = tuple(alloc.tensor_shape)
                    dtype = _mb.dt.np(alloc.dtype)
                    out_avals.append(jax.core.ShapedArray(shape, dtype))
                    zero_outs.append(np.zeros(shape, dtype))
            n_params = len(in_names)
            all_names = in_names + out_names
            donate = tuple(range(n_params, n_params + len(out_names)))

            def _body(*args, _nc=nc, _avals=tuple(out_avals), _all=tuple(all_names),
                      _outs=tuple(out_names)):
                outs = bass2jax._bass_exec_p.bind(
                    *args, out_avals=_avals, in_names=_all, out_names=_outs,
                    lowering_input_output_aliases=(),
                    sim_require_finite=True, sim_require_nnan=True, nc=_nc)
                return tuple(outs)

            devs = [devices[qc], devices[qc + 4]]
            mesh = Mesh(np.asarray(devs), ("core",))
            nio = n_params + len(zero_outs)
            sharded = jax.jit(
                shard_map(_body, mesh=mesh,
                          in_specs=(PartitionSpec("core"),) * nio,
                          out_specs=(PartitionSpec("core"),) * len(out_names),
                          check_rep=False),
                donate_argnums=donate, keep_unused=True)
            _cache["jits"][qc] = (sharded, in_names, out_names, zero_outs)
        sharded, in_names, out_names, zero_outs = _cache["jits"][qc]
        per_core = [[np.asarray(in_maps[b * 4 + qc][n]) for n in in_names] for b in range(2)]
        concat_in = [np.concatenate([per_core[b][i] for b in range(2)], axis=0)
                     for i in range(len(in_names))]
        concat_zero = [np.concatenate([z, z], axis=0) for z in zero_outs]
        handles.append((sharded, concat_in, concat_zero, out_names))

    outs = []
    for sharded, concat_in, concat_zero, out_names in handles:
        outs.append((sharded(*concat_in, *concat_zero), out_names))
    results = [None] * 8
    for qc, (arrs, out_names) in enumerate(outs):
        arrs = [np.asarray(a) for a in arrs]
        for b in range(2):
            rm = {}
            for i, n in enumerate(out_names):
                full = arrs[i]
                half = full.shape[0] // 2
                rm[n] = full[b * half:(b + 1) * half]
            results[b * 4 + qc] = rm
    return results


def _ensure_ntff_hook():
    import types
    try:
        from antenv.axon_hooks import get_axon_ntff_profile_hook  # noqa
        return True
    except ImportError:
        pass
    try:
        import antenv
        sys.path.insert(0, '/root/.axon_site')
        from trn_agent_boot.trn_boot import _ntff_profile_via_ctypes
        hook = _ntff_profile_via_ctypes('/opt/axon/libaxon_pjrt.so')
        if hook is None:
            return False
        mod = types.ModuleType('antenv.axon_hooks')
        _state = {'hook': hook}
        mod.set_axon_ntff_profile_hook = lambda h: _state.__setitem__('hook', h)
        mod.get_axon_ntff_profile_hook = lambda: _state['hook']
        sys.modules['antenv.axon_hooks'] = mod
        antenv.axon_hooks = mod
        return True
    except Exception as e:
        print(f"ntff hook setup failed: {e}")
        return False


def kernel(**inputs):
    if "ncs" not in _cache:
        _cache["ncs"] = [_build(qc) for qc in range(4)]
    ncs = _cache["ncs"]
    in_maps = _prep_inputs(inputs)

    trace = bool(int(os.environ.get("KERNEL_TRACE", "0")))
    if trace and _ensure_ntff_hook():
        import tempfile
        from antenv.axon_hooks import get_axon_ntff_profile_hook
        hook = get_axon_ntff_profile_hook()
        tmpdir = tempfile.mkdtemp()
        _run_multi(ncs, in_maps)  # warm-up/compile outside the profile window
        with hook(tmpdir, list(range(8))):
            results = _run_multi(ncs, in_maps)
        _cache["ntff_dir"] = tmpdir
        print(f"ntff dir: {tmpdir}")
    else:
        results = _run_multi(ncs, in_maps)
    _cache["last_results"] = results

    out = np.empty((B, S, E), dtype=np.float32)
    for c in range(8):
        b, qc = c // 4, c % 4
        o = results[c]["out"]  # [E, QL]
        for i in range(4):
            blk = qc + 4 * i
            out[b, blk * P:(blk + 1) * P, :] = o[:, i * P:(i + 1) * P].T
    return out



# revision 36
# speedup vs baseline: 1.1287x; 1.1287x over previous
"""Trainium2 Bass kernel for nn_EnhancedTransformerLayer (RoPE attention + MoE).

Sharding: 8 cores; core c -> batch b=c//4, qc=c%4. Four distinct NEFFs (one per
qc), each run on 2 cores (b=0,1). Core qc owns interleaved query blocks
{qc, qc+4, qc+8, qc+12} (4 x 128 tokens) so causal work is balanced, and only
computes K/V up to its last block.

v2: fused rope/V/K span pipeline (single x read, in-place rope, no DRAM
round-trip), softmax denominator via ones-column in the ctx stationary,
fp32r matmuls clamped to N>=256, MoE accumulated directly in PSUM across
experts with gate-scaled inputs (no gpsimd, no per-expert combine).
"""
import sys, os
sys.path.insert(0, '/opt/trn_rl_repo')
import numpy as np
import ml_dtypes

import concourse.bass as bass
from concourse import bacc
import concourse.tile as tile
from concourse import mybir
from concourse.masks import make_identity

R = mybir.dt.float32r
F = mybir.dt.float32
BF = mybir.dt.bfloat16
P = 128
B, S, E, H, D, NE = 2, 2048, 1024, 16, 64, 8
NC = E // P
QL = 512
EXP_SCALE = 1.0 / (D ** 0.5)
LN_EPS = 1e-5

_cache = {}


def _kv_plan(qc):
    """K/V token coverage and span split for this qc."""
    kv_tok = 128 * (qc + 13)            # last interleaved block end
    rem = kv_tok - 1536
    rem = max(rem, 256)                 # fp32r needs N>=256
    kv_tok = 1536 + rem
    spans = [(0, 512), (512, 512), (1024, 512), (1536, rem)]
    return kv_tok, spans


def _build(qc):
    nc = bacc.Bacc("TRN2", target_bir_lowering=False, debug=False, num_devices=8,
                   name=f"moe_qc{qc}", enable_partition_id=False)
    kv_tok, kv_spans = _kv_plan(qc)
    KCN = kv_tok // P                   # key chunk count

    def din(name, shape, dt=R):
        return nc.dram_tensor(name, shape, dt, kind="ExternalInput")

    xt = din("xt", [E, S])              # permuted x^T (f32r bits == f32)
    xres = din("xres", [E, QL])
    wq = din("wq", [E, E]); wk = din("wk", [E, E]); wv = din("wv", [E, E])
    bq = din("bq", [P, NC], F); bk = din("bk", [P, NC], F)
    bvr = din("bvr", [1, E])
    wo = din("wo", [E, E]); bo = din("bo", [P, NC], F)
    gw = din("gw", [E, NE]); gb = din("gb", [NE, 1], F)
    cos2 = din("cos2", [P, S], F); sin2 = din("sin2", [P, S], F)
    trid = din("trid", [P, P])          # tri[k, q] = 1 if q >= k (within a block)
    ew = din("ew", [NE, NC, P, E], BF)
    ebT = din("ebT", [NE, E])           # expert bias, experts on partitions
    sel8 = din("sel8", [NE, NE * P])    # sel8[:, e*P:(e+1)*P] row e = 1
    ln1w = din("ln1w", [P, NC], F); ln1b = din("ln1b", [P, NC], F)
    ln2w = din("ln2w", [P, NC], F); ln2b = din("ln2b", [P, NC], F)
    out = nc.dram_tensor("out", [E, QL], R, kind="ExternalOutput")
    KDBG = bool(int(os.environ.get("KDBG", "0")))
    dbg_kind = "ExternalOutput" if KDBG else "Internal"
    vsch = [nc.dram_tensor(f"vsc{i}", [kv_tok, 512], R,
                           kind=dbg_kind) for i in range(2)]
    ctxd = nc.dram_tensor("ctxd", [E, QL], R, kind=dbg_kind)
    if KDBG:
        dbg_qx = nc.dram_tensor("dbg_qx", [E, QL], R, kind="ExternalOutput")
        dbg_qt = nc.dram_tensor("dbg_qt", [E, QL], R, kind="ExternalOutput")
        dbg_kt = nc.dram_tensor("dbg_kt", [P, kv_tok], R, kind="ExternalOutput")
        dbg_x1 = nc.dram_tensor("dbg_x1", [E, QL], R, kind="ExternalOutput")
        dbg_wt = nc.dram_tensor("dbg_wt", [NE, QL], R, kind="ExternalOutput")
        dbg_wbc = nc.dram_tensor("dbg_wbc", [P, NE * QL], BF, kind="ExternalOutput")
        dbg_moe = nc.dram_tensor("dbg_moe", [E, QL], R, kind="ExternalOutput")

    AX = mybir.AxisListType.X
    OP = mybir.AluOpType
    AF = mybir.ActivationFunctionType
    import contextlib

    with tile.TileContext(nc) as tc, \
         nc.allow_low_precision(reason="float32r is bit-identical to float32"), \
         contextlib.ExitStack() as es:
        consts = es.enter_context(tc.tile_pool(name="consts", bufs=1))

        ones_f = consts.tile([P, 1], F, tag="ones_f")
        nc.vector.memset(ones_f[:], 1.0)
        ones = consts.tile([P, 1], R, tag="ones")
        nc.vector.tensor_copy(out=ones[:], in_=ones_f[:])
        ones1_f = consts.tile([1, P], F, tag="ones1_f")
        nc.vector.memset(ones1_f[:], 1.0)
        ones1 = consts.tile([1, P], R, tag="ones1")
        nc.vector.tensor_copy(out=ones1[:], in_=ones1_f[:])
        eps1 = consts.tile([1, 1], F, tag="eps1")
        nc.vector.memset(eps1[:], LN_EPS)
        bq_sb = consts.tile([P, NC], F, tag="bq"); nc.sync.dma_start(bq_sb[:], bq[:])
        bk_sb = consts.tile([P, NC], F, tag="bk"); nc.sync.dma_start(bk_sb[:], bk[:])

        # ===== Phase A: fused V-proj + rope + K-proj per 512-token span =====
        xt_r = xt.rearrange("(c p) s -> p c s", p=P)
        attn_cm = tc.tile_pool(name="attn_res", bufs=1)
        attn_res = attn_cm.__enter__()
        kTs = [attn_res.tile([P, kv_tok], R, tag=f"kT{oc}", name=f"kT{oc}")
               for oc in range(NC)]
        qT = attn_res.tile([P, NC, QL], R, tag="qT")
        qx_cm = tc.tile_pool(name="qx_p", bufs=1)
        qx_p = qx_cm.__enter__()
        qx = qx_p.tile([P, NC, QL], R, tag="qx")

        with tc.tile_pool(name="xsp_p", bufs=2) as xsp_p, \
             tc.tile_pool(name="cs_p", bufs=1) as cs_p, \
             tc.tile_pool(name="a1tmp", bufs=1) as a1tmp, \
             tc.tile_pool(name="wvkp", bufs=1) as wvkp, \
             tc.tile_pool(name="vev_p", bufs=1) as vev_p, \
             tc.tile_pool(name="vps", bufs=4, space="PSUM") as vps, \
             tc.tile_pool(name="kps", bufs=3, space="PSUM") as kps:
            wv_sb = wvkp.tile([P, NC, E], R, tag="wv")
            for c in range(NC):
                nc.sync.dma_start(wv_sb[:, c, :],
                                  wv.rearrange("(c p) m -> p c m", p=P)[:, c, :])
            wk_sb = wvkp.tile([P, NC, E], R, tag="wk")
            for c in range(NC):
                nc.sync.dma_start(wk_sb[:, c, :],
                                  wk.rearrange("(c p) m -> p c m", p=P)[:, c, :])
            bv_sb = wvkp.tile([1, E], R, tag="bv")
            nc.sync.dma_start(bv_sb[:], bvr[:])

            xsp_t, cs_t = {}, {}

            def emit_dma(sp):
                s0, sl = kv_spans[sp]
                xsp = xsp_p.tile([P, NC, 512], R, tag="xsp")
                nc.sync.dma_start(xsp[:, :, :sl], xt_r[:, :, s0:s0 + sl])
                csb = cs_p.tile([P, 2, 512], F, tag="csb")
                nc.sync.dma_start(csb[:, 0, :sl], cos2[:, s0:s0 + sl])
                nc.sync.dma_start(csb[:, 1, :sl], sin2[:, s0:s0 + sl])
                xsp_t[sp] = xsp
                cs_t[sp] = csb

            def emit_V(sp):
                s0, sl = kv_spans[sp]
                xsp = xsp_t[sp]
                for t in range(sl // P):
                    for dvs in range(2):
                        vp = vps.tile([P, 512], F, tag="vps")
                        for dc in range(NC):
                            nc.tensor.matmul(
                                vp[:], xsp[:, dc, t * P:(t + 1) * P],
                                wv_sb[:, dc, dvs * 512:(dvs + 1) * 512],
                                start=(dc == 0), stop=False)
                        nc.tensor.matmul(
                            vp[:], ones1[:, :], bv_sb[:, dvs * 512:(dvs + 1) * 512],
                            start=False, stop=True)
                        vev = vev_p.tile([P, 512], R, tag="vev")
                        nc.scalar.copy(out=vev[:], in_=vp[:])
                        nc.sync.dma_start(
                            vsch[dvs][s0 + t * P:s0 + (t + 1) * P, :], vev[:])

            def emit_rope(sp):
                s0, sl = kv_spans[sp]
                xsp, csb = xsp_t[sp], cs_t[sp]
                cs = csb[:, 0, :sl]; sn = csb[:, 1, :sl]
                for c in range(4):
                    # in-place rotation: a' = a*cos - b*sin; b' = a*sin + b*cos
                    t1 = a1tmp.tile([P, 512], F, tag="ropet1")
                    t2 = a1tmp.tile([P, 512], F, tag="ropet2")
                    a = xsp[:, c, :sl]; b = xsp[:, c + 4, :sl]
                    nc.vector.tensor_tensor(out=t1[:, :sl], in0=a, in1=sn, op=OP.mult)
                    nc.vector.tensor_tensor(out=t2[:, :sl], in0=b, in1=sn, op=OP.mult)
                    nc.vector.tensor_tensor(out=a, in0=a, in1=cs, op=OP.mult)
                    nc.vector.tensor_tensor(out=a, in0=a, in1=t2[:, :sl], op=OP.subtract)
                    nc.vector.tensor_tensor(out=b, in0=b, in1=cs, op=OP.mult)
                    nc.vector.tensor_tensor(out=b, in0=b, in1=t1[:, :sl], op=OP.add)
                # q block sp = global block qc+4*sp lives at cols qc*P within span sp
                for c in range(NC):
                    nc.vector.tensor_copy(
                        out=qx[:, c, sp * P:(sp + 1) * P],
                        in_=xsp[:, c, qc * P:(qc + 1) * P])

            def emit_K(sp):
                s0, sl = kv_spans[sp]
                xsp = xsp_t[sp]
                for oc in range(NC):
                    kp = kps.tile([P, 512], F, tag="kps")
                    for dc in range(NC):
                        nc.tensor.matmul(
                            kp[:, :sl], wk_sb[:, dc, oc * P:(oc + 1) * P],
                            xsp[:, dc, :sl],
                            start=(dc == 0), stop=(dc == NC - 1))
                    nc.vector.tensor_scalar(
                        out=kTs[oc][:, s0:s0 + sl], in0=kp[:, :sl],
                        scalar1=bk_sb[:, oc:oc + 1], scalar2=None, op0=OP.add)

            nsp = len(kv_spans)
            emit_dma(0); emit_V(0); emit_rope(0)
            emit_dma(1); emit_V(1); emit_K(0); emit_rope(1)
            emit_dma(2); emit_V(2); emit_K(1); emit_rope(2)
            emit_dma(3); emit_V(3); emit_K(2); emit_rope(3)
            emit_K(3)

        # ===== Q projection from roped q blocks in SBUF =====
        with tc.tile_pool(name="wqp", bufs=1) as wqp, \
             tc.tile_pool(name="qps_p", bufs=4, space="PSUM") as qps_p:
            wq_sb = wqp.tile([P, NC, E], R, tag="wq_sb")
            for c in range(NC):
                nc.sync.dma_start(wq_sb[:, c, :],
                                  wq.rearrange("(c p) m -> p c m", p=P)[:, c, :])
            for oc in range(NC):
                qp = qps_p.tile([P, 512], F, tag="qps")
                for dc in range(NC):
                    nc.tensor.matmul(
                        qp[:], wq_sb[:, dc, oc * P:(oc + 1) * P], qx[:, dc, :],
                        start=(dc == 0), stop=(dc == NC - 1))
                nc.vector.tensor_scalar(
                    out=qT[:, oc, :], in0=qp[:],
                    scalar1=bq_sb[:, oc:oc + 1], scalar2=None, op0=OP.add)
            if KDBG:
                nc.sync.dma_start(dbg_qx.rearrange("(c p) q -> p c q", p=P)[:], qx[:])
                nc.sync.dma_start(dbg_qt.rearrange("(c p) q -> p c q", p=P)[:], qT[:])
                nc.sync.dma_start(dbg_kt[:], kTs[0][:])
        qx_cm.__exit__(None, None, None)

        # ===== Phase B: causal attention over interleaved blocks =====
        # ctx stationary vt has a ones column per head half: row 64 of the
        # [65, QL] ctx psum accumulates the softmax denominator for free.
        ctxd_r = ctxd.rearrange("(c p) q -> p c q", p=P)
        kcs = [kc for kc in range(KCN) if -(-(kc - qc) // 4) < 4]
        with tc.tile_pool(name="bt", bufs=6) as bt, \
             tc.tile_pool(name="bconst", bufs=1) as bconst, \
             tc.tile_pool(name="vt_p", bufs=2) as vt_p, \
             tc.tile_pool(name="scps", bufs=4, space="PSUM") as scps, \
             tc.tile_pool(name="ctxps", bufs=4, space="PSUM") as ctxps:
            tri_sb = bconst.tile([P, P], R, tag="tri")
            nc.sync.dma_start(tri_sb[:], trid[:])
            ones65_f = bconst.tile([65, P], F, tag="ones65_f")
            nc.vector.memset(ones65_f[:], 1.0)
            ones65 = bconst.tile([65, P], R, tag="ones65")
            nc.vector.tensor_copy(out=ones65[:], in_=ones65_f[:])
            oneskc = bconst.tile([P, KCN], F, tag="oneskc")
            nc.vector.memset(oneskc[:], 1.0)
            zer128 = bconst.tile([P, P], F, tag="zer128")
            nc.vector.memset(zer128[:], 0.0)
            for hp in range(NC):
                vt = vt_p.tile([P, KCN, 130], R, tag="vt")
                for hh in range(2):
                    nc.sync.dma_start(
                        vt[:, :, hh * 65:hh * 65 + 64],
                        vsch[hp // 4].rearrange("(k p) d -> p k d", p=P)
                        [:, :, (hp % 4) * P + hh * 64:(hp % 4) * P + (hh + 1) * 64])
                    nc.vector.tensor_copy(out=vt[:, :, hh * 65 + 64:hh * 65 + 65],
                                          in_=oneskc[:])
                ctxp2 = [ctxps.tile([65, QL], F, tag="ctxps", name=f"ctxp_{hp}_{hh}")
                         for hh in range(2)]
                for kc in kcs:
                    j0 = max(0, -(-(kc - qc) // 4))
                    q0 = j0 * P
                    qn = max(512 - q0, 256)      # fp32r needs N>=256
                    c0 = 512 - qn
                    for hh in range(2):
                        scp = scps.tile([P, QL], F, tag="scps")
                        nc.tensor.matmul(
                            scp[:, c0:], kTs[hp][hh * 64:(hh + 1) * 64, kc * P:(kc + 1) * P],
                            qT[hh * 64:(hh + 1) * 64, hp, c0:], start=True, stop=True)
                        st = bt.tile([P, QL], R, tag="st")
                        nc.scalar.activation(out=st[:, c0:], in_=scp[:, c0:],
                                             func=AF.Exp, scale=EXP_SCALE)
                        if c0 < q0:
                            nc.vector.tensor_copy(out=st[:, c0:q0], in_=zer128[:, :q0 - c0])
                        if kc >= qc and (kc - qc) % 4 == 0:
                            j = (kc - qc) // 4
                            nc.vector.tensor_tensor(
                                out=st[:, j * P:(j + 1) * P], in0=st[:, j * P:(j + 1) * P],
                                in1=tri_sb[:], op=OP.mult)
                        nc.tensor.matmul(ctxp2[hh][:, c0:],
                                         vt[:, kc, hh * 65:(hh + 1) * 65], st[:, c0:],
                                         start=(kc == kcs[0]), stop=(kc == kcs[-1]))
                ctxo = bt.tile([P, QL], R, tag="ctxo")
                for hh in range(2):
                    rden = bt.tile([65, QL], R, tag="rden")
                    nc.vector.reciprocal(out=rden[64:65, :], in_=ctxp2[hh][64:65, :])
                    rbp = scps.tile([P, QL], F, tag="scps")
                    nc.tensor.matmul(rbp[0:64, :], ones65[64:65, 0:64], rden[64:65, :],
                                     start=True, stop=True)
                    rb_sb = bt.tile([64, QL], R, tag="rb_sb")
                    nc.scalar.copy(out=rb_sb[:], in_=rbp[0:64, :])
                    nc.vector.tensor_tensor(
                        out=ctxo[hh * 64:(hh + 1) * 64, :],
                        in0=ctxp2[hh][0:64, :], in1=rb_sb[:], op=OP.mult)
                nc.sync.dma_start(ctxd_r[:, hp, :], ctxo[:])
        attn_cm.__exit__(None, None, None)

        # ===== LN helper =====
        def layernorm(src, dst, wtile, btile, tmp, ps_row, ps_big):
            sp_ = ps_row.tile([1, QL], F, tag="lnsum")
            for c in range(NC):
                nc.tensor.matmul(sp_[:], ones[:], src[:, c, :],
                                 start=(c == 0), stop=(c == NC - 1))
            s2p = ps_row.tile([1, QL], F, tag="lnsum2")
            for c in range(NC):
                sq = tmp.tile([P, QL], R, tag="lnsq")
                nc.scalar.activation(out=sq[:], in_=src[:, c, :], func=AF.Square)
                nc.tensor.matmul(s2p[:], ones[:], sq[:],
                                 start=(c == 0), stop=(c == NC - 1))
            mean = tmp.tile([1, QL], R, tag="lnmean")
            nc.scalar.mul(out=mean[:], in_=sp_[:], mul=1.0 / E)
            msq = tmp.tile([1, QL], R, tag="lnmsq")
            nc.scalar.mul(out=msq[:], in_=s2p[:], mul=1.0 / E)
            var = tmp.tile([1, QL], R, tag="lnvar")
            nc.vector.tensor_tensor(out=var[:], in0=mean[:], in1=mean[:], op=OP.mult)
            nc.vector.tensor_tensor(out=var[:], in0=msq[:], in1=var[:], op=OP.subtract)
            std = tmp.tile([1, QL], R, tag="lnstd")
            nc.scalar.activation(out=std[:], in_=var[:], func=AF.Sqrt, bias=eps1[:])
            rstd = tmp.tile([1, QL], R, tag="lnrstd")
            nc.vector.reciprocal(out=rstd[:], in_=std[:])
            mb = ps_big.tile([P, QL], F, tag="bigc")
            nc.tensor.matmul(mb[:], ones1[:], mean[:], start=True, stop=True)
            rb = ps_big.tile([P, QL], F, tag="bigc")
            nc.tensor.matmul(rb[:], ones1[:], rstd[:], start=True, stop=True)
            mbs = tmp.tile([P, QL], R, tag="lnmbs")
            nc.scalar.copy(out=mbs[:], in_=mb[:])
            rbs = tmp.tile([P, QL], R, tag="lnrbs")
            nc.scalar.copy(out=rbs[:], in_=rb[:])
            for c in range(NC):
                t = tmp.tile([P, QL], R, tag="lnt")
                nc.vector.tensor_tensor(out=t[:], in0=src[:, c, :], in1=mbs[:], op=OP.subtract)
                nc.vector.tensor_tensor(out=t[:], in0=t[:], in1=rbs[:], op=OP.mult)
                nc.vector.tensor_scalar(out=dst[:, c, :], in0=t[:],
                                        scalar1=wtile[:, c:c + 1], scalar2=btile[:, c:c + 1],
                                        op0=OP.mult, op1=OP.add)

        # ===== Phase C: out-proj + LN1 + gates + top-2 weights =====
        cres = es.enter_context(tc.tile_pool(name="cres", bufs=1))
        x1 = cres.tile([P, NC, QL], R, tag="x1")
        x1b = cres.tile([P, NC, QL], BF, tag="x1b")
        wbc = cres.tile([P, NE, QL], BF, tag="wbc")
        wT_sb = cres.tile([NE, QL], R, tag="wT")
        ebT_sb = cres.tile([NE, E], R, tag="ebT"); nc.sync.dma_start(ebT_sb[:], ebT[:])
        ln_sb = {}
        for nm, t in (("ln1w", ln1w), ("ln1b", ln1b), ("ln2w", ln2w), ("ln2b", ln2b)):
            ln_sb[nm] = cres.tile([P, NC], F, tag=nm, name=nm)
            nc.sync.dma_start(ln_sb[nm][:], t[:])
        with tc.tile_pool(name="cslab", bufs=1) as cslab, \
             tc.tile_pool(name="ct", bufs=2) as ct, \
             tc.tile_pool(name="h1p", bufs=1) as h1p, \
             tc.tile_pool(name="cps", bufs=3, space="PSUM") as cps, \
             tc.tile_pool(name="crow", bufs=1, space="PSUM") as crow, \
             tc.tile_pool(name="cg", bufs=1, space="PSUM") as cg:
            wo_sb = cslab.tile([P, NC, E], R, tag="wo_sb")
            for c in range(NC):
                nc.sync.dma_start(wo_sb[:, c, :],
                                  wo.rearrange("(c p) m -> p c m", p=P)[:, c, :])
            ctx_sb = cslab.tile([P, NC, QL], R, tag="ctx")
            nc.sync.dma_start(ctx_sb[:], ctxd_r[:])
            bo_sb = cslab.tile([P, NC], F, tag="bo"); nc.sync.dma_start(bo_sb[:], bo[:])
            gb_sb = cslab.tile([NE, 1], F, tag="gb"); nc.sync.dma_start(gb_sb[:], gb[:])
            gw_sb = cslab.tile([P, NC, NE], R, tag="gw")
            nc.sync.dma_start(gw_sb[:], gw.rearrange("(c p) g -> p c g", p=P))
            sel8_sb = cslab.tile([NE, NE * P], R, tag="sel8")
            nc.sync.dma_start(sel8_sb[:], sel8[:])
            iden_f = cslab.tile([P, P], F, tag="iden_f")
            make_identity(nc, iden_f[:])
            iden = cslab.tile([P, P], R, tag="iden")
            nc.vector.tensor_copy(out=iden[:], in_=iden_f[:])
            xres_sb = h1p.tile([P, NC, QL], R, tag="xres")
            nc.sync.dma_start(xres_sb[:], xres.rearrange("(c p) q -> p c q", p=P))
            h1 = h1p.tile([P, NC, QL], R, tag="h1")
            for oc in range(NC):
                ap = cps.tile([P, QL], F, tag="bigc")
                for dc in range(NC):
                    nc.tensor.matmul(ap[:], wo_sb[:, dc, oc * P:(oc + 1) * P],
                                     ctx_sb[:, dc, :],
                                     start=(dc == 0), stop=(dc == NC - 1))
                nc.vector.tensor_scalar(out=h1[:, oc, :], in0=ap[:],
                                        scalar1=bo_sb[:, oc:oc + 1], scalar2=None, op0=OP.add)
                nc.vector.tensor_tensor(out=h1[:, oc, :], in0=h1[:, oc, :],
                                        in1=xres_sb[:, oc, :], op=OP.add)

            layernorm(h1, x1, ln_sb["ln1w"], ln_sb["ln1b"], ct, crow, cps)

            gp = cg.tile([NE, QL], F, tag="gps")
            for c in range(NC):
                nc.tensor.matmul(gp[:], gw_sb[:, c, :], x1[:, c, :],
                                 start=(c == 0), stop=(c == NC - 1))
            glog = ct.tile([NE, QL], R, tag="glog")
            nc.vector.tensor_scalar(out=glog[:], in0=gp[:], scalar1=gb_sb[:],
                                    scalar2=None, op0=OP.add)
            gexp = ct.tile([NE, QL], R, tag="gexp")
            nc.scalar.activation(out=gexp[:], in_=glog[:], func=AF.Exp)

            # token-orientation gate math: gt[tok, qb, e]
            gt = ct.tile([P, 4, NE], R, tag="gt")
            for qb in range(4):
                gtp = cg.tile([P, NE], R, tag="gmix")
                nc.tensor.transpose(gtp[:], gexp[:, qb * P:(qb + 1) * P], iden[0:NE, 0:NE])
                nc.vector.tensor_copy(out=gt[:, qb, :], in_=gtp[:])
            dsum = ct.tile([P, 4], F, tag="dsum")
            nc.vector.reduce_sum(out=dsum[:], in_=gt[:], axis=AX)
            rgn = ct.tile([P, 4], F, tag="rgn")
            nc.vector.reciprocal(out=rgn[:], in_=dsum[:])
            m1 = ct.tile([P, 4], F, tag="m1")
            nc.vector.reduce_max(out=m1[:], in_=gt[:], axis=AX)
            msel = ct.tile([P, 4, NE], R, tag="msel")
            g2 = ct.tile([P, 4, NE], R, tag="g2")
            for qb in range(4):
                nc.vector.tensor_scalar(out=msel[:, qb, :], in0=gt[:, qb, :],
                                        scalar1=m1[:, qb:qb + 1], scalar2=None, op0=OP.is_equal)
            nc.vector.tensor_tensor(out=g2[:], in0=gt[:], in1=msel[:], op=OP.mult)
            nc.vector.tensor_tensor(out=g2[:], in0=gt[:], in1=g2[:], op=OP.subtract)
            m2 = ct.tile([P, 4], F, tag="m2")
            nc.vector.reduce_max(out=m2[:], in_=g2[:], axis=AX)
            msel2 = ct.tile([P, 4, NE], R, tag="msel2")
            for qb in range(4):
                nc.vector.tensor_scalar(out=msel2[:, qb, :], in0=g2[:, qb, :],
                                        scalar1=m2[:, qb:qb + 1], scalar2=None, op0=OP.is_equal)
            nc.vector.tensor_tensor(out=msel[:], in0=msel[:], in1=msel2[:], op=OP.add)
            wsel = ct.tile([P, 4, NE], R, tag="wsel")
            nc.vector.tensor_tensor(out=wsel[:], in0=gt[:], in1=msel[:], op=OP.mult)
            for qb in range(4):
                nc.vector.tensor_scalar(out=wsel[:, qb, :], in0=wsel[:, qb, :],
                                        scalar1=rgn[:, qb:qb + 1], scalar2=None, op0=OP.mult)

            # transpose per q-block -> wT_sb [NE, QL] (bf16)
            for qb in range(4):
                wtp = cg.tile([NE, P], R, tag="gmix")
                nc.tensor.transpose(wtp[:], wsel[:, qb, :], iden[:])
                nc.vector.tensor_copy(out=wT_sb[:, qb * P:(qb + 1) * P], in_=wtp[:])
            if KDBG:
                nc.sync.dma_start(dbg_x1.rearrange("(c p) q -> p c q", p=P)[:], x1[:])
                nc.sync.dma_start(dbg_wt[:], wT_sb[:])
            # broadcast per expert -> wbc [P, e, QL] (bf16)
            for e in range(NE):
                bcp = cps.tile([P, QL], F, tag="bigc")
                nc.tensor.matmul(bcp[:], sel8_sb[:, e * P:(e + 1) * P], wT_sb[:],
                                 start=True, stop=True)
                nc.vector.tensor_copy(out=wbc[:, e, :], in_=bcp[:])
            for c in range(NC):
                nc.vector.tensor_copy(out=x1b[:, c, :], in_=x1[:, c, :])

        # ===== Phase D: dense MoE with gate-scaled inputs, DVE accumulate =====
        moe = cres.tile([P, NC, QL], R, tag="moe")
        with tc.tile_pool(name="ewp", bufs=2) as ewp, \
             tc.tile_pool(name="xwp", bufs=2) as xwp, \
             tc.tile_pool(name="dps", bufs=4, space="PSUM") as dps:
            for e in range(NE):
                ew_sl = ewp.tile([P, NC, E], BF, tag="ew_sl")
                nc.sync.dma_start(ew_sl[:], ew[e].rearrange("c p d -> p c d"))
                xw = xwp.tile([P, NC, QL], BF, tag="xw")
                for dc in range(NC):
                    nc.vector.tensor_tensor(out=xw[:, dc, :], in0=x1b[:, dc, :],
                                            in1=wbc[:, e, :], op=OP.mult)
                for oc in range(NC):
                    yp = dps.tile([P, QL], F, tag="yps")
                    if e == 0:
                        nc.tensor.matmul(yp[:], ebT_sb[:, oc * P:(oc + 1) * P],
                                         wT_sb[:], start=True, stop=False)
                    for dc in range(NC):
                        nc.tensor.matmul(
                            yp[:], ew_sl[:, dc, oc * P:(oc + 1) * P], xw[:, dc, :],
                            start=(e != 0 and dc == 0), stop=(dc == NC - 1))
                    if e == 0:
                        # fold in the x1 residual for LN2 up front
                        nc.vector.tensor_tensor(out=moe[:, oc, :], in0=x1[:, oc, :],
                                                in1=yp[:], op=OP.add)
                    else:
                        nc.vector.tensor_tensor(out=moe[:, oc, :], in0=moe[:, oc, :],
                                                in1=yp[:], op=OP.add)

        if KDBG:
            nc.sync.dma_start(dbg_moe.rearrange("(c p) q -> p c q", p=P)[:], moe[:])
            nc.sync.dma_start(dbg_wbc.rearrange("p (e q) -> p e q", e=NE)[:], wbc[:])

        # ===== Phase E: LN2 + store =====
        with tc.tile_pool(name="et", bufs=3) as et, \
             tc.tile_pool(name="eps_", bufs=2, space="PSUM") as eps_, \
             tc.tile_pool(name="erow", bufs=1, space="PSUM") as erow:
            layernorm(moe, moe, ln_sb["ln2w"], ln_sb["ln2b"], et, erow, eps_)
            for c in range(NC):
                nc.sync.dma_start(out.rearrange("(c p) q -> p c q", p=P)[:, c, :], moe[:, c, :])

    nc.compile()
    return nc


def _prep_inputs(inputs):
    x = np.asarray(inputs['x'], dtype=np.float32)
    ipw = np.asarray(inputs['in_proj_w'], dtype=np.float32)
    ipb = np.asarray(inputs['in_proj_b'], dtype=np.float32)
    opw = np.asarray(inputs['out_proj_w'], dtype=np.float32)
    opb = np.asarray(inputs['out_proj_b'], dtype=np.float32)
    gww = np.asarray(inputs['gate_w'], dtype=np.float32)
    gbb = np.asarray(inputs['gate_b'], dtype=np.float32)
    eww = np.asarray(inputs['expert_w'], dtype=np.float32)
    ebb = np.asarray(inputs['expert_b'], dtype=np.float32)

    perm = np.empty(E, dtype=np.int64)
    idx = 0
    for h in range(H):
        for i in range(D // 2):
            perm[idx] = 64 * h + 2 * i; idx += 1
    for h in range(H):
        for i in range(D // 2):
            perm[idx] = 64 * h + 2 * i + 1; idx += 1

    Wq, Wk, Wv = ipw[0:E], ipw[E:2 * E], ipw[2 * E:3 * E]
    bq_, bk_, bv_ = ipb[0:E], ipb[E:2 * E], ipb[2 * E:3 * E]
    sel8 = np.zeros((NE, NE * P), dtype=np.float32)
    for e in range(NE):
        sel8[e, e * P:(e + 1) * P] = 1.0
    common = {
        "wq": np.ascontiguousarray(Wq[:, perm].T),
        "wk": np.ascontiguousarray(Wk[:, perm].T),
        "wv": np.ascontiguousarray(Wv[:, perm].T),
        "bq": np.ascontiguousarray(bq_.reshape(NC, P).T),
        "bk": np.ascontiguousarray(bk_.reshape(NC, P).T),
        "bvr": bv_.reshape(1, E).copy(),
        "wo": np.ascontiguousarray(opw.T),
        "bo": np.ascontiguousarray(opb.reshape(NC, P).T),
        "gw": np.ascontiguousarray(gww.T),
        "gb": gbb.reshape(NE, 1).copy(),
        "ew": np.ascontiguousarray(
            eww.transpose(0, 2, 1).reshape(NE, NC, P, E)).astype(ml_dtypes.bfloat16),
        "ebT": ebb,
        "sel8": sel8,
        "ln1w": np.ascontiguousarray(np.asarray(inputs['ln1_w'], np.float32).reshape(NC, P).T),
        "ln1b": np.ascontiguousarray(np.asarray(inputs['ln1_b'], np.float32).reshape(NC, P).T),
        "ln2w": np.ascontiguousarray(np.asarray(inputs['ln2_w'], np.float32).reshape(NC, P).T),
        "ln2b": np.ascontiguousarray(np.asarray(inputs['ln2_b'], np.float32).reshape(NC, P).T),
        "trid": np.ascontiguousarray(
            (np.arange(P)[None, :] >= np.arange(P)[:, None]).astype(np.float32)),
    }
    inv_freq = 1.0 / (10000.0 ** (np.arange(0, D, 2, dtype=np.float64) / D))
    freqs = np.arange(S, dtype=np.float64)[:, None] * inv_freq[None, :]
    cos_t = np.cos(freqs).T.astype(np.float32)
    sin_t = np.sin(freqs).T.astype(np.float32)
    common["cos2"] = np.ascontiguousarray(np.tile(cos_t, (4, 1)))
    common["sin2"] = np.ascontiguousarray(np.tile(sin_t, (4, 1)))

    in_maps = []
    for c in range(8):
        b, qc = c // 4, c % 4
        blocks = [qc + 4 * i for i in range(4)]
        cols = np.concatenate([np.arange(blk * P, (blk + 1) * P) for blk in blocks])
        xtb = np.ascontiguousarray(x[b].T)
        xtp = np.ascontiguousarray(xtb[perm])
        m = dict(common)
        m["xt"] = xtp
        m["xres"] = np.ascontiguousarray(xtb[:, cols])
        in_maps.append(m)
    return in_maps


def _run_multi(ncs, in_maps):
    """Run the 4 NEFFs concurrently: graph qc on devices {qc, qc+4} (b=0,1)."""
    import jax
    from jax.sharding import Mesh, PartitionSpec
    from jax.experimental.shard_map import shard_map
    from concourse import bass2jax
    from concourse import mybir as _mb

    bass2jax.install_neuronx_cc_hook()
    devices = jax.devices()

    if "jits" not in _cache:
        _cache["jits"] = {}
    handles = []
    for qc in range(4):
        nc = ncs[qc]
        if qc not in _cache["jits"]:
            in_names, out_names, out_avals, zero_outs = [], [], [], []
            for alloc in nc.m.functions[0].allocations:
                if not isinstance(alloc, _mb.MemoryLocationSet):
                    continue
                name = alloc.memorylocations[0].name
                if alloc.kind == "ExternalInput":
                    in_names.append(name)
                elif alloc.kind == "ExternalOutput":
                    out_names.append(name)
                    shape = tuple(alloc.tensor_shape)
                    dtype = _mb.dt.np(alloc.dtype)
                    out_avals.append(jax.core.ShapedArray(shape, dtype))
                    zero_outs.append(np.zeros(shape, dtype))
            n_params = len(in_names)
            all_names = in_names + out_names
            donate = tuple(range(n_params, n_params + len(out_names)))

            def _body(*args, _nc=nc, _avals=tuple(out_avals), _all=tuple(all_names),
                      _outs=tuple(out_names)):
                outs = bass2jax._bass_exec_p.bind(
                    *args, out_avals=_avals, in_names=_all, out_names=_outs,
                    lowering_input_output_aliases=(),
                    sim_require_finite=True, sim_require_nnan=True, nc=_nc)
                return tuple(outs)

            devs = [devices[qc], devices[qc + 4]]
            mesh = Mesh(np.asarray(devs), ("core",))
            nio = n_params + len(zero_outs)
            sharded = jax.jit(
                shard_map(_body, mesh=mesh,
                          in_specs=(PartitionSpec("core"),) * nio,
                          out_specs=(PartitionSpec("core"),) * len(out_names),
                          check_rep=False),
                donate_argnums=donate, keep_unused=True)
            _cache["jits"][qc] = (sharded, in_names, out_names, zero_outs)
        sharded, in_names, out_names, zero_outs = _cache["jits"][qc]
        per_core = [[np.asarray(in_maps[b * 4 + qc][n]) for n in in_names] for b in range(2)]
        concat_in = [np.concatenate([per_core[b][i] for b in range(2)], axis=0)
                     for i in range(len(in_names))]
        concat_zero = [np.concatenate([z, z], axis=0) for z in zero_outs]
        handles.append((sharded, concat_in, concat_zero, out_names))

    outs = []
    for sharded, concat_in, concat_zero, out_names in handles:
        outs.append((sharded(*concat_in, *concat_zero), out_names))
    results = [None] * 8
    for qc, (arrs, out_names) in enumerate(outs):
        arrs = [np.asarray(a) for a in arrs]
        for b in range(2):
            rm = {}
            for i, n in enumerate(out_names):
                full = arrs[i]
                half = full.shape[0] // 2
                rm[n] = full[b * half:(b + 1) * half]
            results[b * 4 + qc] = rm
    return results


def _ensure_ntff_hook():
    import types
    try:
        from antenv.axon_hooks import get_axon_ntff_profile_hook  # noqa
        return True
    except ImportError:
        pass
    try:
        import antenv
        sys.path.insert(0, '/root/.axon_site')
        from trn_agent_boot.trn_boot import _ntff_profile_via_ctypes
        hook = _ntff_profile_via_ctypes('/opt/axon/libaxon_pjrt.so')
        if hook is None:
            return False
        mod = types.ModuleType('antenv.axon_hooks')
        _state = {'hook': hook}
        mod.set_axon_ntff_profile_hook = lambda h: _state.__setitem__('hook', h)
        mod.get_axon_ntff_profile_hook = lambda: _state['hook']
        sys.modules['antenv.axon_hooks'] = mod
        antenv.axon_hooks = mod
        return True
    except Exception as e:
        print(f"ntff hook setup failed: {e}")
        return False


def kernel(**inputs):
    if "ncs" not in _cache:
        _cache["ncs"] = [_build(qc) for qc in range(4)]
    ncs = _cache["ncs"]
    in_maps = _prep_inputs(inputs)

    trace = bool(int(os.environ.get("KERNEL_TRACE", "0")))
    if trace and _ensure_ntff_hook():
        import tempfile
        from antenv.axon_hooks import get_axon_ntff_profile_hook
        hook = get_axon_ntff_profile_hook()
        tmpdir = tempfile.mkdtemp()
        _run_multi(ncs, in_maps)  # warm-up/compile outside the profile window
        with hook(tmpdir, list(range(8))):
            results = _run_multi(ncs, in_maps)
        _cache["ntff_dir"] = tmpdir
        print(f"ntff dir: {tmpdir}")
    else:
        results = _run_multi(ncs, in_maps)
    _cache["last_results"] = results

    out = np.empty((B, S, E), dtype=np.float32)
    for c in range(8):
        b, qc = c // 4, c % 4
        o = results[c]["out"]  # [E, QL]
        for i in range(4):
            blk = qc + 4 * i
            out[b, blk * P:(blk + 1) * P, :] = o[:, i * P:(i + 1) * P].T
    return out


# revision 39
# speedup vs baseline: 1.1473x; 1.0164x over previous
"""Trainium2 Bass kernel for nn_EnhancedTransformerLayer (RoPE attention + MoE).

Sharding: 8 cores; core c -> batch b=c//4, qc=c%4. Four distinct NEFFs (one per
qc), each run on 2 cores (b=0,1). Core qc owns interleaved query blocks
{qc, qc+4, qc+8, qc+12} (4 x 128 tokens) so causal work is balanced, and only
computes K/V up to its last block.

v2: fused rope/V/K span pipeline (single x read, in-place rope, no DRAM
round-trip), softmax denominator via ones-column in the ctx stationary,
fp32r matmuls clamped to N>=256, MoE accumulated directly in PSUM across
experts with gate-scaled inputs (no gpsimd, no per-expert combine).
"""
import sys, os
sys.path.insert(0, '/opt/trn_rl_repo')
import numpy as np
import ml_dtypes

import concourse.bass as bass
from concourse import bacc
import concourse.tile as tile
from concourse import mybir
from concourse.masks import make_identity

R = mybir.dt.float32r
F = mybir.dt.float32
BF = mybir.dt.bfloat16
P = 128
B, S, E, H, D, NE = 2, 2048, 1024, 16, 64, 8
NC = E // P
QL = 512
EXP_SCALE = 1.0 / (D ** 0.5)
LN_EPS = 1e-5

_cache = {}


def _kv_plan(qc):
    """K/V token coverage and span split for this qc."""
    kv_tok = 128 * (qc + 13)            # last interleaved block end
    rem = kv_tok - 1536
    rem = max(rem, 256)                 # fp32r needs N>=256
    kv_tok = 1536 + rem
    spans = [(0, 512), (512, 512), (1024, 512), (1536, rem)]
    return kv_tok, spans


def _build(qc):
    nc = bacc.Bacc("TRN2", target_bir_lowering=False, debug=False, num_devices=8,
                   name=f"moe_qc{qc}", enable_partition_id=False)
    kv_tok, kv_spans = _kv_plan(qc)
    KCN = kv_tok // P                   # key chunk count

    def din(name, shape, dt=R):
        return nc.dram_tensor(name, shape, dt, kind="ExternalInput")

    xt = din("xt", [E, S])              # permuted x^T (f32r bits == f32)
    xres = din("xres", [E, QL])
    wq = din("wq", [E, E]); wk = din("wk", [E, E]); wv = din("wv", [E, E])
    bq = din("bq", [P, NC], F); bk = din("bk", [P, NC], F)
    bvr = din("bvr", [1, E])
    wo = din("wo", [E, E]); bo = din("bo", [P, NC], F)
    gw = din("gw", [E, NE]); gb = din("gb", [NE, 1], F)
    cos2 = din("cos2", [P, S], F); sin2 = din("sin2", [P, S], F)
    trid = din("trid", [P, P])          # tri[k, q] = 1 if q >= k (within a block)
    ew = din("ew", [NE, NC, P, E], BF)
    ebT = din("ebT", [NE, E])           # expert bias, experts on partitions
    sel8 = din("sel8", [NE, NE * P])    # sel8[:, e*P:(e+1)*P] row e = 1
    ln1w = din("ln1w", [P, NC], F); ln1b = din("ln1b", [P, NC], F)
    ln2w = din("ln2w", [P, NC], F); ln2b = din("ln2b", [P, NC], F)
    out = nc.dram_tensor("out", [E, QL], R, kind="ExternalOutput")
    KDBG = bool(int(os.environ.get("KDBG", "0")))
    dbg_kind = "ExternalOutput" if KDBG else "Internal"
    vsch = [nc.dram_tensor(f"vsc{i}", [kv_tok, 512], R,
                           kind=dbg_kind) for i in range(2)]
    ctxd = nc.dram_tensor("ctxd", [E, QL], R, kind=dbg_kind)
    if KDBG:
        dbg_qx = nc.dram_tensor("dbg_qx", [E, QL], R, kind="ExternalOutput")
        dbg_qt = nc.dram_tensor("dbg_qt", [E, QL], R, kind="ExternalOutput")
        dbg_kt = nc.dram_tensor("dbg_kt", [P, kv_tok], R, kind="ExternalOutput")
        dbg_x1 = nc.dram_tensor("dbg_x1", [E, QL], R, kind="ExternalOutput")
        dbg_wt = nc.dram_tensor("dbg_wt", [NE, QL], R, kind="ExternalOutput")
        dbg_wbc = nc.dram_tensor("dbg_wbc", [P, NE * QL], BF, kind="ExternalOutput")
        dbg_moe = nc.dram_tensor("dbg_moe", [E, QL], R, kind="ExternalOutput")

    AX = mybir.AxisListType.X
    OP = mybir.AluOpType
    AF = mybir.ActivationFunctionType
    import contextlib

    with tile.TileContext(nc) as tc, \
         nc.allow_low_precision(reason="float32r is bit-identical to float32"), \
         contextlib.ExitStack() as es:
        consts = es.enter_context(tc.tile_pool(name="consts", bufs=1))

        ones_f = consts.tile([P, 1], F, tag="ones_f")
        nc.vector.memset(ones_f[:], 1.0)
        ones = consts.tile([P, 1], R, tag="ones")
        nc.vector.tensor_copy(out=ones[:], in_=ones_f[:])
        ones1_f = consts.tile([1, P], F, tag="ones1_f")
        nc.vector.memset(ones1_f[:], 1.0)
        ones1 = consts.tile([1, P], R, tag="ones1")
        nc.vector.tensor_copy(out=ones1[:], in_=ones1_f[:])
        eps1 = consts.tile([1, 1], F, tag="eps1")
        nc.vector.memset(eps1[:], LN_EPS)
        bq_sb = consts.tile([P, NC], F, tag="bq"); nc.sync.dma_start(bq_sb[:], bq[:])
        bk_sb = consts.tile([P, NC], F, tag="bk"); nc.sync.dma_start(bk_sb[:], bk[:])

        # ===== Phase A: fused V-proj + rope + K-proj per 512-token span =====
        xt_r = xt.rearrange("(c p) s -> p c s", p=P)
        attn_cm = tc.tile_pool(name="attn_res", bufs=1)
        attn_res = attn_cm.__enter__()
        kTs = [attn_res.tile([P, kv_tok], R, tag=f"kT{oc}", name=f"kT{oc}")
               for oc in range(NC)]
        qT = attn_res.tile([P, NC, QL], R, tag="qT")
        qx_cm = tc.tile_pool(name="qx_p", bufs=1)
        qx_p = qx_cm.__enter__()
        qx = qx_p.tile([P, NC, QL], R, tag="qx")

        with tc.tile_pool(name="xsp_p", bufs=2) as xsp_p, \
             tc.tile_pool(name="cs_p", bufs=1) as cs_p, \
             tc.tile_pool(name="a1tmp", bufs=1) as a1tmp, \
             tc.tile_pool(name="wvkp", bufs=1) as wvkp, \
             tc.tile_pool(name="vev_p", bufs=1) as vev_p, \
             tc.tile_pool(name="vps", bufs=4, space="PSUM") as vps, \
             tc.tile_pool(name="kps", bufs=3, space="PSUM") as kps:
            xsp_t, cs_t = {}, {}

            def emit_dma(sp):
                s0, sl = kv_spans[sp]
                xsp = xsp_p.tile([P, NC, 512], R, tag="xsp")
                nc.sync.dma_start(xsp[:, :, :sl], xt_r[:, :, s0:s0 + sl])
                csb = cs_p.tile([P, 2, 512], F, tag="csb")
                nc.sync.dma_start(csb[:, 0, :sl], cos2[:, s0:s0 + sl])
                nc.sync.dma_start(csb[:, 1, :sl], sin2[:, s0:s0 + sl])
                xsp_t[sp] = xsp
                cs_t[sp] = csb

            def emit_V(sp):
                s0, sl = kv_spans[sp]
                xsp = xsp_t[sp]
                for t in range(sl // P):
                    for dvs in range(2):
                        vp = vps.tile([P, 512], F, tag="vps")
                        for dc in range(NC):
                            nc.tensor.matmul(
                                vp[:], xsp[:, dc, t * P:(t + 1) * P],
                                wv_sb[:, dc, dvs * 512:(dvs + 1) * 512],
                                start=(dc == 0), stop=False)
                        nc.tensor.matmul(
                            vp[:], ones1[:, :], bv_sb[:, dvs * 512:(dvs + 1) * 512],
                            start=False, stop=True)
                        vev = vev_p.tile([P, 512], R, tag="vev")
                        nc.scalar.copy(out=vev[:], in_=vp[:])
                        nc.sync.dma_start(
                            vsch[dvs][s0 + t * P:s0 + (t + 1) * P, :], vev[:])

            def emit_rope(sp):
                s0, sl = kv_spans[sp]
                xsp, csb = xsp_t[sp], cs_t[sp]
                cs = csb[:, 0, :sl]; sn = csb[:, 1, :sl]
                for c in range(4):
                    # in-place rotation: a' = a*cos - b*sin; b' = a*sin + b*cos
                    t1 = a1tmp.tile([P, 512], F, tag="ropet1")
                    t2 = a1tmp.tile([P, 512], F, tag="ropet2")
                    a = xsp[:, c, :sl]; b = xsp[:, c + 4, :sl]
                    nc.vector.tensor_tensor(out=t1[:, :sl], in0=a, in1=sn, op=OP.mult)
                    nc.vector.tensor_tensor(out=t2[:, :sl], in0=b, in1=sn, op=OP.mult)
                    nc.vector.tensor_tensor(out=a, in0=a, in1=cs, op=OP.mult)
                    nc.vector.tensor_tensor(out=a, in0=a, in1=t2[:, :sl], op=OP.subtract)
                    nc.vector.tensor_tensor(out=b, in0=b, in1=cs, op=OP.mult)
                    nc.vector.tensor_tensor(out=b, in0=b, in1=t1[:, :sl], op=OP.add)
                # q block sp = global block qc+4*sp lives at cols qc*P within span sp
                for c in range(NC):
                    nc.vector.tensor_copy(
                        out=qx[:, c, sp * P:(sp + 1) * P],
                        in_=xsp[:, c, qc * P:(qc + 1) * P])

            def emit_K(sp):
                s0, sl = kv_spans[sp]
                xsp = xsp_t[sp]
                for oc in range(NC):
                    kp = kps.tile([P, 512], F, tag="kps")
                    for dc in range(NC):
                        nc.tensor.matmul(
                            kp[:, :sl], wk_sb[:, dc, oc * P:(oc + 1) * P],
                            xsp[:, dc, :sl],
                            start=(dc == 0), stop=(dc == NC - 1))
                    nc.vector.tensor_scalar(
                        out=kTs[oc][:, s0:s0 + sl], in0=kp[:, :sl],
                        scalar1=bk_sb[:, oc:oc + 1], scalar2=None, op0=OP.add)

            # x/cos/sin for span 0 first, then wv, so V matmuls start ASAP;
            # wk loads while V(0) computes.
            emit_dma(0)
            wv_sb = wvkp.tile([P, NC, E], R, tag="wv")
            for c in range(NC):
                nc.sync.dma_start(wv_sb[:, c, :],
                                  wv.rearrange("(c p) m -> p c m", p=P)[:, c, :])
            bv_sb = wvkp.tile([1, E], R, tag="bv")
            nc.sync.dma_start(bv_sb[:], bvr[:])
            emit_V(0)
            wk_sb = wvkp.tile([P, NC, E], R, tag="wk")
            for c in range(NC):
                nc.sync.dma_start(wk_sb[:, c, :],
                                  wk.rearrange("(c p) m -> p c m", p=P)[:, c, :])
            emit_rope(0)
            emit_dma(1); emit_V(1); emit_K(0); emit_rope(1)
            emit_dma(2); emit_V(2); emit_K(1); emit_rope(2)
            emit_dma(3); emit_V(3); emit_K(2); emit_rope(3)
            emit_K(3)

        # ===== Q projection from roped q blocks in SBUF =====
        with tc.tile_pool(name="wqp", bufs=1) as wqp, \
             tc.tile_pool(name="qps_p", bufs=4, space="PSUM") as qps_p:
            wq_sb = wqp.tile([P, NC, E], R, tag="wq_sb")
            for c in range(NC):
                nc.sync.dma_start(wq_sb[:, c, :],
                                  wq.rearrange("(c p) m -> p c m", p=P)[:, c, :])
            for oc in range(NC):
                qp = qps_p.tile([P, 512], F, tag="qps")
                for dc in range(NC):
                    nc.tensor.matmul(
                        qp[:], wq_sb[:, dc, oc * P:(oc + 1) * P], qx[:, dc, :],
                        start=(dc == 0), stop=(dc == NC - 1))
                nc.vector.tensor_scalar(
                    out=qT[:, oc, :], in0=qp[:],
                    scalar1=bq_sb[:, oc:oc + 1], scalar2=None, op0=OP.add)
            if KDBG:
                nc.sync.dma_start(dbg_qx.rearrange("(c p) q -> p c q", p=P)[:], qx[:])
                nc.sync.dma_start(dbg_qt.rearrange("(c p) q -> p c q", p=P)[:], qT[:])
                nc.sync.dma_start(dbg_kt[:], kTs[0][:])
        qx_cm.__exit__(None, None, None)

        # ===== Phase B: causal attention over interleaved blocks =====
        # ctx stationary vt has a ones column per head half: row 64 of the
        # [65, QL] ctx psum accumulates the softmax denominator for free.
        ctxd_r = ctxd.rearrange("(c p) q -> p c q", p=P)
        kcs = [kc for kc in range(KCN) if -(-(kc - qc) // 4) < 4]
        with tc.tile_pool(name="bt", bufs=6) as bt, \
             tc.tile_pool(name="bconst", bufs=1) as bconst, \
             tc.tile_pool(name="vt_p", bufs=2) as vt_p, \
             tc.tile_pool(name="scps", bufs=4, space="PSUM") as scps, \
             tc.tile_pool(name="ctxps", bufs=4, space="PSUM") as ctxps:
            tri_sb = bconst.tile([P, P], R, tag="tri")
            nc.sync.dma_start(tri_sb[:], trid[:])
            ones65_f = bconst.tile([65, P], F, tag="ones65_f")
            nc.vector.memset(ones65_f[:], 1.0)
            ones65 = bconst.tile([65, P], R, tag="ones65")
            nc.vector.tensor_copy(out=ones65[:], in_=ones65_f[:])
            oneskc = bconst.tile([P, KCN], F, tag="oneskc")
            nc.vector.memset(oneskc[:], 1.0)
            zer128 = bconst.tile([P, P], F, tag="zer128")
            nc.vector.memset(zer128[:], 0.0)
            for hp in range(NC):
                vt = vt_p.tile([P, KCN, 130], R, tag="vt")
                for hh in range(2):
                    nc.sync.dma_start(
                        vt[:, :, hh * 65:hh * 65 + 64],
                        vsch[hp // 4].rearrange("(k p) d -> p k d", p=P)
                        [:, :, (hp % 4) * P + hh * 64:(hp % 4) * P + (hh + 1) * 64])
                    nc.vector.tensor_copy(out=vt[:, :, hh * 65 + 64:hh * 65 + 65],
                                          in_=oneskc[:])
                ctxp2 = [ctxps.tile([65, QL], F, tag="ctxps", name=f"ctxp_{hp}_{hh}")
                         for hh in range(2)]

                # software pipeline: issue scores for block i+1 before ctx of
                # block i, so ctx matmuls never stall on the Scalar exp.
                def emit_scores(kc):
                    j0 = max(0, -(-(kc - qc) // 4))
                    q0 = j0 * P
                    qn = max(512 - q0, 256)      # fp32r needs N>=256
                    c0 = 512 - qn
                    sts = []
                    for hh in range(2):
                        scp = scps.tile([P, QL], F, tag="scps")
                        nc.tensor.matmul(
                            scp[:, c0:], kTs[hp][hh * 64:(hh + 1) * 64, kc * P:(kc + 1) * P],
                            qT[hh * 64:(hh + 1) * 64, hp, c0:], start=True, stop=True)
                        st = bt.tile([P, QL], R, tag="st")
                        nc.scalar.activation(out=st[:, c0:], in_=scp[:, c0:],
                                             func=AF.Exp, scale=EXP_SCALE)
                        if c0 < q0:
                            nc.vector.tensor_copy(out=st[:, c0:q0], in_=zer128[:, :q0 - c0])
                        if kc >= qc and (kc - qc) % 4 == 0:
                            j = (kc - qc) // 4
                            nc.vector.tensor_tensor(
                                out=st[:, j * P:(j + 1) * P], in0=st[:, j * P:(j + 1) * P],
                                in1=tri_sb[:], op=OP.mult)
                        sts.append(st)
                    return (kc, c0, sts)

                def emit_ctx(pend):
                    kc, c0, sts = pend
                    for hh in range(2):
                        nc.tensor.matmul(ctxp2[hh][:, c0:],
                                         vt[:, kc, hh * 65:(hh + 1) * 65], sts[hh][:, c0:],
                                         start=(kc == kcs[0]), stop=(kc == kcs[-1]))

                pend = None
                for kc in kcs:
                    cur = emit_scores(kc)
                    if pend is not None:
                        emit_ctx(pend)
                    pend = cur
                emit_ctx(pend)
                ctxo = bt.tile([P, QL], R, tag="ctxo")
                for hh in range(2):
                    rden = bt.tile([65, QL], R, tag="rden")
                    nc.vector.reciprocal(out=rden[64:65, :], in_=ctxp2[hh][64:65, :])
                    rbp = scps.tile([P, QL], F, tag="scps")
                    nc.tensor.matmul(rbp[0:64, :], ones65[64:65, 0:64], rden[64:65, :],
                                     start=True, stop=True)
                    rb_sb = bt.tile([64, QL], R, tag="rb_sb")
                    nc.scalar.copy(out=rb_sb[:], in_=rbp[0:64, :])
                    nc.vector.tensor_tensor(
                        out=ctxo[hh * 64:(hh + 1) * 64, :],
                        in0=ctxp2[hh][0:64, :], in1=rb_sb[:], op=OP.mult)
                nc.sync.dma_start(ctxd_r[:, hp, :], ctxo[:])
        attn_cm.__exit__(None, None, None)

        # ===== LN helper =====
        def layernorm(src, dst, wtile, btile, tmp, ps_row, ps_big):
            sp_ = ps_row.tile([1, QL], F, tag="lnsum")
            for c in range(NC):
                nc.tensor.matmul(sp_[:], ones[:], src[:, c, :],
                                 start=(c == 0), stop=(c == NC - 1))
            s2p = ps_row.tile([1, QL], F, tag="lnsum2")
            for c in range(NC):
                sq = tmp.tile([P, QL], R, tag="lnsq")
                nc.scalar.activation(out=sq[:], in_=src[:, c, :], func=AF.Square)
                nc.tensor.matmul(s2p[:], ones[:], sq[:],
                                 start=(c == 0), stop=(c == NC - 1))
            mean = tmp.tile([1, QL], R, tag="lnmean")
            nc.scalar.mul(out=mean[:], in_=sp_[:], mul=1.0 / E)
            msq = tmp.tile([1, QL], R, tag="lnmsq")
            nc.scalar.mul(out=msq[:], in_=s2p[:], mul=1.0 / E)
            var = tmp.tile([1, QL], R, tag="lnvar")
            nc.vector.tensor_tensor(out=var[:], in0=mean[:], in1=mean[:], op=OP.mult)
            nc.vector.tensor_tensor(out=var[:], in0=msq[:], in1=var[:], op=OP.subtract)
            std = tmp.tile([1, QL], R, tag="lnstd")
            nc.scalar.activation(out=std[:], in_=var[:], func=AF.Sqrt, bias=eps1[:])
            rstd = tmp.tile([1, QL], R, tag="lnrstd")
            nc.vector.reciprocal(out=rstd[:], in_=std[:])
            mb = ps_big.tile([P, QL], F, tag="bigc")
            nc.tensor.matmul(mb[:], ones1[:], mean[:], start=True, stop=True)
            rb = ps_big.tile([P, QL], F, tag="bigc")
            nc.tensor.matmul(rb[:], ones1[:], rstd[:], start=True, stop=True)
            mbs = tmp.tile([P, QL], R, tag="lnmbs")
            nc.scalar.copy(out=mbs[:], in_=mb[:])
            rbs = tmp.tile([P, QL], R, tag="lnrbs")
            nc.scalar.copy(out=rbs[:], in_=rb[:])
            for c in range(NC):
                t = tmp.tile([P, QL], R, tag="lnt")
                nc.vector.tensor_tensor(out=t[:], in0=src[:, c, :], in1=mbs[:], op=OP.subtract)
                nc.vector.tensor_tensor(out=t[:], in0=t[:], in1=rbs[:], op=OP.mult)
                nc.vector.tensor_scalar(out=dst[:, c, :], in0=t[:],
                                        scalar1=wtile[:, c:c + 1], scalar2=btile[:, c:c + 1],
                                        op0=OP.mult, op1=OP.add)

        # ===== Phase C: out-proj + LN1 + gates + top-2 weights =====
        cres = es.enter_context(tc.tile_pool(name="cres", bufs=1))
        x1 = cres.tile([P, NC, QL], R, tag="x1")
        x1b = cres.tile([P, NC, QL], BF, tag="x1b")
        wbc = cres.tile([P, NE, QL], BF, tag="wbc")
        wT_sb = cres.tile([NE, QL], R, tag="wT")
        ebT_sb = cres.tile([NE, E], R, tag="ebT"); nc.sync.dma_start(ebT_sb[:], ebT[:])
        ln_sb = {}
        for nm, t in (("ln1w", ln1w), ("ln1b", ln1b), ("ln2w", ln2w), ("ln2b", ln2b)):
            ln_sb[nm] = cres.tile([P, NC], F, tag=nm, name=nm)
            nc.sync.dma_start(ln_sb[nm][:], t[:])
        with tc.tile_pool(name="cslab", bufs=1) as cslab, \
             tc.tile_pool(name="ct", bufs=2) as ct, \
             tc.tile_pool(name="h1p", bufs=1) as h1p, \
             tc.tile_pool(name="cps", bufs=3, space="PSUM") as cps, \
             tc.tile_pool(name="crow", bufs=1, space="PSUM") as crow, \
             tc.tile_pool(name="cg", bufs=1, space="PSUM") as cg:
            wo_sb = cslab.tile([P, NC, E], R, tag="wo_sb")
            for c in range(NC):
                nc.sync.dma_start(wo_sb[:, c, :],
                                  wo.rearrange("(c p) m -> p c m", p=P)[:, c, :])
            ctx_sb = cslab.tile([P, NC, QL], R, tag="ctx")
            nc.sync.dma_start(ctx_sb[:], ctxd_r[:])
            bo_sb = cslab.tile([P, NC], F, tag="bo"); nc.sync.dma_start(bo_sb[:], bo[:])
            gb_sb = cslab.tile([NE, 1], F, tag="gb"); nc.sync.dma_start(gb_sb[:], gb[:])
            gw_sb = cslab.tile([P, NC, NE], R, tag="gw")
            nc.sync.dma_start(gw_sb[:], gw.rearrange("(c p) g -> p c g", p=P))
            sel8_sb = cslab.tile([NE, NE * P], R, tag="sel8")
            nc.sync.dma_start(sel8_sb[:], sel8[:])
            iden_f = cslab.tile([P, P], F, tag="iden_f")
            make_identity(nc, iden_f[:])
            iden = cslab.tile([P, P], R, tag="iden")
            nc.vector.tensor_copy(out=iden[:], in_=iden_f[:])
            xres_sb = h1p.tile([P, NC, QL], R, tag="xres")
            nc.sync.dma_start(xres_sb[:], xres.rearrange("(c p) q -> p c q", p=P))
            h1 = h1p.tile([P, NC, QL], R, tag="h1")
            for oc in range(NC):
                ap = cps.tile([P, QL], F, tag="bigc")
                for dc in range(NC):
                    nc.tensor.matmul(ap[:], wo_sb[:, dc, oc * P:(oc + 1) * P],
                                     ctx_sb[:, dc, :],
                                     start=(dc == 0), stop=(dc == NC - 1))
                nc.vector.tensor_scalar(out=h1[:, oc, :], in0=ap[:],
                                        scalar1=bo_sb[:, oc:oc + 1], scalar2=None, op0=OP.add)
                nc.vector.tensor_tensor(out=h1[:, oc, :], in0=h1[:, oc, :],
                                        in1=xres_sb[:, oc, :], op=OP.add)

            layernorm(h1, x1, ln_sb["ln1w"], ln_sb["ln1b"], ct, crow, cps)

            gp = cg.tile([NE, QL], F, tag="gps")
            for c in range(NC):
                nc.tensor.matmul(gp[:], gw_sb[:, c, :], x1[:, c, :],
                                 start=(c == 0), stop=(c == NC - 1))
            glog = ct.tile([NE, QL], R, tag="glog")
            nc.vector.tensor_scalar(out=glog[:], in0=gp[:], scalar1=gb_sb[:],
                                    scalar2=None, op0=OP.add)
            gexp = ct.tile([NE, QL], R, tag="gexp")
            nc.scalar.activation(out=gexp[:], in_=glog[:], func=AF.Exp)

            # token-orientation gate math: gt[tok, qb, e]
            gt = ct.tile([P, 4, NE], R, tag="gt")
            for qb in range(4):
                gtp = cg.tile([P, NE], R, tag="gmix")
                nc.tensor.transpose(gtp[:], gexp[:, qb * P:(qb + 1) * P], iden[0:NE, 0:NE])
                nc.vector.tensor_copy(out=gt[:, qb, :], in_=gtp[:])
            dsum = ct.tile([P, 4], F, tag="dsum")
            nc.vector.reduce_sum(out=dsum[:], in_=gt[:], axis=AX)
            rgn = ct.tile([P, 4], F, tag="rgn")
            nc.vector.reciprocal(out=rgn[:], in_=dsum[:])
            m1 = ct.tile([P, 4], F, tag="m1")
            nc.vector.reduce_max(out=m1[:], in_=gt[:], axis=AX)
            msel = ct.tile([P, 4, NE], R, tag="msel")
            g2 = ct.tile([P, 4, NE], R, tag="g2")
            for qb in range(4):
                nc.vector.tensor_scalar(out=msel[:, qb, :], in0=gt[:, qb, :],
                                        scalar1=m1[:, qb:qb + 1], scalar2=None, op0=OP.is_equal)
            nc.vector.tensor_tensor(out=g2[:], in0=gt[:], in1=msel[:], op=OP.mult)
            nc.vector.tensor_tensor(out=g2[:], in0=gt[:], in1=g2[:], op=OP.subtract)
            m2 = ct.tile([P, 4], F, tag="m2")
            nc.vector.reduce_max(out=m2[:], in_=g2[:], axis=AX)
            msel2 = ct.tile([P, 4, NE], R, tag="msel2")
            for qb in range(4):
                nc.vector.tensor_scalar(out=msel2[:, qb, :], in0=g2[:, qb, :],
                                        scalar1=m2[:, qb:qb + 1], scalar2=None, op0=OP.is_equal)
            nc.vector.tensor_tensor(out=msel[:], in0=msel[:], in1=msel2[:], op=OP.add)
            wsel = ct.tile([P, 4, NE], R, tag="wsel")
            nc.vector.tensor_tensor(out=wsel[:], in0=gt[:], in1=msel[:], op=OP.mult)
            for qb in range(4):
                nc.vector.tensor_scalar(out=wsel[:, qb, :], in0=wsel[:, qb, :],
                                        scalar1=rgn[:, qb:qb + 1], scalar2=None, op0=OP.mult)

            # transpose per q-block -> wT_sb [NE, QL] (bf16)
            for qb in range(4):
                wtp = cg.tile([NE, P], R, tag="gmix")
                nc.tensor.transpose(wtp[:], wsel[:, qb, :], iden[:])
                nc.vector.tensor_copy(out=wT_sb[:, qb * P:(qb + 1) * P], in_=wtp[:])
            if KDBG:
                nc.sync.dma_start(dbg_x1.rearrange("(c p) q -> p c q", p=P)[:], x1[:])
                nc.sync.dma_start(dbg_wt[:], wT_sb[:])
            # broadcast per expert -> wbc [P, e, QL] (bf16)
            for e in range(NE):
                bcp = cps.tile([P, QL], F, tag="bigc")
                nc.tensor.matmul(bcp[:], sel8_sb[:, e * P:(e + 1) * P], wT_sb[:],
                                 start=True, stop=True)
                nc.vector.tensor_copy(out=wbc[:, e, :], in_=bcp[:])
            for c in range(NC):
                nc.vector.tensor_copy(out=x1b[:, c, :], in_=x1[:, c, :])

        # ===== Phase D: dense MoE with gate-scaled inputs, DVE accumulate =====
        moe = cres.tile([P, NC, QL], R, tag="moe")
        with tc.tile_pool(name="ewp", bufs=2) as ewp, \
             tc.tile_pool(name="xwp", bufs=2) as xwp, \
             tc.tile_pool(name="dps", bufs=4, space="PSUM") as dps:
            for e in range(NE):
                ew_sl = ewp.tile([P, NC, E], BF, tag="ew_sl")
                nc.sync.dma_start(ew_sl[:], ew[e].rearrange("c p d -> p c d"))
                xw = xwp.tile([P, NC, QL], BF, tag="xw")
                for dc in range(NC):
                    nc.vector.tensor_tensor(out=xw[:, dc, :], in0=x1b[:, dc, :],
                                            in1=wbc[:, e, :], op=OP.mult)
                for oc in range(NC):
                    yp = dps.tile([P, QL], F, tag="yps")
                    if e == 0:
                        nc.tensor.matmul(yp[:], ebT_sb[:, oc * P:(oc + 1) * P],
                                         wT_sb[:], start=True, stop=False)
                    for dc in range(NC):
                        nc.tensor.matmul(
                            yp[:], ew_sl[:, dc, oc * P:(oc + 1) * P], xw[:, dc, :],
                            start=(e != 0 and dc == 0), stop=(dc == NC - 1))
                    if e == 0:
                        # fold in the x1 residual for LN2 up front
                        nc.vector.tensor_tensor(out=moe[:, oc, :], in0=x1[:, oc, :],
                                                in1=yp[:], op=OP.add)
                    else:
                        nc.vector.tensor_tensor(out=moe[:, oc, :], in0=moe[:, oc, :],
                                                in1=yp[:], op=OP.add)

        if KDBG:
            nc.sync.dma_start(dbg_moe.rearrange("(c p) q -> p c q", p=P)[:], moe[:])
            nc.sync.dma_start(dbg_wbc.rearrange("p (e q) -> p e q", e=NE)[:], wbc[:])

        # ===== Phase E: LN2 + store =====
        with tc.tile_pool(name="et", bufs=3) as et, \
             tc.tile_pool(name="eps_", bufs=2, space="PSUM") as eps_, \
             tc.tile_pool(name="erow", bufs=1, space="PSUM") as erow:
            layernorm(moe, moe, ln_sb["ln2w"], ln_sb["ln2b"], et, erow, eps_)
            for c in range(NC):
                nc.sync.dma_start(out.rearrange("(c p) q -> p c q", p=P)[:, c, :], moe[:, c, :])

    nc.compile()
    return nc


def _prep_inputs(inputs):
    x = np.asarray(inputs['x'], dtype=np.float32)
    ipw = np.asarray(inputs['in_proj_w'], dtype=np.float32)
    ipb = np.asarray(inputs['in_proj_b'], dtype=np.float32)
    opw = np.asarray(inputs['out_proj_w'], dtype=np.float32)
    opb = np.asarray(inputs['out_proj_b'], dtype=np.float32)
    gww = np.asarray(inputs['gate_w'], dtype=np.float32)
    gbb = np.asarray(inputs['gate_b'], dtype=np.float32)
    eww = np.asarray(inputs['expert_w'], dtype=np.float32)
    ebb = np.asarray(inputs['expert_b'], dtype=np.float32)

    perm = np.empty(E, dtype=np.int64)
    idx = 0
    for h in range(H):
        for i in range(D // 2):
            perm[idx] = 64 * h + 2 * i; idx += 1
    for h in range(H):
        for i in range(D // 2):
            perm[idx] = 64 * h + 2 * i + 1; idx += 1

    Wq, Wk, Wv = ipw[0:E], ipw[E:2 * E], ipw[2 * E:3 * E]
    bq_, bk_, bv_ = ipb[0:E], ipb[E:2 * E], ipb[2 * E:3 * E]
    sel8 = np.zeros((NE, NE * P), dtype=np.float32)
    for e in range(NE):
        sel8[e, e * P:(e + 1) * P] = 1.0
    common = {
        "wq": np.ascontiguousarray(Wq[:, perm].T),
        "wk": np.ascontiguousarray(Wk[:, perm].T),
        "wv": np.ascontiguousarray(Wv[:, perm].T),
        "bq": np.ascontiguousarray(bq_.reshape(NC, P).T),
        "bk": np.ascontiguousarray(bk_.reshape(NC, P).T),
        "bvr": bv_.reshape(1, E).copy(),
        "wo": np.ascontiguousarray(opw.T),
        "bo": np.ascontiguousarray(opb.reshape(NC, P).T),
        "gw": np.ascontiguousarray(gww.T),
        "gb": gbb.reshape(NE, 1).copy(),
        "ew": np.ascontiguousarray(
            eww.transpose(0, 2, 1).reshape(NE, NC, P, E)).astype(ml_dtypes.bfloat16),
        "ebT": ebb,
        "sel8": sel8,
        "ln1w": np.ascontiguousarray(np.asarray(inputs['ln1_w'], np.float32).reshape(NC, P).T),
        "ln1b": np.ascontiguousarray(np.asarray(inputs['ln1_b'], np.float32).reshape(NC, P).T),
        "ln2w": np.ascontiguousarray(np.asarray(inputs['ln2_w'], np.float32).reshape(NC, P).T),
        "ln2b": np.ascontiguousarray(np.asarray(inputs['ln2_b'], np.float32).reshape(NC, P).T),
        "trid": np.ascontiguousarray(
            (np.arange(P)[None, :] >= np.arange(P)[:, None]).astype(np.float32)),
    }
    inv_freq = 1.0 / (10000.0 ** (np.arange(0, D, 2, dtype=np.float64) / D))
    freqs = np.arange(S, dtype=np.float64)[:, None] * inv_freq[None, :]
    cos_t = np.cos(freqs).T.astype(np.float32)
    sin_t = np.sin(freqs).T.astype(np.float32)
    common["cos2"] = np.ascontiguousarray(np.tile(cos_t, (4, 1)))
    common["sin2"] = np.ascontiguousarray(np.tile(sin_t, (4, 1)))

    in_maps = []
    for c in range(8):
        b, qc = c // 4, c % 4
        blocks = [qc + 4 * i for i in range(4)]
        cols = np.concatenate([np.arange(blk * P, (blk + 1) * P) for blk in blocks])
        xtb = np.ascontiguousarray(x[b].T)
        xtp = np.ascontiguousarray(xtb[perm])
        m = dict(common)
        m["xt"] = xtp
        m["xres"] = np.ascontiguousarray(xtb[:, cols])
        in_maps.append(m)
    return in_maps


def _run_multi(ncs, in_maps):
    """Run the 4 NEFFs concurrently: graph qc on devices {qc, qc+4} (b=0,1)."""
    import jax
    from jax.sharding import Mesh, PartitionSpec
    from jax.experimental.shard_map import shard_map
    from concourse import bass2jax
    from concourse import mybir as _mb

    bass2jax.install_neuronx_cc_hook()
    devices = jax.devices()

    if "jits" not in _cache:
        _cache["jits"] = {}
    handles = []
    for qc in range(4):
        nc = ncs[qc]
        if qc not in _cache["jits"]:
            in_names, out_names, out_avals, zero_outs = [], [], [], []
            for alloc in nc.m.functions[0].allocations:
                if not isinstance(alloc, _mb.MemoryLocationSet):
                    continue
                name = alloc.memorylocations[0].name
                if alloc.kind == "ExternalInput":
                    in_names.append(name)
                elif alloc.kind == "ExternalOutput":
                    out_names.append(name)
                    shape = tuple(alloc.tensor_shape)
                    dtype = _mb.dt.np(alloc.dtype)
                    out_avals.append(jax.core.ShapedArray(shape, dtype))
                    zero_outs.append(np.zeros(shape, dtype))
            n_params = len(in_names)
            all_names = in_names + out_names
            donate = tuple(range(n_params, n_params + len(out_names)))

            def _body(*args, _nc=nc, _avals=tuple(out_avals), _all=tuple(all_names),
                      _outs=tuple(out_names)):
                outs = bass2jax._bass_exec_p.bind(
                    *args, out_avals=_avals, in_names=_all, out_names=_outs,
                    lowering_input_output_aliases=(),
                    sim_require_finite=True, sim_require_nnan=True, nc=_nc)
                return tuple(outs)

            devs = [devices[qc], devices[qc + 4]]
            mesh = Mesh(np.asarray(devs), ("core",))
            nio = n_params + len(zero_outs)
            sharded = jax.jit(
                shard_map(_body, mesh=mesh,
                          in_specs=(PartitionSpec("core"),) * nio,
                          out_specs=(PartitionSpec("core"),) * len(out_names),
                          check_rep=False),
                donate_argnums=donate, keep_unused=True)
            _cache["jits"][qc] = (sharded, in_names, out_names, zero_outs)
        sharded, in_names, out_names, zero_outs = _cache["jits"][qc]
        per_core = [[np.asarray(in_maps[b * 4 + qc][n]) for n in in_names] for b in range(2)]
        concat_in = [np.concatenate([per_core[b][i] for b in range(2)], axis=0)
                     for i in range(len(in_names))]
        concat_zero = [np.concatenate([z, z], axis=0) for z in zero_outs]
        handles.append((sharded, concat_in, concat_zero, out_names))

    outs = []
    for sharded, concat_in, concat_zero, out_names in handles:
        outs.append((sharded(*concat_in, *concat_zero), out_names))
    results = [None] * 8
    for qc, (arrs, out_names) in enumerate(outs):
        arrs = [np.asarray(a) for a in arrs]
        for b in range(2):
            rm = {}
            for i, n in enumerate(out_names):
                full = arrs[i]
                half = full.shape[0] // 2
                rm[n] = full[b * half:(b + 1) * half]
            results[b * 4 + qc] = rm
    return results


def _ensure_ntff_hook():
    import types
    try:
        from antenv.axon_hooks import get_axon_ntff_profile_hook  # noqa
        return True
    except ImportError:
        pass
    try:
        import antenv
        sys.path.insert(0, '/root/.axon_site')
        from trn_agent_boot.trn_boot import _ntff_profile_via_ctypes
        hook = _ntff_profile_via_ctypes('/opt/axon/libaxon_pjrt.so')
        if hook is None:
            return False
        mod = types.ModuleType('antenv.axon_hooks')
        _state = {'hook': hook}
        mod.set_axon_ntff_profile_hook = lambda h: _state.__setitem__('hook', h)
        mod.get_axon_ntff_profile_hook = lambda: _state['hook']
        sys.modules['antenv.axon_hooks'] = mod
        antenv.axon_hooks = mod
        return True
    except Exception as e:
        print(f"ntff hook setup failed: {e}")
        return False


def kernel(**inputs):
    if "ncs" not in _cache:
        _cache["ncs"] = [_build(qc) for qc in range(4)]
    ncs = _cache["ncs"]
    in_maps = _prep_inputs(inputs)

    trace = bool(int(os.environ.get("KERNEL_TRACE", "0")))
    if trace and _ensure_ntff_hook():
        import tempfile
        from antenv.axon_hooks import get_axon_ntff_profile_hook
        hook = get_axon_ntff_profile_hook()
        tmpdir = tempfile.mkdtemp()
        _run_multi(ncs, in_maps)  # warm-up/compile outside the profile window
        with hook(tmpdir, list(range(8))):
            results = _run_multi(ncs, in_maps)
        _cache["ntff_dir"] = tmpdir
        print(f"ntff dir: {tmpdir}")
    else:
        results = _run_multi(ncs, in_maps)
    _cache["last_results"] = results

    out = np.empty((B, S, E), dtype=np.float32)
    for c in range(8):
        b, qc = c // 4, c % 4
        o = results[c]["out"]  # [E, QL]
        for i in range(4):
            blk = qc + 4 * i
            out[b, blk * P:(blk + 1) * P, :] = o[:, i * P:(i + 1) * P].T
    return out


# revision 41
# speedup vs baseline: 1.2310x; 1.0729x over previous
"""Trainium2 Bass kernel for nn_EnhancedTransformerLayer (RoPE attention + MoE).

Sharding: 8 cores; core c -> batch b=c//4, qc=c%4. Four distinct NEFFs (one per
qc), each run on 2 cores (b=0,1). Core qc owns interleaved query blocks
{qc, qc+4, qc+8, qc+12} (4 x 128 tokens) so causal work is balanced, and only
computes K/V up to its last block.

v2: fused rope/V/K span pipeline (single x read, in-place rope, no DRAM
round-trip), softmax denominator via ones-column in the ctx stationary,
fp32r matmuls clamped to N>=256, MoE accumulated directly in PSUM across
experts with gate-scaled inputs (no gpsimd, no per-expert combine).
"""
import sys, os
sys.path.insert(0, '/opt/trn_rl_repo')
import numpy as np
import ml_dtypes

import concourse.bass as bass
from concourse import bacc
import concourse.tile as tile
from concourse import mybir
from concourse.masks import make_identity

R = mybir.dt.float32r
F = mybir.dt.float32
BF = mybir.dt.bfloat16
P = 128
B, S, E, H, D, NE = 2, 2048, 1024, 16, 64, 8
NC = E // P
QL = 512
EXP_SCALE = 1.0 / (D ** 0.5)
LN_EPS = 1e-5

_cache = {}


def _kv_plan(qc):
    """K/V token coverage and span split for this qc."""
    kv_tok = 128 * (qc + 13)            # last interleaved block end
    rem = kv_tok - 1536
    rem = max(rem, 256)                 # fp32r needs N>=256
    kv_tok = 1536 + rem
    spans = [(0, 512), (512, 512), (1024, 512), (1536, rem)]
    return kv_tok, spans


def _build(qc):
    nc = bacc.Bacc("TRN2", target_bir_lowering=False, debug=False, num_devices=8,
                   name=f"moe_qc{qc}", enable_partition_id=False)
    kv_tok, kv_spans = _kv_plan(qc)
    KCN = kv_tok // P                   # key chunk count

    def din(name, shape, dt=R):
        return nc.dram_tensor(name, shape, dt, kind="ExternalInput")

    xt = din("xt", [E, S])              # permuted x^T (f32r bits == f32)
    xres = din("xres", [E, QL])
    wq = din("wq", [E, E]); wk = din("wk", [E, E]); wv = din("wv", [E, E])
    bq = din("bq", [P, NC], F); bk = din("bk", [P, NC], F)
    bvr = din("bvr", [1, E])
    wo = din("wo", [E, E]); bo = din("bo", [P, NC], F)
    gw = din("gw", [E, NE]); gb = din("gb", [NE, 1], F)
    cos2 = din("cos2", [P, S], F); sin2 = din("sin2", [P, S], F)
    trid = din("trid", [P, P])          # tri[k, q] = 1 if q >= k (within a block)
    ew = din("ew", [NE, NC, P, E], BF)
    ebT = din("ebT", [NE, E])           # expert bias, experts on partitions
    sel8 = din("sel8", [NE, NE * P])    # sel8[:, e*P:(e+1)*P] row e = 1
    ln1w = din("ln1w", [P, NC], F); ln1b = din("ln1b", [P, NC], F)
    ln2w = din("ln2w", [P, NC], F); ln2b = din("ln2b", [P, NC], F)
    out = nc.dram_tensor("out", [E, QL], R, kind="ExternalOutput")
    KDBG = bool(int(os.environ.get("KDBG", "0")))
    dbg_kind = "ExternalOutput" if KDBG else "Internal"
    vsch = [nc.dram_tensor(f"vsc{i}", [kv_tok, 512], BF,
                           kind=dbg_kind) for i in range(2)]
    ctxd = nc.dram_tensor("ctxd", [E, QL], R, kind=dbg_kind)
    if KDBG:
        dbg_qx = nc.dram_tensor("dbg_qx", [E, QL], R, kind="ExternalOutput")
        dbg_qt = nc.dram_tensor("dbg_qt", [E, QL], BF, kind="ExternalOutput")
        dbg_kt = nc.dram_tensor("dbg_kt", [P, kv_tok], BF, kind="ExternalOutput")
        dbg_x1 = nc.dram_tensor("dbg_x1", [E, QL], R, kind="ExternalOutput")
        dbg_wt = nc.dram_tensor("dbg_wt", [NE, QL], R, kind="ExternalOutput")
        dbg_wbc = nc.dram_tensor("dbg_wbc", [P, NE * QL], BF, kind="ExternalOutput")
        dbg_moe = nc.dram_tensor("dbg_moe", [E, QL], R, kind="ExternalOutput")

    AX = mybir.AxisListType.X
    OP = mybir.AluOpType
    AF = mybir.ActivationFunctionType
    import contextlib

    with tile.TileContext(nc) as tc, \
         nc.allow_low_precision(reason="float32r is bit-identical to float32"), \
         contextlib.ExitStack() as es:
        consts = es.enter_context(tc.tile_pool(name="consts", bufs=1))

        ones_f = consts.tile([P, 1], F, tag="ones_f")
        nc.vector.memset(ones_f[:], 1.0)
        ones = consts.tile([P, 1], R, tag="ones")
        nc.vector.tensor_copy(out=ones[:], in_=ones_f[:])
        ones1_f = consts.tile([1, P], F, tag="ones1_f")
        nc.vector.memset(ones1_f[:], 1.0)
        ones1 = consts.tile([1, P], R, tag="ones1")
        nc.vector.tensor_copy(out=ones1[:], in_=ones1_f[:])
        eps1 = consts.tile([1, 1], F, tag="eps1")
        nc.vector.memset(eps1[:], LN_EPS)
        bq_sb = consts.tile([P, NC], F, tag="bq"); nc.sync.dma_start(bq_sb[:], bq[:])
        bk_sb = consts.tile([P, NC], F, tag="bk"); nc.sync.dma_start(bk_sb[:], bk[:])

        # ===== Phase A: fused V-proj + rope + K-proj per 512-token span =====
        xt_r = xt.rearrange("(c p) s -> p c s", p=P)
        attn_cm = tc.tile_pool(name="attn_res", bufs=1)
        attn_res = attn_cm.__enter__()
        kTs = [attn_res.tile([P, kv_tok], BF, tag=f"kT{oc}", name=f"kT{oc}")
               for oc in range(NC)]
        qT = attn_res.tile([P, NC, QL], BF, tag="qT")
        qx_cm = tc.tile_pool(name="qx_p", bufs=1)
        qx_p = qx_cm.__enter__()
        qx = qx_p.tile([P, NC, QL], R, tag="qx")

        with tc.tile_pool(name="xsp_p", bufs=2) as xsp_p, \
             tc.tile_pool(name="cs_p", bufs=2) as cs_p, \
             tc.tile_pool(name="a1tmp", bufs=1) as a1tmp, \
             tc.tile_pool(name="wvkp", bufs=1) as wvkp, \
             tc.tile_pool(name="vev_p", bufs=2) as vev_p, \
             tc.tile_pool(name="vps", bufs=4, space="PSUM") as vps, \
             tc.tile_pool(name="kps", bufs=3, space="PSUM") as kps:
            xsp_t, cs_t = {}, {}

            def emit_dma(sp):
                s0, sl = kv_spans[sp]
                xsp = xsp_p.tile([P, NC, 512], R, tag="xsp")
                nc.sync.dma_start(xsp[:, :, :sl], xt_r[:, :, s0:s0 + sl])
                csb = cs_p.tile([P, 2, 512], F, tag="csb")
                nc.sync.dma_start(csb[:, 0, :sl], cos2[:, s0:s0 + sl])
                nc.sync.dma_start(csb[:, 1, :sl], sin2[:, s0:s0 + sl])
                xsp_t[sp] = xsp
                cs_t[sp] = csb

            def emit_V(sp):
                s0, sl = kv_spans[sp]
                xsp = xsp_t[sp]
                for t in range(sl // P):
                    for dvs in range(2):
                        vp = vps.tile([P, 512], F, tag="vps")
                        for dc in range(NC):
                            nc.tensor.matmul(
                                vp[:], xsp[:, dc, t * P:(t + 1) * P],
                                wv_sb[:, dc, dvs * 512:(dvs + 1) * 512],
                                start=(dc == 0), stop=False)
                        nc.tensor.matmul(
                            vp[:], ones1[:, :], bv_sb[:, dvs * 512:(dvs + 1) * 512],
                            start=False, stop=True)
                        vev = vev_p.tile([P, 512], BF, tag="vev")
                        nc.scalar.copy(out=vev[:], in_=vp[:])
                        nc.sync.dma_start(
                            vsch[dvs][s0 + t * P:s0 + (t + 1) * P, :], vev[:])

            def emit_rope(sp):
                s0, sl = kv_spans[sp]
                xsp, csb = xsp_t[sp], cs_t[sp]
                cs = csb[:, 0, :sl]; sn = csb[:, 1, :sl]
                for c in range(4):
                    # in-place rotation: a' = a*cos - b*sin; b' = a*sin + b*cos
                    t1 = a1tmp.tile([P, 512], F, tag="ropet1")
                    t2 = a1tmp.tile([P, 512], F, tag="ropet2")
                    a = xsp[:, c, :sl]; b = xsp[:, c + 4, :sl]
                    nc.vector.tensor_tensor(out=t1[:, :sl], in0=a, in1=sn, op=OP.mult)
                    nc.vector.tensor_tensor(out=t2[:, :sl], in0=b, in1=sn, op=OP.mult)
                    nc.vector.tensor_tensor(out=a, in0=a, in1=cs, op=OP.mult)
                    nc.vector.tensor_tensor(out=a, in0=a, in1=t2[:, :sl], op=OP.subtract)
                    nc.vector.tensor_tensor(out=b, in0=b, in1=cs, op=OP.mult)
                    nc.vector.tensor_tensor(out=b, in0=b, in1=t1[:, :sl], op=OP.add)
                # q block sp = global block qc+4*sp lives at cols qc*P within span sp
                for c in range(NC):
                    nc.vector.tensor_copy(
                        out=qx[:, c, sp * P:(sp + 1) * P],
                        in_=xsp[:, c, qc * P:(qc + 1) * P])

            def emit_K(sp):
                s0, sl = kv_spans[sp]
                xsp = xsp_t[sp]
                for oc in range(NC):
                    kp = kps.tile([P, 512], F, tag="kps")
                    for dc in range(NC):
                        nc.tensor.matmul(
                            kp[:, :sl], wk_sb[:, dc, oc * P:(oc + 1) * P],
                            xsp[:, dc, :sl],
                            start=(dc == 0), stop=(dc == NC - 1))
                    nc.vector.tensor_scalar(
                        out=kTs[oc][:, s0:s0 + sl], in0=kp[:, :sl],
                        scalar1=bk_sb[:, oc:oc + 1], scalar2=None, op0=OP.add)

            # x/cos/sin for span 0 first, then wv, so V matmuls start ASAP;
            # wk loads while V(0) computes.
            emit_dma(0)
            wv_sb = wvkp.tile([P, NC, E], R, tag="wv")
            for c in range(NC):
                nc.sync.dma_start(wv_sb[:, c, :],
                                  wv.rearrange("(c p) m -> p c m", p=P)[:, c, :])
            bv_sb = wvkp.tile([1, E], R, tag="bv")
            nc.sync.dma_start(bv_sb[:], bvr[:])
            emit_V(0)
            wk_sb = wvkp.tile([P, NC, E], R, tag="wk")
            for c in range(NC):
                nc.sync.dma_start(wk_sb[:, c, :],
                                  wk.rearrange("(c p) m -> p c m", p=P)[:, c, :])
            emit_rope(0)
            emit_dma(1); emit_V(1); emit_K(0); emit_rope(1)
            emit_dma(2); emit_V(2); emit_K(1); emit_rope(2)
            emit_dma(3); emit_V(3); emit_K(2); emit_rope(3)
            emit_K(3)

        # ===== Q projection from roped q blocks in SBUF =====
        with tc.tile_pool(name="wqp", bufs=1) as wqp, \
             tc.tile_pool(name="qps_p", bufs=4, space="PSUM") as qps_p:
            wq_sb = wqp.tile([P, NC, E], R, tag="wq_sb")
            for c in range(NC):
                nc.sync.dma_start(wq_sb[:, c, :],
                                  wq.rearrange("(c p) m -> p c m", p=P)[:, c, :])
            for oc in range(NC):
                qp = qps_p.tile([P, 512], F, tag="qps")
                for dc in range(NC):
                    nc.tensor.matmul(
                        qp[:], wq_sb[:, dc, oc * P:(oc + 1) * P], qx[:, dc, :],
                        start=(dc == 0), stop=(dc == NC - 1))
                nc.vector.tensor_scalar(
                    out=qT[:, oc, :], in0=qp[:],
                    scalar1=bq_sb[:, oc:oc + 1], scalar2=None, op0=OP.add)
            if KDBG:
                nc.sync.dma_start(dbg_qx.rearrange("(c p) q -> p c q", p=P)[:], qx[:])
                nc.sync.dma_start(dbg_qt.rearrange("(c p) q -> p c q", p=P)[:], qT[:])
                nc.sync.dma_start(dbg_kt[:], kTs[0][:])
        qx_cm.__exit__(None, None, None)

        # ===== Phase B: causal attention over interleaved blocks =====
        # ctx stationary vt has a ones column per head half: row 64 of the
        # [65, QL] ctx psum accumulates the softmax denominator for free.
        ctxd_r = ctxd.rearrange("(c p) q -> p c q", p=P)
        kcs = [kc for kc in range(KCN) if -(-(kc - qc) // 4) < 4]
        with tc.tile_pool(name="bt", bufs=6) as bt, \
             tc.tile_pool(name="bconst", bufs=1) as bconst, \
             tc.tile_pool(name="vt_p", bufs=2) as vt_p, \
             tc.tile_pool(name="scps", bufs=4, space="PSUM") as scps, \
             tc.tile_pool(name="ctxps", bufs=4, space="PSUM") as ctxps:
            tri_f = bconst.tile([P, P], R, tag="tri_f")
            nc.sync.dma_start(tri_f[:], trid[:])
            tri_sb = bconst.tile([P, P], BF, tag="tri")
            nc.vector.tensor_copy(out=tri_sb[:], in_=tri_f[:])
            ones65_f = bconst.tile([65, P], F, tag="ones65_f")
            nc.vector.memset(ones65_f[:], 1.0)
            ones65 = bconst.tile([65, P], BF, tag="ones65")
            nc.vector.tensor_copy(out=ones65[:], in_=ones65_f[:])
            oneskc = bconst.tile([P, KCN], F, tag="oneskc")
            nc.vector.memset(oneskc[:], 1.0)
            for hp in range(NC):
                vt = vt_p.tile([P, KCN, 130], BF, tag="vt")
                for hh in range(2):
                    nc.sync.dma_start(
                        vt[:, :, hh * 65:hh * 65 + 64],
                        vsch[hp // 4].rearrange("(k p) d -> p k d", p=P)
                        [:, :, (hp % 4) * P + hh * 64:(hp % 4) * P + (hh + 1) * 64])
                    nc.vector.tensor_copy(out=vt[:, :, hh * 65 + 64:hh * 65 + 65],
                                          in_=oneskc[:])
                ctxp2 = [ctxps.tile([65, QL], F, tag="ctxps", name=f"ctxp_{hp}_{hh}")
                         for hh in range(2)]

                # software pipeline: issue scores for block i+1 before ctx of
                # block i, so ctx matmuls never stall on the Scalar exp.
                def emit_scores(kc):
                    j0 = max(0, -(-(kc - qc) // 4))
                    c0 = q0 = j0 * P
                    sts = []
                    for hh in range(2):
                        scp = scps.tile([P, QL], F, tag="scps")
                        nc.tensor.matmul(
                            scp[:, c0:], kTs[hp][hh * 64:(hh + 1) * 64, kc * P:(kc + 1) * P],
                            qT[hh * 64:(hh + 1) * 64, hp, c0:], start=True, stop=True)
                        st = bt.tile([P, QL], BF, tag="st")
                        nc.scalar.activation(out=st[:, c0:], in_=scp[:, c0:],
                                             func=AF.Exp, scale=EXP_SCALE)
                        if kc >= qc and (kc - qc) % 4 == 0:
                            j = (kc - qc) // 4
                            nc.vector.tensor_tensor(
                                out=st[:, j * P:(j + 1) * P], in0=st[:, j * P:(j + 1) * P],
                                in1=tri_sb[:], op=OP.mult)
                        sts.append(st)
                    return (kc, c0, sts)

                def emit_ctx(pend):
                    kc, c0, sts = pend
                    for hh in range(2):
                        nc.tensor.matmul(ctxp2[hh][:, c0:],
                                         vt[:, kc, hh * 65:(hh + 1) * 65], sts[hh][:, c0:],
                                         start=(kc == kcs[0]), stop=(kc == kcs[-1]))

                pend = None
                for kc in kcs:
                    cur = emit_scores(kc)
                    if pend is not None:
                        emit_ctx(pend)
                    pend = cur
                emit_ctx(pend)
                ctxo = bt.tile([P, QL], R, tag="ctxo")
                for hh in range(2):
                    rden = bt.tile([65, QL], BF, tag="rden")
                    nc.vector.reciprocal(out=rden[64:65, :], in_=ctxp2[hh][64:65, :])
                    rbp = scps.tile([P, QL], F, tag="scps")
                    nc.tensor.matmul(rbp[0:64, :], ones65[64:65, 0:64], rden[64:65, :],
                                     start=True, stop=True)
                    rb_sb = bt.tile([64, QL], R, tag="rb_sb")
                    nc.scalar.copy(out=rb_sb[:], in_=rbp[0:64, :])
                    nc.vector.tensor_tensor(
                        out=ctxo[hh * 64:(hh + 1) * 64, :],
                        in0=ctxp2[hh][0:64, :], in1=rb_sb[:], op=OP.mult)
                nc.sync.dma_start(ctxd_r[:, hp, :], ctxo[:])
        attn_cm.__exit__(None, None, None)

        # ===== LN helper =====
        def layernorm(src, dst, wtile, btile, tmp, ps_row, ps_big):
            sp_ = ps_row.tile([1, QL], F, tag="lnsum")
            for c in range(NC):
                nc.tensor.matmul(sp_[:], ones[:], src[:, c, :],
                                 start=(c == 0), stop=(c == NC - 1))
            s2p = ps_row.tile([1, QL], F, tag="lnsum2")
            for c in range(NC):
                sq = tmp.tile([P, QL], R, tag="lnsq")
                nc.scalar.activation(out=sq[:], in_=src[:, c, :], func=AF.Square)
                nc.tensor.matmul(s2p[:], ones[:], sq[:],
                                 start=(c == 0), stop=(c == NC - 1))
            mean = tmp.tile([1, QL], R, tag="lnmean")
            nc.scalar.mul(out=mean[:], in_=sp_[:], mul=1.0 / E)
            msq = tmp.tile([1, QL], R, tag="lnmsq")
            nc.scalar.mul(out=msq[:], in_=s2p[:], mul=1.0 / E)
            var = tmp.tile([1, QL], R, tag="lnvar")
            nc.vector.tensor_tensor(out=var[:], in0=mean[:], in1=mean[:], op=OP.mult)
            nc.vector.tensor_tensor(out=var[:], in0=msq[:], in1=var[:], op=OP.subtract)
            std = tmp.tile([1, QL], R, tag="lnstd")
            nc.scalar.activation(out=std[:], in_=var[:], func=AF.Sqrt, bias=eps1[:])
            rstd = tmp.tile([1, QL], R, tag="lnrstd")
            nc.vector.reciprocal(out=rstd[:], in_=std[:])
            mb = ps_big.tile([P, QL], F, tag="bigc")
            nc.tensor.matmul(mb[:], ones1[:], mean[:], start=True, stop=True)
            rb = ps_big.tile([P, QL], F, tag="bigc")
            nc.tensor.matmul(rb[:], ones1[:], rstd[:], start=True, stop=True)
            mbs = tmp.tile([P, QL], R, tag="lnmbs")
            nc.scalar.copy(out=mbs[:], in_=mb[:])
            rbs = tmp.tile([P, QL], R, tag="lnrbs")
            nc.scalar.copy(out=rbs[:], in_=rb[:])
            for c in range(NC):
                t = tmp.tile([P, QL], R, tag="lnt")
                nc.vector.tensor_tensor(out=t[:], in0=src[:, c, :], in1=mbs[:], op=OP.subtract)
                nc.vector.tensor_tensor(out=t[:], in0=t[:], in1=rbs[:], op=OP.mult)
                nc.vector.tensor_scalar(out=dst[:, c, :], in0=t[:],
                                        scalar1=wtile[:, c:c + 1], scalar2=btile[:, c:c + 1],
                                        op0=OP.mult, op1=OP.add)

        # ===== Phase C: out-proj + LN1 + gates + top-2 weights =====
        cres = es.enter_context(tc.tile_pool(name="cres", bufs=1))
        x1 = cres.tile([P, NC, QL], R, tag="x1")
        x1b = cres.tile([P, NC, QL], BF, tag="x1b")
        wbc = cres.tile([P, NE, QL], BF, tag="wbc")
        wT_sb = cres.tile([NE, QL], R, tag="wT")
        ebT_sb = cres.tile([NE, E], R, tag="ebT"); nc.sync.dma_start(ebT_sb[:], ebT[:])
        ln_sb = {}
        for nm, t in (("ln1w", ln1w), ("ln1b", ln1b), ("ln2w", ln2w), ("ln2b", ln2b)):
            ln_sb[nm] = cres.tile([P, NC], F, tag=nm, name=nm)
            nc.sync.dma_start(ln_sb[nm][:], t[:])
        with tc.tile_pool(name="cslab", bufs=1) as cslab, \
             tc.tile_pool(name="ct", bufs=2) as ct, \
             tc.tile_pool(name="h1p", bufs=1) as h1p, \
             tc.tile_pool(name="cps", bufs=3, space="PSUM") as cps, \
             tc.tile_pool(name="crow", bufs=1, space="PSUM") as crow, \
             tc.tile_pool(name="cg", bufs=1, space="PSUM") as cg:
            wo_sb = cslab.tile([P, NC, E], R, tag="wo_sb")
            for c in range(NC):
                nc.sync.dma_start(wo_sb[:, c, :],
                                  wo.rearrange("(c p) m -> p c m", p=P)[:, c, :])
            ctx_sb = cslab.tile([P, NC, QL], R, tag="ctx")
            nc.sync.dma_start(ctx_sb[:], ctxd_r[:])
            bo_sb = cslab.tile([P, NC], F, tag="bo"); nc.sync.dma_start(bo_sb[:], bo[:])
            gb_sb = cslab.tile([NE, 1], F, tag="gb"); nc.sync.dma_start(gb_sb[:], gb[:])
            gw_sb = cslab.tile([P, NC, NE], R, tag="gw")
            nc.sync.dma_start(gw_sb[:], gw.rearrange("(c p) g -> p c g", p=P))
            sel8_sb = cslab.tile([NE, NE * P], R, tag="sel8")
            nc.sync.dma_start(sel8_sb[:], sel8[:])
            iden_f = cslab.tile([P, P], F, tag="iden_f")
            make_identity(nc, iden_f[:])
            iden = cslab.tile([P, P], R, tag="iden")
            nc.vector.tensor_copy(out=iden[:], in_=iden_f[:])
            xres_sb = h1p.tile([P, NC, QL], R, tag="xres")
            nc.sync.dma_start(xres_sb[:], xres.rearrange("(c p) q -> p c q", p=P))
            h1 = h1p.tile([P, NC, QL], R, tag="h1")
            for oc in range(NC):
                ap = cps.tile([P, QL], F, tag="bigc")
                for dc in range(NC):
                    nc.tensor.matmul(ap[:], wo_sb[:, dc, oc * P:(oc + 1) * P],
                                     ctx_sb[:, dc, :],
                                     start=(dc == 0), stop=(dc == NC - 1))
                nc.vector.tensor_scalar(out=h1[:, oc, :], in0=ap[:],
                                        scalar1=bo_sb[:, oc:oc + 1], scalar2=None, op0=OP.add)
                nc.vector.tensor_tensor(out=h1[:, oc, :], in0=h1[:, oc, :],
                                        in1=xres_sb[:, oc, :], op=OP.add)

            layernorm(h1, x1, ln_sb["ln1w"], ln_sb["ln1b"], ct, crow, cps)

            gp = cg.tile([NE, QL], F, tag="gps")
            for c in range(NC):
                nc.tensor.matmul(gp[:], gw_sb[:, c, :], x1[:, c, :],
                                 start=(c == 0), stop=(c == NC - 1))
            glog = ct.tile([NE, QL], R, tag="glog")
            nc.vector.tensor_scalar(out=glog[:], in0=gp[:], scalar1=gb_sb[:],
                                    scalar2=None, op0=OP.add)
            gexp = ct.tile([NE, QL], R, tag="gexp")
            nc.scalar.activation(out=gexp[:], in_=glog[:], func=AF.Exp)

            # token-orientation gate math: gt[tok, qb, e]
            gt = ct.tile([P, 4, NE], R, tag="gt")
            for qb in range(4):
                gtp = cg.tile([P, NE], R, tag="gmix")
                nc.tensor.transpose(gtp[:], gexp[:, qb * P:(qb + 1) * P], iden[0:NE, 0:NE])
                nc.vector.tensor_copy(out=gt[:, qb, :], in_=gtp[:])
            dsum = ct.tile([P, 4], F, tag="dsum")
            nc.vector.reduce_sum(out=dsum[:], in_=gt[:], axis=AX)
            rgn = ct.tile([P, 4], F, tag="rgn")
            nc.vector.reciprocal(out=rgn[:], in_=dsum[:])
            m1 = ct.tile([P, 4], F, tag="m1")
            nc.vector.reduce_max(out=m1[:], in_=gt[:], axis=AX)
            msel = ct.tile([P, 4, NE], R, tag="msel")
            g2 = ct.tile([P, 4, NE], R, tag="g2")
            for qb in range(4):
                nc.vector.tensor_scalar(out=msel[:, qb, :], in0=gt[:, qb, :],
                                        scalar1=m1[:, qb:qb + 1], scalar2=None, op0=OP.is_equal)
            nc.vector.tensor_tensor(out=g2[:], in0=gt[:], in1=msel[:], op=OP.mult)
            nc.vector.tensor_tensor(out=g2[:], in0=gt[:], in1=g2[:], op=OP.subtract)
            m2 = ct.tile([P, 4], F, tag="m2")
            nc.vector.reduce_max(out=m2[:], in_=g2[:], axis=AX)
            msel2 = ct.tile([P, 4, NE], R, tag="msel2")
            for qb in range(4):
                nc.vector.tensor_scalar(out=msel2[:, qb, :], in0=g2[:, qb, :],
                                        scalar1=m2[:, qb:qb + 1], scalar2=None, op0=OP.is_equal)
            nc.vector.tensor_tensor(out=msel[:], in0=msel[:], in1=msel2[:], op=OP.add)
            wsel = ct.tile([P, 4, NE], R, tag="wsel")
            nc.vector.tensor_tensor(out=wsel[:], in0=gt[:], in1=msel[:], op=OP.mult)
            for qb in range(4):
                nc.vector.tensor_scalar(out=wsel[:, qb, :], in0=wsel[:, qb, :],
                                        scalar1=rgn[:, qb:qb + 1], scalar2=None, op0=OP.mult)

            # transpose per q-block -> wT_sb [NE, QL] (bf16)
            for qb in range(4):
                wtp = cg.tile([NE, P], R, tag="gmix")
                nc.tensor.transpose(wtp[:], wsel[:, qb, :], iden[:])
                nc.vector.tensor_copy(out=wT_sb[:, qb * P:(qb + 1) * P], in_=wtp[:])
            if KDBG:
                nc.sync.dma_start(dbg_x1.rearrange("(c p) q -> p c q", p=P)[:], x1[:])
                nc.sync.dma_start(dbg_wt[:], wT_sb[:])
            # broadcast per expert -> wbc [P, e, QL] (bf16)
            for e in range(NE):
                bcp = cps.tile([P, QL], F, tag="bigc")
                nc.tensor.matmul(bcp[:], sel8_sb[:, e * P:(e + 1) * P], wT_sb[:],
                                 start=True, stop=True)
                nc.vector.tensor_copy(out=wbc[:, e, :], in_=bcp[:])
            for c in range(NC):
                nc.vector.tensor_copy(out=x1b[:, c, :], in_=x1[:, c, :])

        # ===== Phase D: dense MoE with gate-scaled inputs, DVE accumulate =====
        moe = cres.tile([P, NC, QL], R, tag="moe")
        with tc.tile_pool(name="ewp", bufs=2) as ewp, \
             tc.tile_pool(name="xwp", bufs=2) as xwp, \
             tc.tile_pool(name="dps", bufs=4, space="PSUM") as dps:
            for e in range(NE):
                ew_sl = ewp.tile([P, NC, E], BF, tag="ew_sl")
                nc.sync.dma_start(ew_sl[:], ew[e].rearrange("c p d -> p c d"))
                xw = xwp.tile([P, NC, QL], BF, tag="xw")
                for dc in range(NC):
                    nc.vector.tensor_tensor(out=xw[:, dc, :], in0=x1b[:, dc, :],
                                            in1=wbc[:, e, :], op=OP.mult)
                for oc in range(NC):
                    yp = dps.tile([P, QL], F, tag="yps")
                    if e == 0:
                        nc.tensor.matmul(yp[:], ebT_sb[:, oc * P:(oc + 1) * P],
                                         wT_sb[:], start=True, stop=False)
                    for dc in range(NC):
                        nc.tensor.matmul(
                            yp[:], ew_sl[:, dc, oc * P:(oc + 1) * P], xw[:, dc, :],
                            start=(e != 0 and dc == 0), stop=(dc == NC - 1))
                    if e == 0:
                        # fold in the x1 residual for LN2 up front
                        nc.vector.tensor_tensor(out=moe[:, oc, :], in0=x1[:, oc, :],
                                                in1=yp[:], op=OP.add)
                    else:
                        nc.vector.tensor_tensor(out=moe[:, oc, :], in0=moe[:, oc, :],
                                                in1=yp[:], op=OP.add)

        if KDBG:
            nc.sync.dma_start(dbg_moe.rearrange("(c p) q -> p c q", p=P)[:], moe[:])
            nc.sync.dma_start(dbg_wbc.rearrange("p (e q) -> p e q", e=NE)[:], wbc[:])

        # ===== Phase E: LN2 + store =====
        with tc.tile_pool(name="et", bufs=3) as et, \
             tc.tile_pool(name="eps_", bufs=2, space="PSUM") as eps_, \
             tc.tile_pool(name="erow", bufs=1, space="PSUM") as erow:
            layernorm(moe, moe, ln_sb["ln2w"], ln_sb["ln2b"], et, erow, eps_)
            for c in range(NC):
                nc.sync.dma_start(out.rearrange("(c p) q -> p c q", p=P)[:, c, :], moe[:, c, :])

    nc.compile()
    return nc


def _prep_inputs(inputs):
    x = np.asarray(inputs['x'], dtype=np.float32)
    ipw = np.asarray(inputs['in_proj_w'], dtype=np.float32)
    ipb = np.asarray(inputs['in_proj_b'], dtype=np.float32)
    opw = np.asarray(inputs['out_proj_w'], dtype=np.float32)
    opb = np.asarray(inputs['out_proj_b'], dtype=np.float32)
    gww = np.asarray(inputs['gate_w'], dtype=np.float32)
    gbb = np.asarray(inputs['gate_b'], dtype=np.float32)
    eww = np.asarray(inputs['expert_w'], dtype=np.float32)
    ebb = np.asarray(inputs['expert_b'], dtype=np.float32)

    perm = np.empty(E, dtype=np.int64)
    idx = 0
    for h in range(H):
        for i in range(D // 2):
            perm[idx] = 64 * h + 2 * i; idx += 1
    for h in range(H):
        for i in range(D // 2):
            perm[idx] = 64 * h + 2 * i + 1; idx += 1

    Wq, Wk, Wv = ipw[0:E], ipw[E:2 * E], ipw[2 * E:3 * E]
    bq_, bk_, bv_ = ipb[0:E], ipb[E:2 * E], ipb[2 * E:3 * E]
    sel8 = np.zeros((NE, NE * P), dtype=np.float32)
    for e in range(NE):
        sel8[e, e * P:(e + 1) * P] = 1.0
    common = {
        "wq": np.ascontiguousarray(Wq[:, perm].T),
        "wk": np.ascontiguousarray(Wk[:, perm].T),
        "wv": np.ascontiguousarray(Wv[:, perm].T),
        "bq": np.ascontiguousarray(bq_.reshape(NC, P).T),
        "bk": np.ascontiguousarray(bk_.reshape(NC, P).T),
        "bvr": bv_.reshape(1, E).copy(),
        "wo": np.ascontiguousarray(opw.T),
        "bo": np.ascontiguousarray(opb.reshape(NC, P).T),
        "gw": np.ascontiguousarray(gww.T),
        "gb": gbb.reshape(NE, 1).copy(),
        "ew": np.ascontiguousarray(
            eww.transpose(0, 2, 1).reshape(NE, NC, P, E)).astype(ml_dtypes.bfloat16),
        "ebT": ebb,
        "sel8": sel8,
        "ln1w": np.ascontiguousarray(np.asarray(inputs['ln1_w'], np.float32).reshape(NC, P).T),
        "ln1b": np.ascontiguousarray(np.asarray(inputs['ln1_b'], np.float32).reshape(NC, P).T),
        "ln2w": np.ascontiguousarray(np.asarray(inputs['ln2_w'], np.float32).reshape(NC, P).T),
        "ln2b": np.ascontiguousarray(np.asarray(inputs['ln2_b'], np.float32).reshape(NC, P).T),
        "trid": np.ascontiguousarray(
            (np.arange(P)[None, :] >= np.arange(P)[:, None]).astype(np.float32)),
    }
    inv_freq = 1.0 / (10000.0 ** (np.arange(0, D, 2, dtype=np.float64) / D))
    freqs = np.arange(S, dtype=np.float64)[:, None] * inv_freq[None, :]
    cos_t = np.cos(freqs).T.astype(np.float32)
    sin_t = np.sin(freqs).T.astype(np.float32)
    common["cos2"] = np.ascontiguousarray(np.tile(cos_t, (4, 1)))
    common["sin2"] = np.ascontiguousarray(np.tile(sin_t, (4, 1)))

    in_maps = []
    for c in range(8):
        b, qc = c // 4, c % 4
        blocks = [qc + 4 * i for i in range(4)]
        cols = np.concatenate([np.arange(blk * P, (blk + 1) * P) for blk in blocks])
        xtb = np.ascontiguousarray(x[b].T)
        xtp = np.ascontiguousarray(xtb[perm])
        m = dict(common)
        m["xt"] = xtp
        m["xres"] = np.ascontiguousarray(xtb[:, cols])
        in_maps.append(m)
    return in_maps


def _run_multi(ncs, in_maps):
    """Run the 4 NEFFs concurrently: graph qc on devices {qc, qc+4} (b=0,1)."""
    import jax
    from jax.sharding import Mesh, PartitionSpec
    from jax.experimental.shard_map import shard_map
    from concourse import bass2jax
    from concourse import mybir as _mb

    bass2jax.install_neuronx_cc_hook()
    devices = jax.devices()

    if "jits" not in _cache:
        _cache["jits"] = {}
    handles = []
    for qc in range(4):
        nc = ncs[qc]
        if qc not in _cache["jits"]:
            in_names, out_names, out_avals, zero_outs = [], [], [], []
            for alloc in nc.m.functions[0].allocations:
                if not isinstance(alloc, _mb.MemoryLocationSet):
                    continue
                name = alloc.memorylocations[0].name
                if alloc.kind == "ExternalInput":
                    in_names.append(name)
                elif alloc.kind == "ExternalOutput":
                    out_names.append(name)
                    shape = tuple(alloc.tensor_shape)
                    dtype = _mb.dt.np(alloc.dtype)
                    out_avals.append(jax.core.ShapedArray(shape, dtype))
                    zero_outs.append(np.zeros(shape, dtype))
            n_params = len(in_names)
            all_names = in_names + out_names
            donate = tuple(range(n_params, n_params + len(out_names)))

            def _body(*args, _nc=nc, _avals=tuple(out_avals), _all=tuple(all_names),
                      _outs=tuple(out_names)):
                outs = bass2jax._bass_exec_p.bind(
                    *args, out_avals=_avals, in_names=_all, out_names=_outs,
                    lowering_input_output_aliases=(),
                    sim_require_finite=True, sim_require_nnan=True, nc=_nc)
                return tuple(outs)

            devs = [devices[qc], devices[qc + 4]]
            mesh = Mesh(np.asarray(devs), ("core",))
            nio = n_params + len(zero_outs)
            sharded = jax.jit(
                shard_map(_body, mesh=mesh,
                          in_specs=(PartitionSpec("core"),) * nio,
                          out_specs=(PartitionSpec("core"),) * len(out_names),
                          check_rep=False),
                donate_argnums=donate, keep_unused=True)
            _cache["jits"][qc] = (sharded, in_names, out_names, zero_outs)
        sharded, in_names, out_names, zero_outs = _cache["jits"][qc]
        per_core = [[np.asarray(in_maps[b * 4 + qc][n]) for n in in_names] for b in range(2)]
        concat_in = [np.concatenate([per_core[b][i] for b in range(2)], axis=0)
                     for i in range(len(in_names))]
        concat_zero = [np.concatenate([z, z], axis=0) for z in zero_outs]
        handles.append((sharded, concat_in, concat_zero, out_names))

    outs = []
    for sharded, concat_in, concat_zero, out_names in handles:
        outs.append((sharded(*concat_in, *concat_zero), out_names))
    results = [None] * 8
    for qc, (arrs, out_names) in enumerate(outs):
        arrs = [np.asarray(a) for a in arrs]
        for b in range(2):
            rm = {}
            for i, n in enumerate(out_names):
                full = arrs[i]
                half = full.shape[0] // 2
                rm[n] = full[b * half:(b + 1) * half]
            results[b * 4 + qc] = rm
    return results


def _ensure_ntff_hook():
    import types
    try:
        from antenv.axon_hooks import get_axon_ntff_profile_hook  # noqa
        return True
    except ImportError:
        pass
    try:
        import antenv
        sys.path.insert(0, '/root/.axon_site')
        from trn_agent_boot.trn_boot import _ntff_profile_via_ctypes
        hook = _ntff_profile_via_ctypes('/opt/axon/libaxon_pjrt.so')
        if hook is None:
            return False
        mod = types.ModuleType('antenv.axon_hooks')
        _state = {'hook': hook}
        mod.set_axon_ntff_profile_hook = lambda h: _state.__setitem__('hook', h)
        mod.get_axon_ntff_profile_hook = lambda: _state['hook']
        sys.modules['antenv.axon_hooks'] = mod
        antenv.axon_hooks = mod
        return True
    except Exception as e:
        print(f"ntff hook setup failed: {e}")
        return False


def kernel(**inputs):
    if "ncs" not in _cache:
        _cache["ncs"] = [_build(qc) for qc in range(4)]
    ncs = _cache["ncs"]
    in_maps = _prep_inputs(inputs)

    trace = bool(int(os.environ.get("KERNEL_TRACE", "0")))
    if trace and _ensure_ntff_hook():
        import tempfile
        from antenv.axon_hooks import get_axon_ntff_profile_hook
        hook = get_axon_ntff_profile_hook()
        tmpdir = tempfile.mkdtemp()
        _run_multi(ncs, in_maps)  # warm-up/compile outside the profile window
        with hook(tmpdir, list(range(8))):
            results = _run_multi(ncs, in_maps)
        _cache["ntff_dir"] = tmpdir
        print(f"ntff dir: {tmpdir}")
    else:
        results = _run_multi(ncs, in_maps)
    _cache["last_results"] = results

    out = np.empty((B, S, E), dtype=np.float32)
    for c in range(8):
        b, qc = c // 4, c % 4
        o = results[c]["out"]  # [E, QL]
        for i in range(4):
            blk = qc + 4 * i
            out[b, blk * P:(blk + 1) * P, :] = o[:, i * P:(i + 1) * P].T
    return out


# revision 44
# speedup vs baseline: 1.3729x; 1.1153x over previous
"""Trainium2 Bass kernel for nn_EnhancedTransformerLayer (RoPE attention + MoE).

Sharding: 8 cores; core c -> batch b=c//4, qc=c%4. Four distinct NEFFs (one per
qc), each run on 2 cores (b=0,1). Core qc owns interleaved query blocks
{qc, qc+4, qc+8, qc+12} (4 x 128 tokens) so causal work is balanced, and only
computes K/V up to its last block.

v2: fused rope/V/K span pipeline (single x read, in-place rope, no DRAM
round-trip), softmax denominator via ones-column in the ctx stationary,
fp32r matmuls clamped to N>=256, MoE accumulated directly in PSUM across
experts with gate-scaled inputs (no gpsimd, no per-expert combine).
"""
import sys, os
sys.path.insert(0, '/opt/trn_rl_repo')
import numpy as np
import ml_dtypes

import concourse.bass as bass
from concourse import bacc
import concourse.tile as tile
from concourse import mybir
from concourse.masks import make_identity

R = mybir.dt.float32r
F = mybir.dt.float32
BF = mybir.dt.bfloat16
P = 128
B, S, E, H, D, NE = 2, 2048, 1024, 16, 64, 8
NC = E // P
QL = 512
EXP_SCALE = 1.0 / (D ** 0.5)
LN_EPS = 1e-5

_cache = {}


def _kv_plan(qc):
    """K/V token coverage and span split for this qc."""
    kv_tok = 128 * (qc + 13)            # last interleaved block end
    rem = kv_tok - 1536
    rem = max(rem, 256)                 # fp32r needs N>=256
    kv_tok = 1536 + rem
    spans = [(0, 512), (512, 512), (1024, 512), (1536, rem)]
    return kv_tok, spans


def _build(qc):
    nc = bacc.Bacc("TRN2", target_bir_lowering=False, debug=False, num_devices=8,
                   name=f"moe_qc{qc}", enable_partition_id=False)
    kv_tok, kv_spans = _kv_plan(qc)
    KCN = kv_tok // P                   # key chunk count

    def din(name, shape, dt=R):
        return nc.dram_tensor(name, shape, dt, kind="ExternalInput")

    xt = din("xt", [E, S])              # permuted x^T (f32r bits == f32)
    xres = din("xres", [E, QL])
    wq = din("wq", [E, E]); wk = din("wk", [E, E]); wv = din("wv", [E, E])
    bq = din("bq", [P, NC], F); bk = din("bk", [P, NC], F)
    bvr = din("bvr", [1, E])
    wo = din("wo", [E, E]); bo = din("bo", [P, NC], F)
    gw = din("gw", [E, NE]); gb = din("gb", [NE, 1], F)
    cos2 = din("cos2", [P, S], F); sin2 = din("sin2", [P, S], F)
    trid = din("trid", [P, P])          # tri[k, q] = 1 if q >= k (within a block)
    ew = din("ew", [NE, NC, P, E], BF)
    ebT = din("ebT", [NE, E])           # expert bias, experts on partitions
    sel8 = din("sel8", [NE, NE * P])    # sel8[:, e*P:(e+1)*P] row e = 1
    ln1w = din("ln1w", [P, NC], F); ln1b = din("ln1b", [P, NC], F)
    ln2w = din("ln2w", [P, NC], F); ln2b = din("ln2b", [P, NC], F)
    out = nc.dram_tensor("out", [E, QL], R, kind="ExternalOutput")
    KDBG = bool(int(os.environ.get("KDBG", "0")))
    dbg_kind = "ExternalOutput" if KDBG else "Internal"
    vsch = [nc.dram_tensor(f"vsc{i}", [kv_tok, 512], BF,
                           kind=dbg_kind) for i in range(2)]
    ctxd = nc.dram_tensor("ctxd", [E, QL], R, kind=dbg_kind)
    if KDBG:
        dbg_qx = nc.dram_tensor("dbg_qx", [E, QL], R, kind="ExternalOutput")
        dbg_qt = nc.dram_tensor("dbg_qt", [E, QL], BF, kind="ExternalOutput")
        dbg_kt = nc.dram_tensor("dbg_kt", [P, kv_tok], BF, kind="ExternalOutput")
        dbg_x1 = nc.dram_tensor("dbg_x1", [E, QL], R, kind="ExternalOutput")
        dbg_wt = nc.dram_tensor("dbg_wt", [NE, QL], R, kind="ExternalOutput")
        dbg_wbc = nc.dram_tensor("dbg_wbc", [P, NE * QL], BF, kind="ExternalOutput")
        dbg_moe = nc.dram_tensor("dbg_moe", [E, QL], R, kind="ExternalOutput")

    AX = mybir.AxisListType.X
    OP = mybir.AluOpType
    AF = mybir.ActivationFunctionType
    import contextlib

    with tile.TileContext(nc) as tc, \
         nc.allow_low_precision(reason="float32r is bit-identical to float32"), \
         contextlib.ExitStack() as es:
        consts = es.enter_context(tc.tile_pool(name="consts", bufs=1))

        ones_f = consts.tile([P, 1], F, tag="ones_f")
        nc.vector.memset(ones_f[:], 1.0)
        ones = consts.tile([P, 1], R, tag="ones")
        nc.vector.tensor_copy(out=ones[:], in_=ones_f[:])
        ones1_f = consts.tile([1, P], F, tag="ones1_f")
        nc.vector.memset(ones1_f[:], 1.0)
        ones1 = consts.tile([1, P], R, tag="ones1")
        nc.vector.tensor_copy(out=ones1[:], in_=ones1_f[:])
        eps1 = consts.tile([1, 1], F, tag="eps1")
        nc.vector.memset(eps1[:], LN_EPS)
        bq_sb = consts.tile([P, NC], F, tag="bq"); nc.sync.dma_start(bq_sb[:], bq[:])
        bk_sb = consts.tile([P, NC], F, tag="bk"); nc.sync.dma_start(bk_sb[:], bk[:])

        # ===== Phase A: fused V-proj + rope + K-proj per 512-token span =====
        xt_r = xt.rearrange("(c p) s -> p c s", p=P)
        attn_cm = tc.tile_pool(name="attn_res", bufs=1)
        attn_res = attn_cm.__enter__()
        kTs = [attn_res.tile([P, kv_tok], BF, tag=f"kT{oc}", name=f"kT{oc}")
               for oc in range(NC)]
        qT = attn_res.tile([P, NC, QL], BF, tag="qT")
        qx_cm = tc.tile_pool(name="qx_p", bufs=1)
        qx_p = qx_cm.__enter__()
        qx = qx_p.tile([P, NC, QL], R, tag="qx")

        with tc.tile_pool(name="xsp_p", bufs=2) as xsp_p, \
             tc.tile_pool(name="cs_p", bufs=2) as cs_p, \
             tc.tile_pool(name="a1tmp", bufs=1) as a1tmp, \
             tc.tile_pool(name="wvkp", bufs=1) as wvkp, \
             tc.tile_pool(name="vev_p", bufs=2) as vev_p, \
             tc.tile_pool(name="vps", bufs=4, space="PSUM") as vps, \
             tc.tile_pool(name="kps", bufs=3, space="PSUM") as kps:
            xsp_t, cs_t = {}, {}

            def emit_dma(sp):
                s0, sl = kv_spans[sp]
                xsp = xsp_p.tile([P, NC, 512], R, tag="xsp")
                nc.sync.dma_start(xsp[:, :, :sl], xt_r[:, :, s0:s0 + sl])
                csb = cs_p.tile([P, 2, 512], F, tag="csb")
                nc.sync.dma_start(csb[:, 0, :sl], cos2[:, s0:s0 + sl])
                nc.sync.dma_start(csb[:, 1, :sl], sin2[:, s0:s0 + sl])
                xsp_t[sp] = xsp
                cs_t[sp] = csb

            def emit_V(sp):
                s0, sl = kv_spans[sp]
                xsp = xsp_t[sp]
                for t in range(sl // P):
                    for dvs in range(2):
                        vp = vps.tile([P, 512], F, tag="vps")
                        for dc in range(NC):
                            nc.tensor.matmul(
                                vp[:], xsp[:, dc, t * P:(t + 1) * P],
                                wv_sb[:, dc, dvs * 512:(dvs + 1) * 512],
                                start=(dc == 0), stop=False)
                        nc.tensor.matmul(
                            vp[:], ones1[:, :], bv_sb[:, dvs * 512:(dvs + 1) * 512],
                            start=False, stop=True)
                        vev = vev_p.tile([P, 512], BF, tag="vev")
                        nc.scalar.copy(out=vev[:], in_=vp[:])
                        nc.sync.dma_start(
                            vsch[dvs][s0 + t * P:s0 + (t + 1) * P, :], vev[:])

            def emit_rope(sp):
                s0, sl = kv_spans[sp]
                xsp, csb = xsp_t[sp], cs_t[sp]
                cs = csb[:, 0, :sl]; sn = csb[:, 1, :sl]
                for c in range(4):
                    # in-place rotation: a' = a*cos - b*sin; b' = a*sin + b*cos
                    t1 = a1tmp.tile([P, 512], F, tag="ropet1")
                    t2 = a1tmp.tile([P, 512], F, tag="ropet2")
                    a = xsp[:, c, :sl]; b = xsp[:, c + 4, :sl]
                    nc.vector.tensor_tensor(out=t1[:, :sl], in0=a, in1=sn, op=OP.mult)
                    nc.vector.tensor_tensor(out=t2[:, :sl], in0=b, in1=sn, op=OP.mult)
                    nc.vector.tensor_tensor(out=a, in0=a, in1=cs, op=OP.mult)
                    nc.vector.tensor_tensor(out=a, in0=a, in1=t2[:, :sl], op=OP.subtract)
                    nc.vector.tensor_tensor(out=b, in0=b, in1=cs, op=OP.mult)
                    nc.vector.tensor_tensor(out=b, in0=b, in1=t1[:, :sl], op=OP.add)
                # q block sp = global block qc+4*sp lives at cols qc*P within span sp
                for c in range(NC):
                    nc.vector.tensor_copy(
                        out=qx[:, c, sp * P:(sp + 1) * P],
                        in_=xsp[:, c, qc * P:(qc + 1) * P])

            def emit_K(sp):
                s0, sl = kv_spans[sp]
                xsp = xsp_t[sp]
                for oc in range(NC):
                    kp = kps.tile([P, 512], F, tag="kps")
                    for dc in range(NC):
                        nc.tensor.matmul(
                            kp[:, :sl], wk_sb[:, dc, oc * P:(oc + 1) * P],
                            xsp[:, dc, :sl],
                            start=(dc == 0), stop=(dc == NC - 1))
                    nc.vector.tensor_scalar(
                        out=kTs[oc][:, s0:s0 + sl], in0=kp[:, :sl],
                        scalar1=bk_sb[:, oc:oc + 1], scalar2=None, op0=OP.add)

            # x/cos/sin for span 0 first, then wv, so V matmuls start ASAP;
            # wk loads while V(0) computes.
            emit_dma(0)
            wv_sb = wvkp.tile([P, NC, E], R, tag="wv")
            for c in range(NC):
                nc.sync.dma_start(wv_sb[:, c, :],
                                  wv.rearrange("(c p) m -> p c m", p=P)[:, c, :])
            bv_sb = wvkp.tile([1, E], R, tag="bv")
            nc.sync.dma_start(bv_sb[:], bvr[:])
            emit_V(0)
            wk_sb = wvkp.tile([P, NC, E], R, tag="wk")
            for c in range(NC):
                nc.sync.dma_start(wk_sb[:, c, :],
                                  wk.rearrange("(c p) m -> p c m", p=P)[:, c, :])
            emit_rope(0)
            emit_dma(1); emit_V(1); emit_K(0); emit_rope(1)
            emit_dma(2); emit_V(2); emit_K(1); emit_rope(2)
            emit_dma(3); emit_V(3); emit_K(2); emit_rope(3)
            emit_K(3)

        # ===== Q projection from roped q blocks in SBUF =====
        with tc.tile_pool(name="wqp", bufs=1) as wqp, \
             tc.tile_pool(name="qps_p", bufs=4, space="PSUM") as qps_p:
            wq_sb = wqp.tile([P, NC, E], R, tag="wq_sb")
            for c in range(NC):
                nc.sync.dma_start(wq_sb[:, c, :],
                                  wq.rearrange("(c p) m -> p c m", p=P)[:, c, :])
            for oc in range(NC):
                qp = qps_p.tile([P, 512], F, tag="qps")
                for dc in range(NC):
                    nc.tensor.matmul(
                        qp[:], wq_sb[:, dc, oc * P:(oc + 1) * P], qx[:, dc, :],
                        start=(dc == 0), stop=(dc == NC - 1))
                nc.vector.tensor_scalar(
                    out=qT[:, oc, :], in0=qp[:],
                    scalar1=bq_sb[:, oc:oc + 1], scalar2=None, op0=OP.add)
            if KDBG:
                nc.sync.dma_start(dbg_qx.rearrange("(c p) q -> p c q", p=P)[:], qx[:])
                nc.sync.dma_start(dbg_qt.rearrange("(c p) q -> p c q", p=P)[:], qT[:])
                nc.sync.dma_start(dbg_kt[:], kTs[0][:])
        qx_cm.__exit__(None, None, None)

        # ===== Phase B: causal attention over interleaved blocks =====
        # ctx stationary vt has a ones column per head half: row 64 of the
        # [65, QL] ctx psum accumulates the softmax denominator for free.
        ctxd_r = ctxd.rearrange("(c p) q -> p c q", p=P)
        kcs = [kc for kc in range(KCN) if -(-(kc - qc) // 4) < 4]
        with tc.tile_pool(name="bt", bufs=6) as bt, \
             tc.tile_pool(name="bconst", bufs=1) as bconst, \
             tc.tile_pool(name="vt_p", bufs=2) as vt_p, \
             tc.tile_pool(name="scps", bufs=2, space="PSUM") as scps, \
             tc.tile_pool(name="ctxps", bufs=4, space="PSUM") as ctxps:
            tri_f = bconst.tile([P, P], R, tag="tri_f")
            nc.sync.dma_start(tri_f[:], trid[:])
            tri_sb = bconst.tile([P, P], BF, tag="tri")
            nc.vector.tensor_copy(out=tri_sb[:], in_=tri_f[:])
            ones65_f = bconst.tile([65, P], F, tag="ones65_f")
            nc.vector.memset(ones65_f[:], 1.0)
            ones65 = bconst.tile([65, P], BF, tag="ones65")
            nc.vector.tensor_copy(out=ones65[:], in_=ones65_f[:])
            oneskc = bconst.tile([P, KCN], F, tag="oneskc")
            nc.vector.memset(oneskc[:], 1.0)
            for hp in range(NC):
                vt = vt_p.tile([P, KCN, 130], BF, tag="vt")
                for hh in range(2):
                    nc.sync.dma_start(
                        vt[:, :, hh * 65:hh * 65 + 64],
                        vsch[hp // 4].rearrange("(k p) d -> p k d", p=P)
                        [:, :, (hp % 4) * P + hh * 64:(hp % 4) * P + (hh + 1) * 64])
                    nc.vector.tensor_copy(out=vt[:, :, hh * 65 + 64:hh * 65 + 65],
                                          in_=oneskc[:])
                ctxp2 = [ctxps.tile([65, QL], F, tag="ctxps", name=f"ctxp_{hp}_{hh}")
                         for hh in range(2)]

                # software pipeline: issue scores for block i+1 before ctx of
                # block i, so ctx matmuls never stall on the Scalar exp.
                # Both head halves share one 2-bank psum tile -> single exp op.
                def emit_scores(kc):
                    j0 = max(0, -(-(kc - qc) // 4))
                    c0 = q0 = j0 * P
                    scp = scps.tile([P, 2, QL], F, tag="scps")
                    for hh in range(2):
                        nc.tensor.matmul(
                            scp[:, hh, c0:], kTs[hp][hh * 64:(hh + 1) * 64, kc * P:(kc + 1) * P],
                            qT[hh * 64:(hh + 1) * 64, hp, c0:], start=True, stop=True)
                    st = bt.tile([P, 2, QL], BF, tag="st")
                    nc.scalar.activation(out=st[:, :, c0:], in_=scp[:, :, c0:],
                                         func=AF.Exp, scale=EXP_SCALE)
                    if kc >= qc and (kc - qc) % 4 == 0:
                        j = (kc - qc) // 4
                        for hh in range(2):
                            nc.vector.tensor_tensor(
                                out=st[:, hh, j * P:(j + 1) * P],
                                in0=st[:, hh, j * P:(j + 1) * P],
                                in1=tri_sb[:], op=OP.mult)
                    return (kc, c0, st)

                def emit_ctx(pend):
                    kc, c0, st = pend
                    for hh in range(2):
                        nc.tensor.matmul(ctxp2[hh][:, c0:],
                                         vt[:, kc, hh * 65:(hh + 1) * 65], st[:, hh, c0:],
                                         start=(kc == kcs[0]), stop=(kc == kcs[-1]))

                pend = None
                for kc in kcs:
                    cur = emit_scores(kc)
                    if pend is not None:
                        emit_ctx(pend)
                    pend = cur
                emit_ctx(pend)
                ctxo = bt.tile([P, QL], R, tag="ctxo")
                for hh in range(2):
                    rden = bt.tile([65, QL], BF, tag="rden")
                    nc.vector.reciprocal(out=rden[64:65, :], in_=ctxp2[hh][64:65, :])
                    rbp = ctxps.tile([65, QL], F, tag="ctxps", name=f"rbp_{hp}_{hh}")
                    nc.tensor.matmul(rbp[0:64, :], ones65[64:65, 0:64], rden[64:65, :],
                                     start=True, stop=True)
                    rb_sb = bt.tile([64, QL], R, tag="rb_sb")
                    nc.scalar.copy(out=rb_sb[:], in_=rbp[0:64, :])
                    nc.vector.tensor_tensor(
                        out=ctxo[hh * 64:(hh + 1) * 64, :],
                        in0=ctxp2[hh][0:64, :], in1=rb_sb[:], op=OP.mult)
                nc.sync.dma_start(ctxd_r[:, hp, :], ctxo[:])
        attn_cm.__exit__(None, None, None)

        # ===== LN helper =====
        def layernorm(src, dst, wtile, btile, tmp, ps_row, ps_big):
            sp_ = ps_row.tile([1, QL], F, tag="lnsum")
            for c in range(NC):
                nc.tensor.matmul(sp_[:], ones[:], src[:, c, :],
                                 start=(c == 0), stop=(c == NC - 1))
            s2p = ps_row.tile([1, QL], F, tag="lnsum2")
            for c in range(NC):
                sq = tmp.tile([P, QL], R, tag="lnsq")
                nc.scalar.activation(out=sq[:], in_=src[:, c, :], func=AF.Square)
                nc.tensor.matmul(s2p[:], ones[:], sq[:],
                                 start=(c == 0), stop=(c == NC - 1))
            mean = tmp.tile([1, QL], R, tag="lnmean")
            nc.scalar.mul(out=mean[:], in_=sp_[:], mul=1.0 / E)
            msq = tmp.tile([1, QL], R, tag="lnmsq")
            nc.scalar.mul(out=msq[:], in_=s2p[:], mul=1.0 / E)
            var = tmp.tile([1, QL], R, tag="lnvar")
            nc.vector.tensor_tensor(out=var[:], in0=mean[:], in1=mean[:], op=OP.mult)
            nc.vector.tensor_tensor(out=var[:], in0=msq[:], in1=var[:], op=OP.subtract)
            std = tmp.tile([1, QL], R, tag="lnstd")
            nc.scalar.activation(out=std[:], in_=var[:], func=AF.Sqrt, bias=eps1[:])
            rstd = tmp.tile([1, QL], R, tag="lnrstd")
            nc.vector.reciprocal(out=rstd[:], in_=std[:])
            mb = ps_big.tile([P, QL], F, tag="bigc")
            nc.tensor.matmul(mb[:], ones1[:], mean[:], start=True, stop=True)
            rb = ps_big.tile([P, QL], F, tag="bigc")
            nc.tensor.matmul(rb[:], ones1[:], rstd[:], start=True, stop=True)
            mbs = tmp.tile([P, QL], R, tag="lnmbs")
            nc.scalar.copy(out=mbs[:], in_=mb[:])
            rbs = tmp.tile([P, QL], R, tag="lnrbs")
            nc.scalar.copy(out=rbs[:], in_=rb[:])
            for c in range(NC):
                t = tmp.tile([P, QL], R, tag="lnt")
                nc.vector.tensor_tensor(out=t[:], in0=src[:, c, :], in1=mbs[:], op=OP.subtract)
                nc.vector.tensor_tensor(out=t[:], in0=t[:], in1=rbs[:], op=OP.mult)
                nc.vector.tensor_scalar(out=dst[:, c, :], in0=t[:],
                                        scalar1=wtile[:, c:c + 1], scalar2=btile[:, c:c + 1],
                                        op0=OP.mult, op1=OP.add)

        # ===== Phase C: out-proj + LN1 + gates + top-2 weights =====
        cres = es.enter_context(tc.tile_pool(name="cres", bufs=1))
        x1 = cres.tile([P, NC, QL], R, tag="x1")
        x1b = cres.tile([P, NC, QL], BF, tag="x1b")
        wbc = cres.tile([P, NE, QL], BF, tag="wbc")
        wT_sb = cres.tile([NE, QL], R, tag="wT")
        ebT_sb = cres.tile([NE, E], R, tag="ebT"); nc.sync.dma_start(ebT_sb[:], ebT[:])
        ln_sb = {}
        for nm, t in (("ln1w", ln1w), ("ln1b", ln1b), ("ln2w", ln2w), ("ln2b", ln2b)):
            ln_sb[nm] = cres.tile([P, NC], F, tag=nm, name=nm)
            nc.sync.dma_start(ln_sb[nm][:], t[:])
        with tc.tile_pool(name="cslab", bufs=1) as cslab, \
             tc.tile_pool(name="ct", bufs=2) as ct, \
             tc.tile_pool(name="h1p", bufs=1) as h1p, \
             tc.tile_pool(name="cps", bufs=3, space="PSUM") as cps, \
             tc.tile_pool(name="crow", bufs=1, space="PSUM") as crow, \
             tc.tile_pool(name="cg", bufs=1, space="PSUM") as cg:
            wo_sb = cslab.tile([P, NC, E], R, tag="wo_sb")
            for c in range(NC):
                nc.sync.dma_start(wo_sb[:, c, :],
                                  wo.rearrange("(c p) m -> p c m", p=P)[:, c, :])
            ctx_sb = cslab.tile([P, NC, QL], R, tag="ctx")
            nc.sync.dma_start(ctx_sb[:], ctxd_r[:])
            bo_sb = cslab.tile([P, NC], F, tag="bo"); nc.sync.dma_start(bo_sb[:], bo[:])
            gb_sb = cslab.tile([NE, 1], F, tag="gb"); nc.sync.dma_start(gb_sb[:], gb[:])
            gw_sb = cslab.tile([P, NC, NE], R, tag="gw")
            nc.sync.dma_start(gw_sb[:], gw.rearrange("(c p) g -> p c g", p=P))
            sel8_sb = cslab.tile([NE, NE * P], R, tag="sel8")
            nc.sync.dma_start(sel8_sb[:], sel8[:])
            iden_f = cslab.tile([P, P], F, tag="iden_f")
            make_identity(nc, iden_f[:])
            iden = cslab.tile([P, P], R, tag="iden")
            nc.vector.tensor_copy(out=iden[:], in_=iden_f[:])
            xres_sb = h1p.tile([P, NC, QL], R, tag="xres")
            nc.sync.dma_start(xres_sb[:], xres.rearrange("(c p) q -> p c q", p=P))
            h1 = h1p.tile([P, NC, QL], R, tag="h1")
            for oc in range(NC):
                ap = cps.tile([P, QL], F, tag="bigc")
                for dc in range(NC):
                    nc.tensor.matmul(ap[:], wo_sb[:, dc, oc * P:(oc + 1) * P],
                                     ctx_sb[:, dc, :],
                                     start=(dc == 0), stop=(dc == NC - 1))
                nc.vector.tensor_scalar(out=h1[:, oc, :], in0=ap[:],
                                        scalar1=bo_sb[:, oc:oc + 1], scalar2=None, op0=OP.add)
                nc.vector.tensor_tensor(out=h1[:, oc, :], in0=h1[:, oc, :],
                                        in1=xres_sb[:, oc, :], op=OP.add)

            layernorm(h1, x1, ln_sb["ln1w"], ln_sb["ln1b"], ct, crow, cps)

            gp = cg.tile([NE, QL], F, tag="gps")
            for c in range(NC):
                nc.tensor.matmul(gp[:], gw_sb[:, c, :], x1[:, c, :],
                                 start=(c == 0), stop=(c == NC - 1))
            glog = ct.tile([NE, QL], R, tag="glog")
            nc.vector.tensor_scalar(out=glog[:], in0=gp[:], scalar1=gb_sb[:],
                                    scalar2=None, op0=OP.add)
            gexp = ct.tile([NE, QL], R, tag="gexp")
            nc.scalar.activation(out=gexp[:], in_=glog[:], func=AF.Exp)

            # token-orientation gate math: gt[tok, qb, e]
            gt = ct.tile([P, 4, NE], R, tag="gt")
            for qb in range(4):
                gtp = cg.tile([P, NE], R, tag="gmix")
                nc.tensor.transpose(gtp[:], gexp[:, qb * P:(qb + 1) * P], iden[0:NE, 0:NE])
                nc.vector.tensor_copy(out=gt[:, qb, :], in_=gtp[:])
            dsum = ct.tile([P, 4], F, tag="dsum")
            nc.vector.reduce_sum(out=dsum[:], in_=gt[:], axis=AX)
            rgn = ct.tile([P, 4], F, tag="rgn")
            nc.vector.reciprocal(out=rgn[:], in_=dsum[:])
            m1 = ct.tile([P, 4], F, tag="m1")
            nc.vector.reduce_max(out=m1[:], in_=gt[:], axis=AX)
            msel = ct.tile([P, 4, NE], R, tag="msel")
            g2 = ct.tile([P, 4, NE], R, tag="g2")
            for qb in range(4):
                nc.vector.tensor_scalar(out=msel[:, qb, :], in0=gt[:, qb, :],
                                        scalar1=m1[:, qb:qb + 1], scalar2=None, op0=OP.is_equal)
            nc.vector.tensor_tensor(out=g2[:], in0=gt[:], in1=msel[:], op=OP.mult)
            nc.vector.tensor_tensor(out=g2[:], in0=gt[:], in1=g2[:], op=OP.subtract)
            m2 = ct.tile([P, 4], F, tag="m2")
            nc.vector.reduce_max(out=m2[:], in_=g2[:], axis=AX)
            msel2 = ct.tile([P, 4, NE], R, tag="msel2")
            for qb in range(4):
                nc.vector.tensor_scalar(out=msel2[:, qb, :], in0=g2[:, qb, :],
                                        scalar1=m2[:, qb:qb + 1], scalar2=None, op0=OP.is_equal)
            nc.vector.tensor_tensor(out=msel[:], in0=msel[:], in1=msel2[:], op=OP.add)
            wsel = ct.tile([P, 4, NE], R, tag="wsel")
            nc.vector.tensor_tensor(out=wsel[:], in0=gt[:], in1=msel[:], op=OP.mult)
            for qb in range(4):
                nc.vector.tensor_scalar(out=wsel[:, qb, :], in0=wsel[:, qb, :],
                                        scalar1=rgn[:, qb:qb + 1], scalar2=None, op0=OP.mult)

            # transpose per q-block -> wT_sb [NE, QL] (bf16)
            for qb in range(4):
                wtp = cg.tile([NE, P], R, tag="gmix")
                nc.tensor.transpose(wtp[:], wsel[:, qb, :], iden[:])
                nc.vector.tensor_copy(out=wT_sb[:, qb * P:(qb + 1) * P], in_=wtp[:])
            if KDBG:
                nc.sync.dma_start(dbg_x1.rearrange("(c p) q -> p c q", p=P)[:], x1[:])
                nc.sync.dma_start(dbg_wt[:], wT_sb[:])
            # broadcast per expert -> wbc [P, e, QL] (bf16)
            for e in range(NE):
                bcp = cps.tile([P, QL], F, tag="bigc")
                nc.tensor.matmul(bcp[:], sel8_sb[:, e * P:(e + 1) * P], wT_sb[:],
                                 start=True, stop=True)
                nc.vector.tensor_copy(out=wbc[:, e, :], in_=bcp[:])
            for c in range(NC):
                nc.vector.tensor_copy(out=x1b[:, c, :], in_=x1[:, c, :])

        # ===== Phase D: dense MoE with gate-scaled inputs, DVE accumulate =====
        moe = cres.tile([P, NC, QL], R, tag="moe")
        with tc.tile_pool(name="ewp", bufs=2) as ewp, \
             tc.tile_pool(name="xwp", bufs=2) as xwp, \
             tc.tile_pool(name="dps", bufs=4, space="PSUM") as dps:
            for e in range(NE):
                ew_sl = ewp.tile([P, NC, E], BF, tag="ew_sl")
                nc.sync.dma_start(ew_sl[:], ew[e].rearrange("c p d -> p c d"))
                xw = xwp.tile([P, NC, QL], BF, tag="xw")
                for dc in range(NC):
                    nc.vector.tensor_tensor(out=xw[:, dc, :], in0=x1b[:, dc, :],
                                            in1=wbc[:, e, :], op=OP.mult)
                for oc in range(NC):
                    yp = dps.tile([P, QL], F, tag="yps")
                    if e == 0:
                        nc.tensor.matmul(yp[:], ebT_sb[:, oc * P:(oc + 1) * P],
                                         wT_sb[:], start=True, stop=False)
                    for dc in range(NC):
                        nc.tensor.matmul(
                            yp[:], ew_sl[:, dc, oc * P:(oc + 1) * P], xw[:, dc, :],
                            start=(e != 0 and dc == 0), stop=(dc == NC - 1))
                    if e == 0:
                        # fold in the x1 residual for LN2 up front
                        nc.vector.tensor_tensor(out=moe[:, oc, :], in0=x1[:, oc, :],
                                                in1=yp[:], op=OP.add)
                    else:
                        nc.vector.tensor_tensor(out=moe[:, oc, :], in0=moe[:, oc, :],
                                                in1=yp[:], op=OP.add)

        if KDBG:
            nc.sync.dma_start(dbg_moe.rearrange("(c p) q -> p c q", p=P)[:], moe[:])
            nc.sync.dma_start(dbg_wbc.rearrange("p (e q) -> p e q", e=NE)[:], wbc[:])

        # ===== Phase E: LN2 + store =====
        with tc.tile_pool(name="et", bufs=3) as et, \
             tc.tile_pool(name="eps_", bufs=2, space="PSUM") as eps_, \
             tc.tile_pool(name="erow", bufs=1, space="PSUM") as erow:
            layernorm(moe, moe, ln_sb["ln2w"], ln_sb["ln2b"], et, erow, eps_)
            for c in range(NC):
                nc.sync.dma_start(out.rearrange("(c p) q -> p c q", p=P)[:, c, :], moe[:, c, :])

    nc.compile()
    return nc


def _prep_inputs(inputs):
    x = np.asarray(inputs['x'], dtype=np.float32)
    ipw = np.asarray(inputs['in_proj_w'], dtype=np.float32)
    ipb = np.asarray(inputs['in_proj_b'], dtype=np.float32)
    opw = np.asarray(inputs['out_proj_w'], dtype=np.float32)
    opb = np.asarray(inputs['out_proj_b'], dtype=np.float32)
    gww = np.asarray(inputs['gate_w'], dtype=np.float32)
    gbb = np.asarray(inputs['gate_b'], dtype=np.float32)
    eww = np.asarray(inputs['expert_w'], dtype=np.float32)
    ebb = np.asarray(inputs['expert_b'], dtype=np.float32)

    perm = np.empty(E, dtype=np.int64)
    idx = 0
    for h in range(H):
        for i in range(D // 2):
            perm[idx] = 64 * h + 2 * i; idx += 1
    for h in range(H):
        for i in range(D // 2):
            perm[idx] = 64 * h + 2 * i + 1; idx += 1

    Wq, Wk, Wv = ipw[0:E], ipw[E:2 * E], ipw[2 * E:3 * E]
    bq_, bk_, bv_ = ipb[0:E], ipb[E:2 * E], ipb[2 * E:3 * E]
    sel8 = np.zeros((NE, NE * P), dtype=np.float32)
    for e in range(NE):
        sel8[e, e * P:(e + 1) * P] = 1.0
    common = {
        "wq": np.ascontiguousarray(Wq[:, perm].T),
        "wk": np.ascontiguousarray(Wk[:, perm].T),
        "wv": np.ascontiguousarray(Wv[:, perm].T),
        "bq": np.ascontiguousarray(bq_.reshape(NC, P).T),
        "bk": np.ascontiguousarray(bk_.reshape(NC, P).T),
        "bvr": bv_.reshape(1, E).copy(),
        "wo": np.ascontiguousarray(opw.T),
        "bo": np.ascontiguousarray(opb.reshape(NC, P).T),
        "gw": np.ascontiguousarray(gww.T),
        "gb": gbb.reshape(NE, 1).copy(),
        "ew": np.ascontiguousarray(
            eww.transpose(0, 2, 1).reshape(NE, NC, P, E)).astype(ml_dtypes.bfloat16),
        "ebT": ebb,
        "sel8": sel8,
        "ln1w": np.ascontiguousarray(np.asarray(inputs['ln1_w'], np.float32).reshape(NC, P).T),
        "ln1b": np.ascontiguousarray(np.asarray(inputs['ln1_b'], np.float32).reshape(NC, P).T),
        "ln2w": np.ascontiguousarray(np.asarray(inputs['ln2_w'], np.float32).reshape(NC, P).T),
        "ln2b": np.ascontiguousarray(np.asarray(inputs['ln2_b'], np.float32).reshape(NC, P).T),
        "trid": np.ascontiguousarray(
            (np.arange(P)[None, :] >= np.arange(P)[:, None]).astype(np.float32)),
    }
    inv_freq = 1.0 / (10000.0 ** (np.arange(0, D, 2, dtype=np.float64) / D))
    freqs = np.arange(S, dtype=np.float64)[:, None] * inv_freq[None, :]
    cos_t = np.cos(freqs).T.astype(np.float32)
    sin_t = np.sin(freqs).T.astype(np.float32)
    common["cos2"] = np.ascontiguousarray(np.tile(cos_t, (4, 1)))
    common["sin2"] = np.ascontiguousarray(np.tile(sin_t, (4, 1)))

    in_maps = []
    for c in range(8):
        b, qc = c // 4, c % 4
        blocks = [qc + 4 * i for i in range(4)]
        cols = np.concatenate([np.arange(blk * P, (blk + 1) * P) for blk in blocks])
        xtb = np.ascontiguousarray(x[b].T)
        xtp = np.ascontiguousarray(xtb[perm])
        m = dict(common)
        m["xt"] = xtp
        m["xres"] = np.ascontiguousarray(xtb[:, cols])
        in_maps.append(m)
    return in_maps


def _run_multi(ncs, in_maps):
    """Run the 4 NEFFs concurrently: graph qc on devices {qc, qc+4} (b=0,1)."""
    import jax
    from jax.sharding import Mesh, PartitionSpec
    from jax.experimental.shard_map import shard_map
    from concourse import bass2jax
    from concourse import mybir as _mb

    bass2jax.install_neuronx_cc_hook()
    devices = jax.devices()

    if "jits" not in _cache:
        _cache["jits"] = {}
    handles = []
    for qc in range(4):
        nc = ncs[qc]
        if qc not in _cache["jits"]:
            in_names, out_names, out_avals, zero_outs = [], [], [], []
            for alloc in nc.m.functions[0].allocations:
                if not isinstance(alloc, _mb.MemoryLocationSet):
                    continue
                name = alloc.memorylocations[0].name
                if alloc.kind == "ExternalInput":
                    in_names.append(name)
                elif alloc.kind == "ExternalOutput":
                    out_names.append(name)
                    shape = tuple(alloc.tensor_shape)
                    dtype = _mb.dt.np(alloc.dtype)
                    out_avals.append(jax.core.ShapedArray(shape, dtype))
                    zero_outs.append(np.zeros(shape, dtype))
            n_params = len(in_names)
            all_names = in_names + out_names
            donate = tuple(range(n_params, n_params + len(out_names)))

            def _body(*args, _nc=nc, _avals=tuple(out_avals), _all=tuple(all_names),
                      _outs=tuple(out_names)):
                outs = bass2jax._bass_exec_p.bind(
                    *args, out_avals=_avals, in_names=_all, out_names=_outs,
                    lowering_input_output_aliases=(),
                    sim_require_finite=True, sim_require_nnan=True, nc=_nc)
                return tuple(outs)

            devs = [devices[qc], devices[qc + 4]]
            mesh = Mesh(np.asarray(devs), ("core",))
            nio = n_params + len(zero_outs)
            sharded = jax.jit(
                shard_map(_body, mesh=mesh,
                          in_specs=(PartitionSpec("core"),) * nio,
                          out_specs=(PartitionSpec("core"),) * len(out_names),
                          check_rep=False),
                donate_argnums=donate, keep_unused=True)
            _cache["jits"][qc] = (sharded, in_names, out_names, zero_outs)
        sharded, in_names, out_names, zero_outs = _cache["jits"][qc]
        per_core = [[np.asarray(in_maps[b * 4 + qc][n]) for n in in_names] for b in range(2)]
        concat_in = [np.concatenate([per_core[b][i] for b in range(2)], axis=0)
                     for i in range(len(in_names))]
        concat_zero = [np.concatenate([z, z], axis=0) for z in zero_outs]
        handles.append((sharded, concat_in, concat_zero, out_names))

    outs = []
    for sharded, concat_in, concat_zero, out_names in handles:
        outs.append((sharded(*concat_in, *concat_zero), out_names))
    results = [None] * 8
    for qc, (arrs, out_names) in enumerate(outs):
        arrs = [np.asarray(a) for a in arrs]
        for b in range(2):
            rm = {}
            for i, n in enumerate(out_names):
                full = arrs[i]
                half = full.shape[0] // 2
                rm[n] = full[b * half:(b + 1) * half]
            results[b * 4 + qc] = rm
    return results


def _ensure_ntff_hook():
    import types
    try:
        from antenv.axon_hooks import get_axon_ntff_profile_hook  # noqa
        return True
    except ImportError:
        pass
    try:
        import antenv
        sys.path.insert(0, '/root/.axon_site')
        from trn_agent_boot.trn_boot import _ntff_profile_via_ctypes
        hook = _ntff_profile_via_ctypes('/opt/axon/libaxon_pjrt.so')
        if hook is None:
            return False
        mod = types.ModuleType('antenv.axon_hooks')
        _state = {'hook': hook}
        mod.set_axon_ntff_profile_hook = lambda h: _state.__setitem__('hook', h)
        mod.get_axon_ntff_profile_hook = lambda: _state['hook']
        sys.modules['antenv.axon_hooks'] = mod
        antenv.axon_hooks = mod
        return True
    except Exception as e:
        print(f"ntff hook setup failed: {e}")
        return False


def kernel(**inputs):
    if "ncs" not in _cache:
        _cache["ncs"] = [_build(qc) for qc in range(4)]
    ncs = _cache["ncs"]
    in_maps = _prep_inputs(inputs)

    trace = bool(int(os.environ.get("KERNEL_TRACE", "0")))
    if trace and _ensure_ntff_hook():
        import tempfile
        from antenv.axon_hooks import get_axon_ntff_profile_hook
        hook = get_axon_ntff_profile_hook()
        tmpdir = tempfile.mkdtemp()
        _run_multi(ncs, in_maps)  # warm-up/compile outside the profile window
        with hook(tmpdir, list(range(8))):
            results = _run_multi(ncs, in_maps)
        _cache["ntff_dir"] = tmpdir
        print(f"ntff dir: {tmpdir}")
    else:
        results = _run_multi(ncs, in_maps)
    _cache["last_results"] = results

    out = np.empty((B, S, E), dtype=np.float32)
    for c in range(8):
        b, qc = c // 4, c % 4
        o = results[c]["out"]  # [E, QL]
        for i in range(4):
            blk = qc + 4 * i
            out[b, blk * P:(blk + 1) * P, :] = o[:, i * P:(i + 1) * P].T
    return out


# revision 48
# speedup vs baseline: 1.4412x; 1.0498x over previous
"""Trainium2 Bass kernel for nn_EnhancedTransformerLayer (RoPE attention + MoE).

Sharding: 8 cores; core c -> batch b=c//4, qc=c%4. Four distinct NEFFs (one per
qc), each run on 2 cores (b=0,1). Core qc owns interleaved query blocks
{qc, qc+4, qc+8, qc+12} (4 x 128 tokens) so causal work is balanced, and only
computes K/V up to its last block.

v2: fused rope/V/K span pipeline (single x read, in-place rope, no DRAM
round-trip), softmax denominator via ones-column in the ctx stationary,
fp32r matmuls clamped to N>=256, MoE accumulated directly in PSUM across
experts with gate-scaled inputs (no gpsimd, no per-expert combine).
"""
import sys, os
sys.path.insert(0, '/opt/trn_rl_repo')
import numpy as np
import ml_dtypes

import concourse.bass as bass
from concourse import bacc
import concourse.tile as tile
from concourse import mybir
from concourse.masks import make_identity

R = mybir.dt.float32r
F = mybir.dt.float32
BF = mybir.dt.bfloat16
P = 128
B, S, E, H, D, NE = 2, 2048, 1024, 16, 64, 8
NC = E // P
QL = 512
EXP_SCALE = 1.0 / (D ** 0.5)
LN_EPS = 1e-5

_cache = {}


def _kv_plan(qc):
    """K/V token coverage and span split for this qc."""
    kv_tok = 128 * (qc + 13)            # last interleaved block end
    rem = kv_tok - 1536
    rem = max(rem, 256)                 # fp32r needs N>=256
    kv_tok = 1536 + rem
    spans = [(0, 512), (512, 512), (1024, 512), (1536, rem)]
    return kv_tok, spans


def _build(qc):
    nc = bacc.Bacc("TRN2", target_bir_lowering=False, debug=False, num_devices=8,
                   name=f"moe_qc{qc}", enable_partition_id=False)
    kv_tok, kv_spans = _kv_plan(qc)
    KCN = kv_tok // P                   # key chunk count

    def din(name, shape, dt=R):
        return nc.dram_tensor(name, shape, dt, kind="ExternalInput")

    xt = din("xt", [E, S], BF)          # permuted x^T, bf16
    xres = din("xres", [E, QL])
    wq = din("wq", [E, E], BF); wk = din("wk", [E, E], BF); wv = din("wv", [E, E], BF)
    bq = din("bq", [P, NC], F); bk = din("bk", [P, NC], F)
    bvr = din("bvr", [1, E])
    wo = din("wo", [E, E]); bo = din("bo", [P, NC], F)
    gw = din("gw", [E, NE]); gb = din("gb", [NE, 1], F)
    cos2 = din("cos2", [P, S], BF); sin2 = din("sin2", [P, S], BF)
    trid = din("trid", [P, P])          # tri[k, q] = 1 if q >= k (within a block)
    ew = din("ew", [NE, NC, P, E], BF)
    ebT = din("ebT", [NE, E])           # expert bias, experts on partitions
    sel8 = din("sel8", [NE, NE * P])    # sel8[:, e*P:(e+1)*P] row e = 1
    ln1w = din("ln1w", [P, NC], F); ln1b = din("ln1b", [P, NC], F)
    ln2w = din("ln2w", [P, NC], F); ln2b = din("ln2b", [P, NC], F)
    out = nc.dram_tensor("out", [E, QL], R, kind="ExternalOutput")
    KDBG = bool(int(os.environ.get("KDBG", "0")))
    dbg_kind = "ExternalOutput" if KDBG else "Internal"
    vsch = [nc.dram_tensor(f"vsc{i}", [kv_tok, 512], BF,
                           kind=dbg_kind) for i in range(2)]
    if KDBG:
        ctxd = nc.dram_tensor("ctxd", [E, QL], R, kind="ExternalOutput")
        dbg_qx = nc.dram_tensor("dbg_qx", [E, QL], BF, kind="ExternalOutput")
        dbg_qt = nc.dram_tensor("dbg_qt", [E, QL], BF, kind="ExternalOutput")
        dbg_kt = nc.dram_tensor("dbg_kt", [P, kv_tok], BF, kind="ExternalOutput")
        dbg_x1 = nc.dram_tensor("dbg_x1", [E, QL], R, kind="ExternalOutput")
        dbg_wt = nc.dram_tensor("dbg_wt", [NE, QL], R, kind="ExternalOutput")
        dbg_wbc = nc.dram_tensor("dbg_wbc", [P, NE * QL], BF, kind="ExternalOutput")
        dbg_moe = nc.dram_tensor("dbg_moe", [E, QL], R, kind="ExternalOutput")

    AX = mybir.AxisListType.X
    OP = mybir.AluOpType
    AF = mybir.ActivationFunctionType
    import contextlib

    with tile.TileContext(nc) as tc, \
         nc.allow_low_precision(reason="float32r is bit-identical to float32"), \
         contextlib.ExitStack() as es:
        consts = es.enter_context(tc.tile_pool(name="consts", bufs=1))

        ones_f = consts.tile([P, 1], F, tag="ones_f")
        nc.vector.memset(ones_f[:], 1.0)
        ones = consts.tile([P, 1], R, tag="ones")
        nc.vector.tensor_copy(out=ones[:], in_=ones_f[:])
        ones1_f = consts.tile([1, P], F, tag="ones1_f")
        nc.vector.memset(ones1_f[:], 1.0)
        ones1 = consts.tile([1, P], R, tag="ones1")
        nc.vector.tensor_copy(out=ones1[:], in_=ones1_f[:])
        eps1 = consts.tile([1, 1], F, tag="eps1")
        nc.vector.memset(eps1[:], LN_EPS)
        bq_sb = consts.tile([P, NC], F, tag="bq"); nc.sync.dma_start(bq_sb[:], bq[:])
        bk_sb = consts.tile([P, NC], F, tag="bk"); nc.sync.dma_start(bk_sb[:], bk[:])

        # ===== Phase A: fused V-proj + rope + K-proj per 512-token span =====
        xt_r = xt.rearrange("(c p) s -> p c s", p=P)
        ctx_p = es.enter_context(tc.tile_pool(name="ctx_p", bufs=1))
        ctx_sb = ctx_p.tile([P, NC, QL], R, tag="ctx")
        attn_cm = tc.tile_pool(name="attn_res", bufs=1)
        attn_res = attn_cm.__enter__()
        kTs = [attn_res.tile([P, kv_tok], BF, tag=f"kT{oc}", name=f"kT{oc}")
               for oc in range(NC)]
        qT = attn_res.tile([P, NC, QL], BF, tag="qT")
        qx_cm = tc.tile_pool(name="qx_p", bufs=1)
        qx_p = qx_cm.__enter__()
        qx = qx_p.tile([P, NC, QL], BF, tag="qx")

        with tc.tile_pool(name="xsp_p", bufs=2) as xsp_p, \
             tc.tile_pool(name="cs_p", bufs=2) as cs_p, \
             tc.tile_pool(name="a1tmp", bufs=1) as a1tmp, \
             tc.tile_pool(name="wvkp", bufs=1) as wvkp, \
             tc.tile_pool(name="vev_p", bufs=2) as vev_p, \
             tc.tile_pool(name="vps", bufs=4, space="PSUM") as vps, \
             tc.tile_pool(name="kps", bufs=3, space="PSUM") as kps:
            xsp_t, cs_t = {}, {}

            def emit_dma(sp):
                s0, sl = kv_spans[sp]
                xsp = xsp_p.tile([P, NC, 512], BF, tag="xsp")
                nc.sync.dma_start(xsp[:, :, :sl], xt_r[:, :, s0:s0 + sl])
                csb = cs_p.tile([P, 2, 512], BF, tag="csb")
                nc.sync.dma_start(csb[:, 0, :sl], cos2[:, s0:s0 + sl])
                nc.sync.dma_start(csb[:, 1, :sl], sin2[:, s0:s0 + sl])
                xsp_t[sp] = xsp
                cs_t[sp] = csb

            def emit_V(sp):
                s0, sl = kv_spans[sp]
                xsp = xsp_t[sp]
                for t in range(sl // P):
                    for dvs in range(2):
                        vp = vps.tile([P, 512], F, tag="vps")
                        for dc in range(NC):
                            nc.tensor.matmul(
                                vp[:], xsp[:, dc, t * P:(t + 1) * P],
                                wv_sb[:, dc, dvs * 512:(dvs + 1) * 512],
                                start=(dc == 0), stop=False)
                        nc.tensor.matmul(
                            vp[:], ones1[:, :], bv_sb[:, dvs * 512:(dvs + 1) * 512],
                            start=False, stop=True)
                        vev = vev_p.tile([P, 512], BF, tag="vev")
                        nc.scalar.copy(out=vev[:], in_=vp[:])
                        nc.sync.dma_start(
                            vsch[dvs][s0 + t * P:s0 + (t + 1) * P, :], vev[:])

            def emit_rope(sp):
                s0, sl = kv_spans[sp]
                xsp, csb = xsp_t[sp], cs_t[sp]
                cs = csb[:, 0, :sl]; sn = csb[:, 1, :sl]
                for c in range(4):
                    # in-place rotation: a' = a*cos - b*sin; b' = a*sin + b*cos
                    t1 = a1tmp.tile([P, 512], BF, tag="ropet1")
                    t2 = a1tmp.tile([P, 512], BF, tag="ropet2")
                    a = xsp[:, c, :sl]; b = xsp[:, c + 4, :sl]
                    nc.vector.tensor_tensor(out=t1[:, :sl], in0=a, in1=sn, op=OP.mult)
                    nc.vector.tensor_tensor(out=t2[:, :sl], in0=b, in1=sn, op=OP.mult)
                    nc.vector.tensor_tensor(out=a, in0=a, in1=cs, op=OP.mult)
                    nc.vector.tensor_tensor(out=a, in0=a, in1=t2[:, :sl], op=OP.subtract)
                    nc.vector.tensor_tensor(out=b, in0=b, in1=cs, op=OP.mult)
                    nc.vector.tensor_tensor(out=b, in0=b, in1=t1[:, :sl], op=OP.add)
                # q block sp = global block qc+4*sp lives at cols qc*P within span sp
                for c in range(NC):
                    nc.vector.tensor_copy(
                        out=qx[:, c, sp * P:(sp + 1) * P],
                        in_=xsp[:, c, qc * P:(qc + 1) * P])

            def emit_K(sp):
                s0, sl = kv_spans[sp]
                xsp = xsp_t[sp]
                for oc in range(NC):
                    kp = kps.tile([P, 512], F, tag="kps")
                    for dc in range(NC):
                        nc.tensor.matmul(
                            kp[:, :sl], wk_sb[:, dc, oc * P:(oc + 1) * P],
                            xsp[:, dc, :sl],
                            start=(dc == 0), stop=(dc == NC - 1))
                    nc.vector.tensor_scalar(
                        out=kTs[oc][:, s0:s0 + sl], in0=kp[:, :sl],
                        scalar1=bk_sb[:, oc:oc + 1], scalar2=None, op0=OP.add)

            # x/cos/sin for span 0 first, then wv, so V matmuls start ASAP;
            # wk loads while V(0) computes.
            emit_dma(0)
            wv_sb = wvkp.tile([P, NC, E], BF, tag="wv")
            for c in range(NC):
                nc.sync.dma_start(wv_sb[:, c, :],
                                  wv.rearrange("(c p) m -> p c m", p=P)[:, c, :])
            bv_sb = wvkp.tile([1, E], R, tag="bv")
            nc.sync.dma_start(bv_sb[:], bvr[:])
            emit_V(0)
            wk_sb = wvkp.tile([P, NC, E], BF, tag="wk")
            for c in range(NC):
                nc.sync.dma_start(wk_sb[:, c, :],
                                  wk.rearrange("(c p) m -> p c m", p=P)[:, c, :])
            emit_rope(0)
            emit_dma(1); emit_V(1); emit_K(0); emit_rope(1)
            emit_dma(2); emit_V(2); emit_K(1); emit_rope(2)
            emit_dma(3); emit_V(3); emit_K(2); emit_rope(3)
            emit_K(3)

        # ===== Q projection from roped q blocks in SBUF =====
        with tc.tile_pool(name="wqp", bufs=1) as wqp, \
             tc.tile_pool(name="qps_p", bufs=4, space="PSUM") as qps_p:
            wq_sb = wqp.tile([P, NC, E], BF, tag="wq_sb")
            for c in range(NC):
                nc.sync.dma_start(wq_sb[:, c, :],
                                  wq.rearrange("(c p) m -> p c m", p=P)[:, c, :])
            for oc in range(NC):
                qp = qps_p.tile([P, 512], F, tag="qps")
                for dc in range(NC):
                    nc.tensor.matmul(
                        qp[:], wq_sb[:, dc, oc * P:(oc + 1) * P], qx[:, dc, :],
                        start=(dc == 0), stop=(dc == NC - 1))
                nc.vector.tensor_scalar(
                    out=qT[:, oc, :], in0=qp[:],
                    scalar1=bq_sb[:, oc:oc + 1], scalar2=None, op0=OP.add)
            if KDBG:
                nc.sync.dma_start(dbg_qx.rearrange("(c p) q -> p c q", p=P)[:], qx[:])
                nc.sync.dma_start(dbg_qt.rearrange("(c p) q -> p c q", p=P)[:], qT[:])
                nc.sync.dma_start(dbg_kt[:], kTs[0][:])
        qx_cm.__exit__(None, None, None)

        # ===== Phase B: causal attention over interleaved blocks =====
        # ctx stationary vt has a ones column per head half: row 64 of the
        # [65, QL] ctx psum accumulates the softmax denominator for free.
        kcs = [kc for kc in range(KCN) if -(-(kc - qc) // 4) < 4]
        with tc.tile_pool(name="bt", bufs=6) as bt, \
             tc.tile_pool(name="bconst", bufs=1) as bconst, \
             tc.tile_pool(name="vt_p", bufs=2) as vt_p, \
             tc.tile_pool(name="scps", bufs=2, space="PSUM") as scps, \
             tc.tile_pool(name="ctxps", bufs=4, space="PSUM") as ctxps:
            tri_f = bconst.tile([P, P], R, tag="tri_f")
            nc.sync.dma_start(tri_f[:], trid[:])
            tri_sb = bconst.tile([P, P], BF, tag="tri")
            nc.vector.tensor_copy(out=tri_sb[:], in_=tri_f[:])
            ones65_f = bconst.tile([65, P], F, tag="ones65_f")
            nc.vector.memset(ones65_f[:], 1.0)
            ones65 = bconst.tile([65, P], BF, tag="ones65")
            nc.vector.tensor_copy(out=ones65[:], in_=ones65_f[:])
            oneskc = bconst.tile([P, KCN], F, tag="oneskc")
            nc.vector.memset(oneskc[:], 1.0)
            for hp in range(NC):
                vt = vt_p.tile([P, KCN, 130], BF, tag="vt")
                for hh in range(2):
                    nc.sync.dma_start(
                        vt[:, :, hh * 65:hh * 65 + 64],
                        vsch[hp // 4].rearrange("(k p) d -> p k d", p=P)
                        [:, :, (hp % 4) * P + hh * 64:(hp % 4) * P + (hh + 1) * 64])
                    nc.vector.tensor_copy(out=vt[:, :, hh * 65 + 64:hh * 65 + 65],
                                          in_=oneskc[:])
                ctxp2 = [ctxps.tile([65, QL], F, tag="ctxps", name=f"ctxp_{hp}_{hh}")
                         for hh in range(2)]

                # software pipeline: issue scores for block i+1 before ctx of
                # block i, so ctx matmuls never stall on the Scalar exp.
                # Both head halves share one 2-bank psum tile -> single exp op.
                def emit_scores(kc):
                    j0 = max(0, -(-(kc - qc) // 4))
                    c0 = q0 = j0 * P
                    scp = scps.tile([P, 2, QL], F, tag="scps")
                    for hh in range(2):
                        nc.tensor.matmul(
                            scp[:, hh, c0:], kTs[hp][hh * 64:(hh + 1) * 64, kc * P:(kc + 1) * P],
                            qT[hh * 64:(hh + 1) * 64, hp, c0:], start=True, stop=True)
                    st = bt.tile([P, 2, QL], BF, tag="st")
                    nc.scalar.activation(out=st[:, :, c0:], in_=scp[:, :, c0:],
                                         func=AF.Exp, scale=EXP_SCALE)
                    if kc >= qc and (kc - qc) % 4 == 0:
                        j = (kc - qc) // 4
                        for hh in range(2):
                            nc.vector.tensor_tensor(
                                out=st[:, hh, j * P:(j + 1) * P],
                                in0=st[:, hh, j * P:(j + 1) * P],
                                in1=tri_sb[:], op=OP.mult)
                    return (kc, c0, st)

                def emit_ctx(pend):
                    kc, c0, st = pend
                    for hh in range(2):
                        nc.tensor.matmul(ctxp2[hh][:, c0:],
                                         vt[:, kc, hh * 65:(hh + 1) * 65], st[:, hh, c0:],
                                         start=(kc == kcs[0]), stop=(kc == kcs[-1]))

                pend = None
                for kc in kcs:
                    cur = emit_scores(kc)
                    if pend is not None:
                        emit_ctx(pend)
                    pend = cur
                emit_ctx(pend)
                for hh in range(2):
                    rden = bt.tile([65, QL], BF, tag="rden")
                    nc.vector.reciprocal(out=rden[64:65, :], in_=ctxp2[hh][64:65, :])
                    rbp = ctxps.tile([65, QL], F, tag="ctxps", name=f"rbp_{hp}_{hh}")
                    nc.tensor.matmul(rbp[0:64, :], ones65[64:65, 0:64], rden[64:65, :],
                                     start=True, stop=True)
                    rb_sb = bt.tile([64, QL], R, tag="rb_sb")
                    nc.scalar.copy(out=rb_sb[:], in_=rbp[0:64, :])
                    nc.vector.tensor_tensor(
                        out=ctx_sb[hh * 64:(hh + 1) * 64, hp, :],
                        in0=ctxp2[hh][0:64, :], in1=rb_sb[:], op=OP.mult)
        attn_cm.__exit__(None, None, None)
        if KDBG:
            nc.sync.dma_start(ctxd.rearrange("(c p) q -> p c q", p=P)[:], ctx_sb[:])

        # ===== LN helper =====
        def layernorm(src, dst, wtile, btile, tmp, ps_row, ps_big):
            sp_ = ps_row.tile([1, QL], F, tag="lnsum")
            for c in range(NC):
                nc.tensor.matmul(sp_[:], ones[:], src[:, c, :],
                                 start=(c == 0), stop=(c == NC - 1))
            s2p = ps_row.tile([1, QL], F, tag="lnsum2")
            for c in range(NC):
                sq = tmp.tile([P, QL], R, tag="lnsq")
                nc.scalar.activation(out=sq[:], in_=src[:, c, :], func=AF.Square)
                nc.tensor.matmul(s2p[:], ones[:], sq[:],
                                 start=(c == 0), stop=(c == NC - 1))
            mean = tmp.tile([1, QL], R, tag="lnmean")
            nc.scalar.mul(out=mean[:], in_=sp_[:], mul=1.0 / E)
            msq = tmp.tile([1, QL], R, tag="lnmsq")
            nc.scalar.mul(out=msq[:], in_=s2p[:], mul=1.0 / E)
            var = tmp.tile([1, QL], R, tag="lnvar")
            nc.vector.tensor_tensor(out=var[:], in0=mean[:], in1=mean[:], op=OP.mult)
            nc.vector.tensor_tensor(out=var[:], in0=msq[:], in1=var[:], op=OP.subtract)
            std = tmp.tile([1, QL], R, tag="lnstd")
            nc.scalar.activation(out=std[:], in_=var[:], func=AF.Sqrt, bias=eps1[:])
            rstd = tmp.tile([1, QL], R, tag="lnrstd")
            nc.vector.reciprocal(out=rstd[:], in_=std[:])
            mb = ps_big.tile([P, QL], F, tag="bigc")
            nc.tensor.matmul(mb[:], ones1[:], mean[:], start=True, stop=True)
            rb = ps_big.tile([P, QL], F, tag="bigc")
            nc.tensor.matmul(rb[:], ones1[:], rstd[:], start=True, stop=True)
            mbs = tmp.tile([P, QL], R, tag="lnmbs")
            nc.scalar.copy(out=mbs[:], in_=mb[:])
            rbs = tmp.tile([P, QL], R, tag="lnrbs")
            nc.scalar.copy(out=rbs[:], in_=rb[:])
            for c in range(NC):
                t = tmp.tile([P, QL], R, tag="lnt")
                nc.vector.tensor_tensor(out=t[:], in0=src[:, c, :], in1=mbs[:], op=OP.subtract)
                nc.vector.tensor_tensor(out=t[:], in0=t[:], in1=rbs[:], op=OP.mult)
                nc.vector.tensor_scalar(out=dst[:, c, :], in0=t[:],
                                        scalar1=wtile[:, c:c + 1], scalar2=btile[:, c:c + 1],
                                        op0=OP.mult, op1=OP.add)

        # ===== Phase C: out-proj + LN1 + gates + top-2 weights =====
        cres = es.enter_context(tc.tile_pool(name="cres", bufs=1))
        x1 = cres.tile([P, NC, QL], R, tag="x1")
        x1b = cres.tile([P, NC, QL], BF, tag="x1b")
        wbc = cres.tile([P, NE, QL], BF, tag="wbc")
        wT_sb = cres.tile([NE, QL], R, tag="wT")
        ebT_sb = cres.tile([NE, E], R, tag="ebT"); nc.sync.dma_start(ebT_sb[:], ebT[:])
        ln_sb = {}
        for nm, t in (("ln1w", ln1w), ("ln1b", ln1b), ("ln2w", ln2w), ("ln2b", ln2b)):
            ln_sb[nm] = cres.tile([P, NC], F, tag=nm, name=nm)
            nc.sync.dma_start(ln_sb[nm][:], t[:])
        with tc.tile_pool(name="cslab", bufs=1) as cslab, \
             tc.tile_pool(name="ct", bufs=2) as ct, \
             tc.tile_pool(name="h1p", bufs=1) as h1p, \
             tc.tile_pool(name="cps", bufs=3, space="PSUM") as cps, \
             tc.tile_pool(name="crow", bufs=1, space="PSUM") as crow, \
             tc.tile_pool(name="cg", bufs=1, space="PSUM") as cg:
            bo_sb = cslab.tile([P, NC], F, tag="bo"); nc.sync.dma_start(bo_sb[:], bo[:])
            gb_sb = cslab.tile([NE, 1], F, tag="gb"); nc.sync.dma_start(gb_sb[:], gb[:])
            gw_sb = cslab.tile([P, NC, NE], R, tag="gw")
            nc.sync.dma_start(gw_sb[:], gw.rearrange("(c p) g -> p c g", p=P))
            sel8_sb = cslab.tile([NE, NE * P], R, tag="sel8")
            nc.sync.dma_start(sel8_sb[:], sel8[:])
            iden_f = cslab.tile([P, P], F, tag="iden_f")
            make_identity(nc, iden_f[:])
            iden = cslab.tile([P, P], R, tag="iden")
            nc.vector.tensor_copy(out=iden[:], in_=iden_f[:])
            xres_sb = h1p.tile([P, NC, QL], R, tag="xres")
            nc.sync.dma_start(xres_sb[:], xres.rearrange("(c p) q -> p c q", p=P))
            h1 = h1p.tile([P, NC, QL], R, tag="h1")
            wo_r = wo.rearrange("(c p) m -> p c m", p=P)
            for oc in range(NC):
                wo_sl = cslab.tile([P, NC, P], R, tag="wo_sl")
                nc.sync.dma_start(wo_sl[:], wo_r[:, :, oc * P:(oc + 1) * P])
                ap = cps.tile([P, QL], F, tag="bigc")
                for dc in range(NC):
                    nc.tensor.matmul(ap[:], wo_sl[:, dc, :],
                                     ctx_sb[:, dc, :],
                                     start=(dc == 0), stop=(dc == NC - 1))
                nc.vector.tensor_scalar(out=h1[:, oc, :], in0=ap[:],
                                        scalar1=bo_sb[:, oc:oc + 1], scalar2=None, op0=OP.add)
                nc.vector.tensor_tensor(out=h1[:, oc, :], in0=h1[:, oc, :],
                                        in1=xres_sb[:, oc, :], op=OP.add)

            layernorm(h1, x1, ln_sb["ln1w"], ln_sb["ln1b"], ct, crow, cps)

            gp = cg.tile([NE, QL], F, tag="gps")
            for c in range(NC):
                nc.tensor.matmul(gp[:], gw_sb[:, c, :], x1[:, c, :],
                                 start=(c == 0), stop=(c == NC - 1))
            glog = ct.tile([NE, QL], R, tag="glog")
            nc.vector.tensor_scalar(out=glog[:], in0=gp[:], scalar1=gb_sb[:],
                                    scalar2=None, op0=OP.add)
            gexp = ct.tile([NE, QL], R, tag="gexp")
            nc.scalar.activation(out=gexp[:], in_=glog[:], func=AF.Exp)

            # token-orientation gate math: gt[tok, qb, e]
            gt = ct.tile([P, 4, NE], R, tag="gt")
            for qb in range(4):
                gtp = cg.tile([P, NE], R, tag="gmix")
                nc.tensor.transpose(gtp[:], gexp[:, qb * P:(qb + 1) * P], iden[0:NE, 0:NE])
                nc.vector.tensor_copy(out=gt[:, qb, :], in_=gtp[:])
            dsum = ct.tile([P, 4], F, tag="dsum")
            nc.vector.reduce_sum(out=dsum[:], in_=gt[:], axis=AX)
            rgn = ct.tile([P, 4], F, tag="rgn")
            nc.vector.reciprocal(out=rgn[:], in_=dsum[:])
            m1 = ct.tile([P, 4], F, tag="m1")
            nc.vector.reduce_max(out=m1[:], in_=gt[:], axis=AX)
            msel = ct.tile([P, 4, NE], R, tag="msel")
            g2 = ct.tile([P, 4, NE], R, tag="g2")
            for qb in range(4):
                nc.vector.tensor_scalar(out=msel[:, qb, :], in0=gt[:, qb, :],
                                        scalar1=m1[:, qb:qb + 1], scalar2=None, op0=OP.is_equal)
            nc.vector.tensor_tensor(out=g2[:], in0=gt[:], in1=msel[:], op=OP.mult)
            nc.vector.tensor_tensor(out=g2[:], in0=gt[:], in1=g2[:], op=OP.subtract)
            m2 = ct.tile([P, 4], F, tag="m2")
            nc.vector.reduce_max(out=m2[:], in_=g2[:], axis=AX)
            msel2 = ct.tile([P, 4, NE], R, tag="msel2")
            for qb in range(4):
                nc.vector.tensor_scalar(out=msel2[:, qb, :], in0=g2[:, qb, :],
                                        scalar1=m2[:, qb:qb + 1], scalar2=None, op0=OP.is_equal)
            nc.vector.tensor_tensor(out=msel[:], in0=msel[:], in1=msel2[:], op=OP.add)
            wsel = ct.tile([P, 4, NE], R, tag="wsel")
            nc.vector.tensor_tensor(out=wsel[:], in0=gt[:], in1=msel[:], op=OP.mult)
            for qb in range(4):
                nc.vector.tensor_scalar(out=wsel[:, qb, :], in0=wsel[:, qb, :],
                                        scalar1=rgn[:, qb:qb + 1], scalar2=None, op0=OP.mult)

            # transpose per q-block -> wT_sb [NE, QL] (bf16)
            for qb in range(4):
                wtp = cg.tile([NE, P], R, tag="gmix")
                nc.tensor.transpose(wtp[:], wsel[:, qb, :], iden[:])
                nc.vector.tensor_copy(out=wT_sb[:, qb * P:(qb + 1) * P], in_=wtp[:])
            if KDBG:
                nc.sync.dma_start(dbg_x1.rearrange("(c p) q -> p c q", p=P)[:], x1[:])
                nc.sync.dma_start(dbg_wt[:], wT_sb[:])
            # broadcast per expert -> wbc [P, e, QL] (bf16)
            for e in range(NE):
                bcp = cps.tile([P, QL], F, tag="bigc")
                nc.tensor.matmul(bcp[:], sel8_sb[:, e * P:(e + 1) * P], wT_sb[:],
                                 start=True, stop=True)
                nc.vector.tensor_copy(out=wbc[:, e, :], in_=bcp[:])
            for c in range(NC):
                nc.vector.tensor_copy(out=x1b[:, c, :], in_=x1[:, c, :])

        # ===== Phase D: dense MoE with gate-scaled inputs, DVE accumulate =====
        moe = cres.tile([P, NC, QL], R, tag="moe")
        with tc.tile_pool(name="ewp", bufs=2) as ewp, \
             tc.tile_pool(name="xwp", bufs=2) as xwp, \
             tc.tile_pool(name="dps", bufs=4, space="PSUM") as dps:
            for e in range(NE):
                ew_sl = ewp.tile([P, NC, E], BF, tag="ew_sl")
                nc.sync.dma_start(ew_sl[:], ew[e].rearrange("c p d -> p c d"))
                xw = xwp.tile([P, NC, QL], BF, tag="xw")
                for dc in range(NC):
                    nc.vector.tensor_tensor(out=xw[:, dc, :], in0=x1b[:, dc, :],
                                            in1=wbc[:, e, :], op=OP.mult)
                for oc in range(NC):
                    yp = dps.tile([P, QL], F, tag="yps")
                    if e == 0:
                        nc.tensor.matmul(yp[:], ebT_sb[:, oc * P:(oc + 1) * P],
                                         wT_sb[:], start=True, stop=False)
                    for dc in range(NC):
                        nc.tensor.matmul(
                            yp[:], ew_sl[:, dc, oc * P:(oc + 1) * P], xw[:, dc, :],
                            start=(e != 0 and dc == 0), stop=(dc == NC - 1))
                    if e == 0:
                        # fold in the x1 residual for LN2 up front
                        nc.vector.tensor_tensor(out=moe[:, oc, :], in0=x1[:, oc, :],
                                                in1=yp[:], op=OP.add)
                    else:
                        nc.vector.tensor_tensor(out=moe[:, oc, :], in0=moe[:, oc, :],
                                                in1=yp[:], op=OP.add)

        if KDBG:
            nc.sync.dma_start(dbg_moe.rearrange("(c p) q -> p c q", p=P)[:], moe[:])
            nc.sync.dma_start(dbg_wbc.rearrange("p (e q) -> p e q", e=NE)[:], wbc[:])

        # ===== Phase E: LN2 + store =====
        with tc.tile_pool(name="et", bufs=3) as et, \
             tc.tile_pool(name="eps_", bufs=2, space="PSUM") as eps_, \
             tc.tile_pool(name="erow", bufs=1, space="PSUM") as erow:
            layernorm(moe, moe, ln_sb["ln2w"], ln_sb["ln2b"], et, erow, eps_)
            for c in range(NC):
                nc.sync.dma_start(out.rearrange("(c p) q -> p c q", p=P)[:, c, :], moe[:, c, :])

    nc.compile()
    return nc


def _prep_inputs(inputs):
    x = np.asarray(inputs['x'], dtype=np.float32)
    ipw = np.asarray(inputs['in_proj_w'], dtype=np.float32)
    ipb = np.asarray(inputs['in_proj_b'], dtype=np.float32)
    opw = np.asarray(inputs['out_proj_w'], dtype=np.float32)
    opb = np.asarray(inputs['out_proj_b'], dtype=np.float32)
    gww = np.asarray(inputs['gate_w'], dtype=np.float32)
    gbb = np.asarray(inputs['gate_b'], dtype=np.float32)
    eww = np.asarray(inputs['expert_w'], dtype=np.float32)
    ebb = np.asarray(inputs['expert_b'], dtype=np.float32)

    perm = np.empty(E, dtype=np.int64)
    idx = 0
    for h in range(H):
        for i in range(D // 2):
            perm[idx] = 64 * h + 2 * i; idx += 1
    for h in range(H):
        for i in range(D // 2):
            perm[idx] = 64 * h + 2 * i + 1; idx += 1

    Wq, Wk, Wv = ipw[0:E], ipw[E:2 * E], ipw[2 * E:3 * E]
    bq_, bk_, bv_ = ipb[0:E], ipb[E:2 * E], ipb[2 * E:3 * E]
    sel8 = np.zeros((NE, NE * P), dtype=np.float32)
    for e in range(NE):
        sel8[e, e * P:(e + 1) * P] = 1.0
    common = {
        "wq": np.ascontiguousarray(Wq[:, perm].T).astype(ml_dtypes.bfloat16),
        "wk": np.ascontiguousarray(Wk[:, perm].T).astype(ml_dtypes.bfloat16),
        "wv": np.ascontiguousarray(Wv[:, perm].T).astype(ml_dtypes.bfloat16),
        "bq": np.ascontiguousarray(bq_.reshape(NC, P).T),
        "bk": np.ascontiguousarray(bk_.reshape(NC, P).T),
        "bvr": bv_.reshape(1, E).copy(),
        "wo": np.ascontiguousarray(opw.T),
        "bo": np.ascontiguousarray(opb.reshape(NC, P).T),
        "gw": np.ascontiguousarray(gww.T),
        "gb": gbb.reshape(NE, 1).copy(),
        "ew": np.ascontiguousarray(
            eww.transpose(0, 2, 1).reshape(NE, NC, P, E)).astype(ml_dtypes.bfloat16),
        "ebT": ebb,
        "sel8": sel8,
        "ln1w": np.ascontiguousarray(np.asarray(inputs['ln1_w'], np.float32).reshape(NC, P).T),
        "ln1b": np.ascontiguousarray(np.asarray(inputs['ln1_b'], np.float32).reshape(NC, P).T),
        "ln2w": np.ascontiguousarray(np.asarray(inputs['ln2_w'], np.float32).reshape(NC, P).T),
        "ln2b": np.ascontiguousarray(np.asarray(inputs['ln2_b'], np.float32).reshape(NC, P).T),
        "trid": np.ascontiguousarray(
            (np.arange(P)[None, :] >= np.arange(P)[:, None]).astype(np.float32)),
    }
    inv_freq = 1.0 / (10000.0 ** (np.arange(0, D, 2, dtype=np.float64) / D))
    freqs = np.arange(S, dtype=np.float64)[:, None] * inv_freq[None, :]
    cos_t = np.cos(freqs).T.astype(np.float32)
    sin_t = np.sin(freqs).T.astype(np.float32)
    common["cos2"] = np.ascontiguousarray(np.tile(cos_t, (4, 1))).astype(ml_dtypes.bfloat16)
    common["sin2"] = np.ascontiguousarray(np.tile(sin_t, (4, 1))).astype(ml_dtypes.bfloat16)

    in_maps = []
    for c in range(8):
        b, qc = c // 4, c % 4
        blocks = [qc + 4 * i for i in range(4)]
        cols = np.concatenate([np.arange(blk * P, (blk + 1) * P) for blk in blocks])
        xtb = np.ascontiguousarray(x[b].T)
        xtp = np.ascontiguousarray(xtb[perm])
        m = dict(common)
        m["xt"] = xtp.astype(ml_dtypes.bfloat16)
        m["xres"] = np.ascontiguousarray(xtb[:, cols])
        in_maps.append(m)
    return in_maps


def _run_multi(ncs, in_maps):
    """Run the 4 NEFFs concurrently: graph qc on devices {qc, qc+4} (b=0,1)."""
    import jax
    from jax.sharding import Mesh, PartitionSpec
    from jax.experimental.shard_map import shard_map
    from concourse import bass2jax
    from concourse import mybir as _mb

    bass2jax.install_neuronx_cc_hook()
    devices = jax.devices()

    if "jits" not in _cache:
        _cache["jits"] = {}
    handles = []
    for qc in range(4):
        nc = ncs[qc]
        if qc not in _cache["jits"]:
            in_names, out_names, out_avals, zero_outs = [], [], [], []
            for alloc in nc.m.functions[0].allocations:
                if not isinstance(alloc, _mb.MemoryLocationSet):
                    continue
                name = alloc.memorylocations[0].name
                if alloc.kind == "ExternalInput":
                    in_names.append(name)
                elif alloc.kind == "ExternalOutput":
                    out_names.append(name)
                    shape = tuple(alloc.tensor_shape)
                    dtype = _mb.dt.np(alloc.dtype)
                    out_avals.append(jax.core.ShapedArray(shape, dtype))
                    zero_outs.append(np.zeros(shape, dtype))
            n_params = len(in_names)
            all_names = in_names + out_names
            donate = tuple(range(n_params, n_params + len(out_names)))

            def _body(*args, _nc=nc, _avals=tuple(out_avals), _all=tuple(all_names),
                      _outs=tuple(out_names)):
                outs = bass2jax._bass_exec_p.bind(
                    *args, out_avals=_avals, in_names=_all, out_names=_outs,
                    lowering_input_output_aliases=(),
                    sim_require_finite=True, sim_require_nnan=True, nc=_nc)
                return tuple(outs)

            devs = [devices[qc], devices[qc + 4]]
            mesh = Mesh(np.asarray(devs), ("core",))
            nio = n_params + len(zero_outs)
            sharded = jax.jit(
                shard_map(_body, mesh=mesh,
                          in_specs=(PartitionSpec("core"),) * nio,
                          out_specs=(PartitionSpec("core"),) * len(out_names),
                          check_rep=False),
                donate_argnums=donate, keep_unused=True)
            _cache["jits"][qc] = (sharded, in_names, out_names, zero_outs)
        sharded, in_names, out_names, zero_outs = _cache["jits"][qc]
        per_core = [[np.asarray(in_maps[b * 4 + qc][n]) for n in in_names] for b in range(2)]
        concat_in = [np.concatenate([per_core[b][i] for b in range(2)], axis=0)
                     for i in range(len(in_names))]
        concat_zero = [np.concatenate([z, z], axis=0) for z in zero_outs]
        handles.append((sharded, concat_in, concat_zero, out_names))

    outs = []
    for sharded, concat_in, concat_zero, out_names in handles:
        outs.append((sharded(*concat_in, *concat_zero), out_names))
    results = [None] * 8
    for qc, (arrs, out_names) in enumerate(outs):
        arrs = [np.asarray(a) for a in arrs]
        for b in range(2):
            rm = {}
            for i, n in enumerate(out_names):
                full = arrs[i]
                half = full.shape[0] // 2
                rm[n] = full[b * half:(b + 1) * half]
            results[b * 4 + qc] = rm
    return results


def _ensure_ntff_hook():
    import types
    try:
        from antenv.axon_hooks import get_axon_ntff_profile_hook  # noqa
        return True
    except ImportError:
        pass
    try:
        import antenv
        sys.path.insert(0, '/root/.axon_site')
        from trn_agent_boot.trn_boot import _ntff_profile_via_ctypes
        hook = _ntff_profile_via_ctypes('/opt/axon/libaxon_pjrt.so')
        if hook is None:
            return False
        mod = types.ModuleType('antenv.axon_hooks')
        _state = {'hook': hook}
        mod.set_axon_ntff_profile_hook = lambda h: _state.__setitem__('hook', h)
        mod.get_axon_ntff_profile_hook = lambda: _state['hook']
        sys.modules['antenv.axon_hooks'] = mod
        antenv.axon_hooks = mod
        return True
    except Exception as e:
        print(f"ntff hook setup failed: {e}")
        return False


def kernel(**inputs):
    if "ncs" not in _cache:
        _cache["ncs"] = [_build(qc) for qc in range(4)]
    ncs = _cache["ncs"]
    in_maps = _prep_inputs(inputs)

    trace = bool(int(os.environ.get("KERNEL_TRACE", "0")))
    if trace and _ensure_ntff_hook():
        import tempfile
        from antenv.axon_hooks import get_axon_ntff_profile_hook
        hook = get_axon_ntff_profile_hook()
        tmpdir = tempfile.mkdtemp()
        _run_multi(ncs, in_maps)  # warm-up/compile outside the profile window
        with hook(tmpdir, list(range(8))):
            results = _run_multi(ncs, in_maps)
        _cache["ntff_dir"] = tmpdir
        print(f"ntff dir: {tmpdir}")
    else:
        results = _run_multi(ncs, in_maps)
    _cache["last_results"] = results

    out = np.empty((B, S, E), dtype=np.float32)
    for c in range(8):
        b, qc = c // 4, c % 4
        o = results[c]["out"]  # [E, QL]
        for i in range(4):
            blk = qc + 4 * i
            out[b, blk * P:(blk + 1) * P, :] = o[:, i * P:(i + 1) * P].T
    return out
